# revision 1
# baseline (speedup 1.0000x reference)
"""RWKV-4 block (TimeMix WKV recurrence + ChannelMix) on 8 Trainium2 NeuronCores.

Sharding: 8 cores = 4 batch elements x 2 halves of T (1024 output rows each).
All compute is row-local except the WKV scan carry state, which is passed
between half-pairs with one tiny AllGather ([128, 2*DG] fp32 per core).

Device layout: channels-on-partitions [c, t].  The host pre-transposes x and
all weight matrices so every DMA is contiguous.  WKV runs as a hardware
tensor_tensor_scan (state = ew*state + x) per 128-channel group.  Large
intermediates (ek, ek*v, sigmoid(r), x2, sigmoid(r2)) are spilled to DRAM and
streamed back so SBUF tile-pool lifetimes nest (stack allocator).
"""

import os
import sys

import numpy as np

for _p in ("/opt/trn_rl_repo", "/root/.axon_site/_ro/trn_rl_repo"):
    if os.path.isdir(_p) and _p not in sys.path:
        sys.path.insert(0, _p)

import ml_dtypes  # noqa: E402

import concourse.bass as bass  # noqa: E402,F401
import concourse.mybir as mybir  # noqa: E402
import concourse.tile as tile  # noqa: E402
from concourse import bacc  # noqa: E402
from concourse.bass_utils import run_bass_kernel_spmd  # noqa: E402

F32 = mybir.dt.float32
F32R = mybir.dt.float32r
BF16 = mybir.dt.bfloat16
ALU = mybir.AluOpType
ACT = mybir.ActivationFunctionType

B, T, C, D_ATT, D_FFN = 4, 2048, 2048, 2048, 8192
EPS = 1e-5
N_CORES = 8
DEN_EPS = 1e-30  # keeps Den finite on the virtual row of first-half cores


def _splits(total, sz):
    return [(s, min(sz, total - s)) for s in range(0, total, sz)]


def _even_splits(total, mx):
    n = -(-total // mx)
    base, rem = divmod(total, n)
    out, s = [], 0
    for i in range(n):
        sz = base + (1 if i < rem else 0)
        out.append((s, sz))
        s += sz
    return out


def build_program(Cc=C, Dd=D_ATT, Ff=D_FFN, rows_out=T // 2, n_cores=N_CORES,
                  mm_dt=BF16, no_collective=False):
    """Build the (uniform SPMD) Bass program."""
    P = 128
    CG, DG, FG = Cc // P, Dd // P, Ff // P
    RO = rows_out              # output rows per core
    RS = RO + 1                # scan rows (one leading row)
    R = RS + 1                 # loaded x rows (two leading rows)
    NV = 11

    nc = bacc.Bacc("TRN2", target_bir_lowering=False, debug=False,
                   num_devices=n_cores)

    xT = nc.dram_tensor("xT", [Cc, R], F32, kind="ExternalInput").ap()
    wk = nc.dram_tensor("wk", [P, CG, Dd], mm_dt, kind="ExternalInput").ap()
    wv = nc.dram_tensor("wv", [P, CG, Dd], mm_dt, kind="ExternalInput").ap()
    wr = nc.dram_tensor("wr", [P, CG, Dd], mm_dt, kind="ExternalInput").ap()
    wo = nc.dram_tensor("wo", [P, DG, Cc], mm_dt, kind="ExternalInput").ap()
    wck = nc.dram_tensor("wck", [P, CG, Ff], mm_dt, kind="ExternalInput").ap()
    wcv = nc.dram_tensor("wcv", [P, FG, Cc], mm_dt, kind="ExternalInput").ap()
    wcr = nc.dram_tensor("wcr", [P, CG, Cc], mm_dt, kind="ExternalInput").ap()
    cvec = nc.dram_tensor("cvec", [P, CG, NV], F32, kind="ExternalInput").ap()
    m0d = nc.dram_tensor("m0", [P, 1], F32, kind="ExternalInput").ap()
    seld = nc.dram_tensor("sel", [P, n_cores], F32, kind="ExternalInput").ap()
    outT = nc.dram_tensor("outT", [Cc, RO], F32, kind="ExternalOutput").ap()

    xTv = xT.rearrange("(g p) r -> p g r", p=P)
    outTv = outT.rearrange("(g p) r -> p g r", p=P)

    I_LN1W, I_LN1B, I_TMK, I_TMV, I_TMR, I_EW, I_EU, I_LN2W, I_LN2B, \
        I_CMK, I_CMR = range(NV)

    TS = 512                 # matmul moving free-dim tile
    LTS = min(256, RS)       # layernorm streaming tile

    with tile.TileContext(nc) as tc:
        const = tc.alloc_tile_pool(name="const", bufs=1)
        con = const.tile([P, CG, NV], F32, tag="con")
        nc.sync.dma_start(out=con[:], in_=cvec)
        m0 = const.tile([P, 1], F32, tag="m0")
        nc.sync.dma_start(out=m0[:], in_=m0d)
        selt = const.tile([P, n_cores], F32, tag="sel")
        nc.sync.dma_start(out=selt[:], in_=seld)
        onesc = const.tile([P, 1], F32, tag="ones")
        nc.vector.memset(onesc[:], 1.0)
        onesb = const.tile([P, 1], BF16, tag="onesb")
        nc.vector.memset(onesb[:], 1.0)
        epsc = const.tile([1, 1], F32, tag="epsc")
        nc.vector.memset(epsc[:], EPS)
        onesP = const.tile([1, P], F32, tag="onesP")
        nc.vector.memset(onesP[:], 1.0)

        def ccol(g, i):
            return con[:, g, i:i + 1]

        dram = tc.alloc_tile_pool(name="dram", bufs=1, space="DRAM")
        ekdram = dram.tile([Dd, RS], BF16)
        ekdv = ekdram.rearrange("(g p) r -> p g r", p=P)
        xkvdram = dram.tile([Dd, RS], BF16)
        xkvdv = xkvdram.rearrange("(g p) r -> p g r", p=P)
        srdram = dram.tile([Dd, RS], BF16)
        srdv = srdram.rearrange("(g p) r -> p g r", p=P)
        x2dram = dram.tile([Cc, RS], F32)
        x2dv = x2dram.rearrange("(g p) r -> p g r", p=P)
        sgdram = dram.tile([Cc, RO], BF16)
        sgdv = sgdram.rearrange("(g p) r -> p g r", p=P)
        cc_in = dram.tile([P, 2 * DG], F32)
        cc_out = dram.tile([P * n_cores, 2 * DG], F32)

        # ---- LayerNorm over partition-dim channels, streaming from DRAM ----
        def ln_stream(src_v, nrows, iw, out_sb, name):
            """src_v: DRAM view [P, CG, nrows] fp32 -> out_sb [P,CG,nrows] bf16."""
            st = tc.alloc_tile_pool(name=f"{name}_st", bufs=1)
            sp = tc.alloc_tile_pool(name=f"{name}_sp", bufs=2)
            psum = tc.alloc_tile_pool(name=f"{name}_ps", bufs=2, space="PSUM")
            ssum = st.tile([1, nrows], F32, tag="sum", name="ssum")
            ssq = st.tile([1, nrows], F32, tag="sq", name="ssq")
            for t0, tsz in _splits(nrows, LTS):
                xls = sp.tile([P, CG, LTS], F32, tag="xls", name="xls")
                nc.sync.dma_start(out=xls[:, :, :tsz],
                                  in_=src_v[:, :, t0:t0 + tsz])
                xsq = sp.tile([P, CG, LTS], BF16, tag="lnsq", name="xsq")
                nc.scalar.activation(xsq[:, :, :tsz], xls[:, :, :tsz],
                                     ACT.Square)
                xbf = sp.tile([P, CG, LTS], BF16, tag="lnbf", name="xbf")
                nc.vector.tensor_copy(out=xbf[:, :, :tsz],
                                      in_=xls[:, :, :tsz])
                ps = psum.tile([1, LTS], F32, tag="ln_ps", name="ps")
                ps2 = psum.tile([1, LTS], F32, tag="ln_ps2", name="ps2")
                for g in range(CG):
                    nc.tensor.matmul(
                        ps[:, :tsz], onesb[:], xbf[:, g, :tsz],
                        start=(g == 0), stop=(g == CG - 1))
                    nc.tensor.matmul(
                        ps2[:, :tsz], onesb[:], xsq[:, g, :tsz],
                        start=(g == 0), stop=(g == CG - 1))
                nc.vector.tensor_copy(out=ssum[:, t0:t0 + tsz],
                                      in_=ps[:, :tsz])
                nc.vector.tensor_copy(out=ssq[:, t0:t0 + tsz],
                                      in_=ps2[:, :tsz])
            mu = st.tile([1, nrows], F32, tag="mu", name="mu")
            rstd = st.tile([1, nrows], F32, tag="rstd", name="rstd")
            var = st.tile([1, nrows], F32, tag="var", name="var")
            musq = st.tile([1, nrows], F32, tag="musq", name="musq")
            nc.vector.tensor_scalar_mul(mu[:], ssum[:], 1.0 / Cc)
            nc.vector.tensor_scalar_mul(var[:], ssq[:], 1.0 / Cc)
            nc.vector.tensor_tensor(musq[:], mu[:], mu[:], ALU.mult)
            nc.vector.tensor_tensor(var[:], var[:], musq[:], ALU.subtract)
            nc.scalar.activation(var[:], var[:], ACT.Ln, bias=epsc[:])
            nc.scalar.activation(rstd[:], var[:], ACT.Exp, scale=-0.5)
            for t0, tsz in _splits(nrows, LTS):
                xls = sp.tile([P, CG, LTS], F32, tag="xls", name="xls")
                nc.sync.dma_start(out=xls[:, :, :tsz],
                                  in_=src_v[:, :, t0:t0 + tsz])
                # broadcast per-row stats to all 128 partitions via K=1 matmul
                mups = psum.tile([P, LTS], F32, tag="mups", name="mups")
                nc.tensor.matmul(mups[:, :tsz], onesP[:],
                                 mu[:, t0:t0 + tsz],
                                 start=True, stop=True)
                rsps = psum.tile([P, LTS], F32, tag="rsps", name="rsps")
                nc.tensor.matmul(rsps[:, :tsz], onesP[:],
                                 rstd[:, t0:t0 + tsz],
                                 start=True, stop=True)
                for g in range(CG):
                    xm = sp.tile([P, LTS], F32, tag="ln_xm", name="xm")
                    nc.vector.tensor_tensor(xm[:, :tsz], xls[:, g, :tsz],
                                            mups[:, :tsz], ALU.subtract)
                    nc.vector.scalar_tensor_tensor(
                        out_sb[:, g, t0:t0 + tsz], xm[:, :tsz], ccol(g, iw),
                        rsps[:, :tsz], ALU.mult, ALU.mult)
            for p in (psum, sp, st):
                p.release()

        # ================= Phase A: LN1 =================
        pHs = tc.alloc_tile_pool(name="pHs", bufs=1)
        hs = pHs.tile([P, CG, R], BF16, tag="hs")
        ln_stream(xTv, R, I_LN1W, hs, "ln1")
        # zero the two lead rows on first-half cores (time_shift zero pad)
        nc.vector.tensor_scalar_mul(hs[:, :, 0:2], hs[:, :, 0:2], m0[:])

        # ============ Phase B: mixes + k/v/r matmuls ============
        pMix = tc.alloc_tile_pool(name="pMix", bufs=2)
        wpB = tc.alloc_tile_pool(name="wpB", bufs=2)
        stg = tc.alloc_tile_pool(name="stg", bufs=3)
        psB = tc.alloc_tile_pool(name="psB", bufs=4, space="PSUM")

        DBLK = min(512, Dd)

        def make_mix(icoef):
            mix = pMix.tile([P, CG, RS], BF16, tag="mix", name="mix")
            for g in range(CG):
                dmix = stg.tile([P, RS], BF16, tag="dmix", name="dmix")
                nc.vector.tensor_tensor(dmix[:], hs[:, g, 1:R],
                                        hs[:, g, 0:RS], ALU.subtract)
                nc.vector.scalar_tensor_tensor(
                    mix[:, g, :], dmix[:], ccol(g, icoef), hs[:, g, 0:RS],
                    ALU.mult, ALU.add)
            return mix

        def mm_phase(wdram, rhs, n_out, nrows, evict):
            for d0, dsz in _splits(n_out, DBLK):
                wbuf = wpB.tile([P, CG, DBLK], mm_dt, tag="w3", name="w3")
                nc.sync.dma_start(out=wbuf[:, :, :dsz],
                                  in_=wdram[:, :, d0:d0 + dsz])
                for gl in range(dsz // P):
                    g_out = (d0 + gl * P) // P
                    for t0, tsz in _even_splits(nrows, TS):
                        ps = psB.tile([P, TS], F32, tag="mm_ps", name="mm_ps")
                        for gi in range(CG):
                            nc.tensor.matmul(
                                ps[:, :tsz],
                                wbuf[:, gi, gl * P:(gl + 1) * P],
                                rhs[:, gi, t0:t0 + tsz],
                                start=(gi == 0), stop=(gi == CG - 1))
                        evict(g_out, t0, tsz, ps)

        def evict_k(g, t0, tsz, ps):
            est = stg.tile([P, TS], BF16, tag="est", name="est")
            nc.scalar.activation(est[:, :tsz], ps[:, :tsz], ACT.Exp)
            if t0 == 0:  # mask the virtual lead row on first-half cores
                nc.vector.tensor_scalar_mul(est[:, 0:1], est[:, 0:1], m0[:])
            nc.sync.dma_start(out=ekdv[:, g, t0:t0 + tsz], in_=est[:, :tsz])

        def evict_v(g, t0, tsz, ps):
            eld = stg.tile([P, TS], BF16, tag="eld", name="eld")
            nc.sync.dma_start(out=eld[:, :tsz], in_=ekdv[:, g, t0:t0 + tsz])
            xst = stg.tile([P, TS], BF16, tag="xst", name="xst")
            nc.vector.tensor_tensor(xst[:, :tsz], eld[:, :tsz], ps[:, :tsz],
                                    ALU.mult)
            nc.sync.dma_start(out=xkvdv[:, g, t0:t0 + tsz], in_=xst[:, :tsz])

        def evict_r(g, t0, tsz, ps):
            srt = stg.tile([P, TS], BF16, tag="srt", name="srt")
            nc.scalar.activation(srt[:, :tsz], ps[:, :tsz], ACT.Sigmoid)
            nc.sync.dma_start(out=srdv[:, g, t0:t0 + tsz], in_=srt[:, :tsz])

        mixk = make_mix(I_TMK)
        mm_phase(wk, mixk, Dd, RS, evict_k)
        mixv = make_mix(I_TMV)
        mm_phase(wv, mixv, Dd, RS, evict_v)
        mixr = make_mix(I_TMR)
        mm_phase(wr, mixr, Dd, RS, evict_r)

        psB.release()
        stg.release()
        wpB.release()
        pMix.release()
        pHs.release()

        # ============ Phase C: boundary states + AllGather ============
        # Right-side pool: C's DVE scans overlap phase B's matmuls without
        # waiting on B's pool-zone releases.
        pC = tc.alloc_tile_pool(name="pC", bufs=2, side="right")

        state = pC.tile([P, 2 * DG], F32, tag="state", name="state")
        for g in range(DG):
            ekg = pC.tile([P, RS], BF16, tag="ekg", name="ekg")
            nc.sync.dma_start(out=ekg[:], in_=ekdv[:, g, :])
            xkg = pC.tile([P, RS], BF16, tag="xkg", name="xkg")
            nc.sync.dma_start(out=xkg[:], in_=xkvdv[:, g, :])
            ewbc = ccol(g, I_EW).to_broadcast([P, RS - 1])
            apre = pC.tile([P, RS - 1], F32, tag="apre", name="apre")
            nc.vector.tensor_tensor_scan(
                apre[:], ewbc, xkg[:, :RS - 1], 0.0, ALU.mult, ALU.add)
            nc.gpsimd.tensor_copy(out=state[:, g:g + 1],
                                  in_=apre[:, RS - 2:RS - 1])
            bpre = pC.tile([P, RS - 1], F32, tag="bpre", name="bpre")
            nc.vector.tensor_tensor_scan(
                bpre[:], ewbc, ekg[:, :RS - 1], 0.0, ALU.mult, ALU.add)
            nc.gpsimd.tensor_copy(out=state[:, DG + g:DG + g + 1],
                                  in_=bpre[:, RS - 2:RS - 1])
        nc.sync.dma_start(out=cc_in[:], in_=state[:])
        if not no_collective:
            nc.gpsimd.collective_compute(
                "AllGather", ALU.bypass,
                replica_groups=[list(range(n_cores))],
                ins=[cc_in[:].opt()], outs=[cc_out[:].opt()])
        else:  # timing-equivalent stand-in for TimelineSim profiling
            for jj in range(n_cores):
                nc.sync.dma_start(out=cc_out[jj * P:(jj + 1) * P, :],
                                  in_=cc_in[:])
        gsb = pC.tile([P, n_cores, 2 * DG], F32, tag="gsb", name="gsb")
        nc.sync.dma_start(
            out=gsb[:], in_=cc_out[:].rearrange("(j p) s -> p j s", p=P))
        a0b0 = pC.tile([P, 2 * DG], F32, tag="a0b0", name="a0b0")
        nc.vector.memset(a0b0[:, 0:DG], 0.0)
        nc.vector.memset(a0b0[:, DG:2 * DG], DEN_EPS)
        for j in range(n_cores):
            nc.vector.scalar_tensor_tensor(
                a0b0[:], gsb[:, j, :], selt[:, j:j + 1], a0b0[:],
                ALU.mult, ALU.add)

        # ============ Phase D: WKV scans + rwkv ============
        pRw = tc.alloc_tile_pool(name="pRw", bufs=1)
        rwkv = pRw.tile([P, DG, RS], BF16, tag="rwkv")
        pD = tc.alloc_tile_pool(name="pD", bufs=2)
        for g in range(DG):
            ekg = pD.tile([P, RS], BF16, tag="ekg", name="ekg")
            nc.sync.dma_start(out=ekg[:], in_=ekdv[:, g, :])
            xkg = pD.tile([P, RS], BF16, tag="xkg", name="xkg")
            nc.sync.dma_start(out=xkg[:], in_=xkvdv[:, g, :])
            srg = pD.tile([P, RS], BF16, tag="srg", name="srg")
            nc.sync.dma_start(out=srg[:], in_=srdv[:, g, :])
            ewbd = ccol(g, I_EW).to_broadcast([P, RS])
            abuf = pD.tile([P, RS + 1], F32, tag="abuf", name="abuf")
            nc.gpsimd.tensor_copy(out=abuf[:, 0:1], in_=a0b0[:, g:g + 1])
            nc.vector.tensor_tensor_scan(
                abuf[:, 1:RS + 1], ewbd, xkg[:], a0b0[:, g:g + 1],
                ALU.mult, ALU.add)
            bbuf = pD.tile([P, RS + 1], F32, tag="bbuf", name="bbuf")
            nc.gpsimd.tensor_copy(out=bbuf[:, 0:1],
                                  in_=a0b0[:, DG + g:DG + g + 1])
            nc.vector.tensor_tensor_scan(
                bbuf[:, 1:RS + 1], ewbd, ekg[:],
                a0b0[:, DG + g:DG + g + 1], ALU.mult, ALU.add)
            num = pD.tile([P, RS], F32, tag="num", name="num")
            nc.vector.scalar_tensor_tensor(
                num[:], xkg[:], ccol(g, I_EU), abuf[:, 0:RS],
                ALU.mult, ALU.add)
            den = pD.tile([P, RS], F32, tag="den", name="den")
            nc.vector.scalar_tensor_tensor(
                den[:], ekg[:], ccol(g, I_EU), bbuf[:, 0:RS],
                ALU.mult, ALU.add)
            rden = pD.tile([P, RS], F32, tag="rden", name="rden")
            nc.vector.reciprocal_approx_fast(out=rden[:], in_=den[:])
            nc.gpsimd.tensor_tensor(num[:], num[:], rden[:], ALU.mult)
            nc.gpsimd.tensor_tensor(rwkv[:, g, :], num[:], srg[:], ALU.mult)
        pD.release()

        # ============ Phase E: Wo matmul -> x2 (to DRAM) ============
        wpE = tc.alloc_tile_pool(name="wpE", bufs=2, side="right")
        spE = tc.alloc_tile_pool(name="spE", bufs=3, side="right")
        psE = tc.alloc_tile_pool(name="psE", bufs=2, space="PSUM")

        CBLK = min(512, Cc)
        for c0, csz in _splits(Cc, CBLK):
            wbuf = wpE.tile([P, DG, CBLK], mm_dt, tag="wo", name="wo")
            nc.sync.dma_start(out=wbuf[:, :, :csz], in_=wo[:, :, c0:c0 + csz])
            for gl in range(csz // P):
                g_c = (c0 + gl * P) // P
                for t0, tsz in _even_splits(RS, TS):
                    ps = psE.tile([P, TS], F32, tag="wo_ps", name="wo_ps")
                    for gi in range(DG):
                        nc.tensor.matmul(
                            ps[:, :tsz], wbuf[:, gi, gl * P:(gl + 1) * P],
                            rwkv[:, gi, t0:t0 + tsz],
                            start=(gi == 0), stop=(gi == DG - 1))
                    xst = spE.tile([P, TS], F32, tag="xst", name="xst")
                    nc.sync.dma_start(
                        out=xst[:, :tsz],
                        in_=xTv[:, g_c, 1 + t0:1 + t0 + tsz])
                    x2st = spE.tile([P, TS], F32, tag="x2st", name="x2st")
                    nc.vector.tensor_tensor(x2st[:, :tsz], xst[:, :tsz],
                                            ps[:, :tsz], ALU.add)
                    nc.sync.dma_start(out=x2dv[:, g_c, t0:t0 + tsz],
                                      in_=x2st[:, :tsz])
        psE.release()
        spE.release()
        wpE.release()
        pC.release()
        pRw.release()

        # ============ Phase F: LN2 + mixes2 ============
        pMx2 = tc.alloc_tile_pool(name="pMx2", bufs=1)
        pXr2 = tc.alloc_tile_pool(name="pXr2", bufs=1)
        pG2 = tc.alloc_tile_pool(name="pG2", bufs=1)
        xk2 = pMx2.tile([P, CG, RO], BF16, tag="xk2")
        xr2 = pXr2.tile([P, CG, RO], BF16, tag="xr2")
        g2 = pG2.tile([P, CG, RS], BF16, tag="g2")
        ln_stream(x2dv, RS, I_LN2W, g2, "ln2")
        nc.vector.tensor_scalar_mul(g2[:, :, 0:1], g2[:, :, 0:1], m0[:])

        spF = tc.alloc_tile_pool(name="spF", bufs=2)
        for g in range(CG):
            dmix = spF.tile([P, RO], BF16, tag="dmix2", name="dmix2")
            nc.vector.tensor_tensor(dmix[:], g2[:, g, 1:RS], g2[:, g, 0:RO],
                                    ALU.subtract)
            nc.vector.scalar_tensor_tensor(
                xk2[:, g, :], dmix[:], ccol(g, I_CMK), g2[:, g, 0:RO],
                ALU.mult, ALU.add)
            nc.vector.scalar_tensor_tensor(
                xr2[:, g, :], dmix[:], ccol(g, I_CMR), g2[:, g, 0:RO],
                ALU.mult, ALU.add)
        spF.release()
        pG2.release()

        # ============ Phase G: r2 = sigmoid(xr2 @ WcrT) -> DRAM ============
        wpG = tc.alloc_tile_pool(name="wpG", bufs=2)
        spG = tc.alloc_tile_pool(name="spG", bufs=2)
        psG = tc.alloc_tile_pool(name="psG", bufs=3, space="PSUM")
        for c0, csz in _splits(Cc, CBLK):
            wbuf = wpG.tile([P, CG, CBLK], mm_dt, tag="wcr", name="wcr")
            nc.sync.dma_start(out=wbuf[:, :, :csz], in_=wcr[:, :, c0:c0 + csz])
            for gl in range(csz // P):
                g_c = (c0 + gl * P) // P
                for t0, tsz in _splits(RO, TS):
                    ps = psG.tile([P, TS], F32, tag="wcr_ps", name="wcr_ps")
                    for gi in range(CG):
                        nc.tensor.matmul(
                            ps[:, :tsz], wbuf[:, gi, gl * P:(gl + 1) * P],
                            xr2[:, gi, t0:t0 + tsz],
                            start=(gi == 0), stop=(gi == CG - 1))
                    sgt = spG.tile([P, TS], BF16, tag="sgt", name="sgt")
                    nc.scalar.activation(sgt[:, :tsz], ps[:, :tsz],
                                         ACT.Sigmoid)
                    nc.sync.dma_start(out=sgdv[:, g_c, t0:t0 + tsz],
                                      in_=sgt[:, :tsz])
        psG.release()
        spG.release()
        wpG.release()
        pXr2.release()

        # ============ Phase H: FFN ============
        FBLK = min(512, Ff)
        FQ = 16 if FG >= 16 else FG
        for t0, tsz in _splits(RO, TS):
            pH = tc.alloc_tile_pool(name=f"pH{t0}", bufs=1)
            wpH = tc.alloc_tile_pool(name=f"wpH{t0}", bufs=2)
            psH = tc.alloc_tile_pool(name=f"psH{t0}", bufs=3, space="PSUM")
            psKV = tc.alloc_tile_pool(name=f"psKV{t0}", bufs=1, space="PSUM")
            kfsq = pH.tile([P, FG, TS], BF16, tag="kfsq", name="kfsq")
            # FFN1: kf = relu(xk2 @ WckT)^2
            for f0, fsz in _splits(Ff, FBLK):
                wbuf = wpH.tile([P, CG, FBLK], mm_dt, tag="wf", name="wf")
                nc.sync.dma_start(out=wbuf[:, :, :fsz],
                                  in_=wck[:, :, f0:f0 + fsz])
                for fl in range(fsz // P):
                    g_f = (f0 + fl * P) // P
                    ps = psH.tile([P, TS], F32, tag="ffn1_ps", name="ffn1_ps")
                    for gi in range(CG):
                        nc.tensor.matmul(
                            ps[:, :tsz], wbuf[:, gi, fl * P:(fl + 1) * P],
                            xk2[:, gi, t0:t0 + tsz],
                            start=(gi == 0), stop=(gi == CG - 1))
                    nc.scalar.activation(kfsq[:, g_f, :tsz], ps[:, :tsz],
                                         ACT.Relu)
                    nc.vector.tensor_tensor(kfsq[:, g_f, :tsz],
                                            kfsq[:, g_f, :tsz],
                                            kfsq[:, g_f, :tsz], ALU.mult)
            # FFN2 + final: out = x2 + sg * (kfsq @ WcvT)
            for c0, csz in _splits(Cc, CBLK):
                kvps = [psKV.tile([P, TS], F32, tag=f"kv_ps{i}",
                                  name=f"kv_ps{i}")
                        for i in range(csz // P)]
                nq = (FG + FQ - 1) // FQ
                for q in range(nq):
                    f_lo = q * FQ
                    f_n = min(FQ, FG - f_lo)
                    wbuf = wpH.tile([P, FQ, CBLK], mm_dt, tag="wf2",
                                    name="wf2")
                    nc.sync.dma_start(
                        out=wbuf[:, :f_n, :csz],
                        in_=wcv[:, f_lo:f_lo + f_n, c0:c0 + csz])
                    for gl in range(csz // P):
                        for fi in range(f_n):
                            nc.tensor.matmul(
                                kvps[gl][:, :tsz],
                                wbuf[:, fi, gl * P:(gl + 1) * P],
                                kfsq[:, f_lo + fi, :tsz],
                                start=(q == 0 and fi == 0),
                                stop=(q == nq - 1 and fi == f_n - 1))
                for gl in range(csz // P):
                    g_c = (c0 + gl * P) // P
                    x2s = wpH.tile([P, TS], F32, tag="x2s", name="x2s")
                    nc.sync.dma_start(
                        out=x2s[:, :tsz],
                        in_=x2dv[:, g_c, 1 + t0:1 + t0 + tsz])
                    sgs = wpH.tile([P, TS], BF16, tag="sgs", name="sgs")
                    nc.sync.dma_start(out=sgs[:, :tsz],
                                      in_=sgdv[:, g_c, t0:t0 + tsz])
                    ot = wpH.tile([P, TS], F32, tag="ot", name="ot")
                    nc.vector.tensor_tensor(ot[:, :tsz], sgs[:, :tsz],
                                            kvps[gl][:, :tsz], ALU.mult)
                    nc.vector.tensor_tensor(ot[:, :tsz], ot[:, :tsz],
                                            x2s[:, :tsz], ALU.add)
                    nc.sync.dma_start(out=outTv[:, g_c, t0:t0 + tsz],
                                      in_=ot[:, :tsz])
            for p in (psKV, psH, wpH, pH):
                p.release()
        pMx2.release()
        dram.release()
        const.release()

    nc.compile()
    return nc


_PROGRAM_CACHE = {}


def _get_program(key, **kw):
    if key not in _PROGRAM_CACHE:
        _PROGRAM_CACHE[key] = build_program(**kw)
    return _PROGRAM_CACHE[key]


def _host_prep(inputs, Cc=C, Dd=D_ATT, Ff=D_FFN, Bb=B, Tt=T, n_cores=N_CORES):
    """Build per-core input maps (numpy only)."""
    P = 128
    CG, DG, FG = Cc // P, Dd // P, Ff // P
    half = Tt // 2
    RO, RS, R = half, half + 1, half + 2
    bf = ml_dtypes.bfloat16

    f = {k: np.asarray(v, np.float32) for k, v in inputs.items()}
    x = f["x"]

    def swz(wT, kg):  # [K, N] -> [128, kg, N] with [p, gi, n] = wT[gi*128+p, n]
        Kdim, Ndim = wT.shape
        return np.ascontiguousarray(
            wT.reshape(kg, P, Ndim).transpose(1, 0, 2)).astype(bf)

    wk_h = swz(f["Wk"].T, CG)
    wv_h = swz(f["Wv"].T, CG)
    wr_h = swz(f["Wr"].T, CG)
    wo_h = swz(f["Wo"].T, DG)
    wck_h = swz(f["Wck"].T, CG)
    wcv_h = swz(f["Wcv"].T, FG)
    wcr_h = swz(f["Wcr"].T, CG)

    def col(v):  # [C] -> [128, CG]
        return np.ascontiguousarray(
            np.asarray(v, np.float32).reshape(-1).reshape(CG, P).T)

    ew = np.exp(-np.exp(f["time_decay"].astype(np.float64)))
    cvec_h = np.stack([
        col(f["ln1_w"]), col(f["ln1_b"]),
        col(f["tm_k"]), col(f["tm_v"]), col(f["tm_r"]),
        col(ew.astype(np.float32)), col(np.exp(f["time_first"])),
        col(f["ln2_w"]), col(f["ln2_b"]),
        col(f["cm_k"]), col(f["cm_r"]),
    ], axis=-1).astype(np.float32)  # [128, CG, 11]

    in_maps = []
    for core in range(n_cores):
        b, hh = core // 2, core % 2
        t0 = hh * half
        xr = np.zeros((R, Cc), np.float32)
        lo = t0 - 2
        src_lo = max(lo, 0)
        xr[src_lo - lo:, :] = x[b, src_lo:t0 + RO, :]
        m0 = np.full((P, 1), float(hh), np.float32)
        sel = np.zeros((P, n_cores), np.float32)
        if hh == 1:
            sel[:, core - 1] = 1.0
        in_maps.append({
            "xT": np.ascontiguousarray(xr.T),
            "wk": wk_h, "wv": wv_h, "wr": wr_h, "wo": wo_h,
            "wck": wck_h, "wcv": wcv_h, "wcr": wcr_h,
            "cvec": cvec_h, "m0": m0, "sel": sel,
        })
    return in_maps


def kernel(**inputs):
    in_maps = _host_prep(inputs)
    nc = _get_program("full")
    res = run_bass_kernel_spmd(nc, in_maps, core_ids=list(range(N_CORES)))
    half = T // 2
    out = np.empty((B, T, C), np.float32)
    for core in range(N_CORES):
        b, hh = core // 2, core % 2
        out[b, hh * half:(hh + 1) * half, :] = res.results[core]["outT"].T
    return out



# revision 5
# speedup vs baseline: 1.3702x; 1.3702x over previous
"""RWKV-4 block on 8 trn2 cores — fp8e4 DoubleRow version.

Sharding: 8 cores = 4 batch x 2 T-halves (as baseline). All big matmuls run
as fp8e4 DoubleRow (K=256/instr, 0.5 cyc/row). Precision scheme (measured
offline: final rel err ~1.2e-2 vs the 2e-2 gate):
  Wk, Wr, Wcr: pure fp8 (weights e4m3 x64, acts e4m3 x16)
  Wv, Wo:      2-term (weight hi+lo at the same scale; lo rides subnormals)
  Wck, Wcv:    3-term (weight hi+lo AND activation hi+lo, same scale)
Same-scale lo parts make every term share one PSUM scale, so all terms
accumulate natively in PSUM with no combine ops.
"""

import os
import sys

import numpy as np

for _p in ("/opt/trn_rl_repo", "/root/.axon_site/_ro/trn_rl_repo"):
    if os.path.isdir(_p) and _p not in sys.path:
        sys.path.insert(0, _p)

import ml_dtypes  # noqa: E402

import concourse.bass as bass  # noqa: E402,F401
import concourse.mybir as mybir  # noqa: E402
import concourse.tile as tile  # noqa: E402
from concourse import bacc  # noqa: E402
from concourse.bass_utils import run_bass_kernel_spmd  # noqa: E402

F32 = mybir.dt.float32
F32R = mybir.dt.float32r
BF16 = mybir.dt.bfloat16
F8 = mybir.dt.float8e4
ALU = mybir.AluOpType
ACT = mybir.ActivationFunctionType
DR = mybir.MatmulPerfMode.DoubleRow
E4M3 = ml_dtypes.float8_e4m3

B, T, C, D_ATT, D_FFN = 4, 2048, 2048, 2048, 8192
EPS = 1e-5
N_CORES = 8
DEN_EPS = 1e-30

SA = 16.0          # activation fp8 scale
SW = 64.0          # weight fp8 scale
SKF = 8.0          # kf fp8 scale
PS_INV = 1.0 / (SA * SW)     # psum -> true scale (2^-10)
SQ8 = float(np.sqrt(SKF))


def _splits(total, sz):
    return [(s, min(sz, total - s)) for s in range(0, total, sz)]


def _even_splits(total, mx):
    n = -(-total // mx)
    base, rem = divmod(total, n)
    out, s = [], 0
    for i in range(n):
        sz = base + (1 if i < rem else 0)
        out.append((s, sz))
        s += sz
    return out


def build_program(Cc=C, Dd=D_ATT, Ff=D_FFN, rows_out=T // 2, n_cores=N_CORES,
                  no_collective=False):
    P = 128
    CG, DG, FG = Cc // P, Dd // P, Ff // P
    RO = rows_out
    RS = RO + 1
    R = RS + 1
    RSP = -(-RS // 16) * 16   # fp8 moving tiles padded: pair stride %16 == 0
    NV = 11

    nc = bacc.Bacc("TRN2", target_bir_lowering=False, debug=False,
                   num_devices=n_cores)

    xT = nc.dram_tensor("xT", [Cc, R], F32, kind="ExternalInput").ap()
    xTb = nc.dram_tensor("xTb", [Cc, R], BF16, kind="ExternalInput").ap()
    wkh = nc.dram_tensor("wkh", [P, CG, Dd], F8, kind="ExternalInput").ap()
    wvh = nc.dram_tensor("wvh", [P, CG, Dd], F8, kind="ExternalInput").ap()
    wvl = nc.dram_tensor("wvl", [P, CG, Dd], F8, kind="ExternalInput").ap()
    wrh = nc.dram_tensor("wrh", [P, CG, Dd], F8, kind="ExternalInput").ap()
    woh = nc.dram_tensor("woh", [P, DG, Cc], F8, kind="ExternalInput").ap()
    wol = nc.dram_tensor("wol", [P, DG, Cc], F8, kind="ExternalInput").ap()
    wckh = nc.dram_tensor("wckh", [P, CG, Ff], F8, kind="ExternalInput").ap()
    wckl = nc.dram_tensor("wckl", [P, CG, Ff], F8, kind="ExternalInput").ap()
    wcvh = nc.dram_tensor("wcvh", [P, FG, Cc], F8, kind="ExternalInput").ap()
    wcvl = nc.dram_tensor("wcvl", [P, FG, Cc], F8, kind="ExternalInput").ap()
    wcrh = nc.dram_tensor("wcrh", [P, CG, Cc], F8, kind="ExternalInput").ap()
    cvec = nc.dram_tensor("cvec", [P, CG, NV], F32, kind="ExternalInput").ap()
    m0d = nc.dram_tensor("m0", [P, 1], F32, kind="ExternalInput").ap()
    seld = nc.dram_tensor("sel", [P, n_cores], F32, kind="ExternalInput").ap()
    outT = nc.dram_tensor("outT", [Cc, RO], F32, kind="ExternalOutput").ap()

    xTv = xT.rearrange("(g p) r -> p g r", p=P)
    xTbv = xTb.rearrange("(g p) r -> p g r", p=P)
    outTv = outT.rearrange("(g p) r -> p g r", p=P)

    I_LN1W, I_LN1B, I_TMK, I_TMV, I_TMR, I_EW, I_EU, I_LN2W, I_LN2B, \
        I_CMK, I_CMR = range(NV)

    TS = 512
    LTS = 256

    with tile.TileContext(nc) as tc:
        const = tc.alloc_tile_pool(name="const", bufs=1)
        con = const.tile([P, CG, NV], F32, tag="con")
        nc.sync.dma_start(out=con[:], in_=cvec)
        m0 = const.tile([P, 1], F32, tag="m0")
        nc.sync.dma_start(out=m0[:], in_=m0d)
        selt = const.tile([P, n_cores], F32, tag="sel")
        nc.sync.dma_start(out=selt[:], in_=seld)
        onesc = const.tile([P, 1], F32, tag="ones")
        nc.vector.memset(onesc[:], 1.0)
        onesb = const.tile([P, 1], BF16, tag="onesb")
        nc.vector.memset(onesb[:], 1.0)
        epsc = const.tile([1, 1], F32, tag="epsc")
        nc.vector.memset(epsc[:], EPS)
        onesPb = const.tile([1, P], BF16, tag="onesPb")
        nc.vector.memset(onesPb[:], 1.0)

        def ccol(g, i):
            return con[:, g, i:i + 1]

        dram = tc.alloc_tile_pool(name="dram", bufs=1, space="DRAM")
        x2dram = dram.tile([Cc, RS], F32)
        x2dv = x2dram.rearrange("(g p) r -> p g r", p=P)
        srdram = dram.tile([Dd, RS], BF16)
        srdv = srdram.rearrange("(g p) r -> p g r", p=P)
        sgdram = dram.tile([Cc, RO], BF16)
        sgdv = sgdram.rearrange("(g p) r -> p g r", p=P)
        cc_in = dram.tile([P, 2 * DG], F32)
        cc_out = dram.tile([P * n_cores, 2 * DG], F32)

        # ---- LayerNorm (streaming; PE sums via f32r bitcast) ----
        def ln_stream(src_v, nrows, iw, out_sb, name, sbuf_src=False,
                      src_bf16=False, lts=None):
            LTS = lts or 256
            src_dt = BF16 if src_bf16 else F32
            st = tc.alloc_tile_pool(name=f"{name}_st", bufs=1)
            sp = tc.alloc_tile_pool(name=f"{name}_sp", bufs=2)
            spx = tc.alloc_tile_pool(name=f"{name}_spx", bufs=8)
            psum = tc.alloc_tile_pool(name=f"{name}_ps", bufs=2, space="PSUM")
            ssum = st.tile([1, nrows], F32, tag="sum", name="ssum")
            ssq = st.tile([1, nrows], F32, tag="sq", name="ssq")
            for t0, tsz in _splits(nrows, LTS):
                if sbuf_src:
                    xls = src_v[:, :, t0:t0 + tsz]
                else:
                    xlt = sp.tile([P, CG, LTS], src_dt, tag="xls",
                                  name="xls")
                    nc.sync.dma_start(out=xlt[:, :, :tsz],
                                      in_=src_v[:, :, t0:t0 + tsz])
                    xls = xlt[:, :, :tsz]
                xsq = sp.tile([P, CG, LTS], BF16, tag="lnsq", name="xsq")
                nc.scalar.activation(xsq[:, :, :tsz], xls,
                                     ACT.Square)
                ps = psum.tile([1, LTS], F32, tag="ln_ps", name="ps")
                ps2 = psum.tile([1, LTS], F32, tag="ln_ps2", name="ps2")
                for g in range(CG):
                    nc.tensor.matmul(
                        ps[:, :tsz], onesb[:], xls[:, g, :],
                        start=(g == 0), stop=(g == CG - 1))
                    nc.tensor.matmul(
                        ps2[:, :tsz], onesb[:], xsq[:, g, :tsz],
                        start=(g == 0), stop=(g == CG - 1))
                nc.vector.tensor_copy(out=ssum[:, t0:t0 + tsz],
                                      in_=ps[:, :tsz])
                nc.vector.tensor_copy(out=ssq[:, t0:t0 + tsz],
                                      in_=ps2[:, :tsz])
            mu = st.tile([1, nrows], BF16, tag="mu", name="mu")
            rstd = st.tile([1, nrows], BF16, tag="rstd", name="rstd")
            var = st.tile([1, nrows], F32, tag="var", name="var")
            musq = st.tile([1, nrows], F32, tag="musq", name="musq")
            nc.vector.tensor_scalar_mul(mu[:], ssum[:], 1.0 / Cc)
            nc.vector.tensor_scalar_mul(var[:], ssq[:], 1.0 / Cc)
            nc.vector.tensor_tensor(musq[:], mu[:], mu[:], ALU.mult)
            nc.vector.tensor_tensor(var[:], var[:], musq[:], ALU.subtract)
            nc.scalar.activation(var[:], var[:], ACT.Ln, bias=epsc[:])
            nc.scalar.activation(rstd[:], var[:], ACT.Exp, scale=-0.5)
            for t0, tsz in _splits(nrows, LTS):
                if sbuf_src:
                    xls = src_v[:, :, t0:t0 + tsz]
                else:
                    xlt = sp.tile([P, CG, LTS], src_dt, tag="xls",
                                  name="xls")
                    nc.sync.dma_start(out=xlt[:, :, :tsz],
                                      in_=src_v[:, :, t0:t0 + tsz])
                    xls = xlt[:, :, :tsz]
                mups = psum.tile([P, LTS], F32, tag="mups", name="mups")
                nc.tensor.matmul(mups[:, :tsz], onesPb[:],
                                 mu[:, t0:t0 + tsz],
                                 start=True, stop=True)
                rsps = psum.tile([P, LTS], F32, tag="rsps", name="rsps")
                nc.tensor.matmul(rsps[:, :tsz], onesPb[:],
                                 rstd[:, t0:t0 + tsz],
                                 start=True, stop=True)
                for g in range(CG):
                    xm = spx.tile([P, LTS], BF16, tag="ln_xm", name="xm")
                    nc.vector.tensor_tensor(xm[:, :tsz], xls[:, g, :],
                                            mups[:, :tsz], ALU.subtract)
                    nc.vector.scalar_tensor_tensor(
                        out_sb[:, g, t0:t0 + tsz], xm[:, :tsz], ccol(g, iw),
                        rsps[:, :tsz], ALU.mult, ALU.mult)
            for p in (psum, spx, sp, st):
                p.release()

        # ================= Phase A: LN1 (h = 16*ln(x), bf16) ============
        pEk = tc.alloc_tile_pool(name="pEk", bufs=1)
        eksb = [pEk.tile([P, RS], BF16, tag=f"eksb{g}", name=f"eksb{g}")
                for g in range(DG)]
        ekvsb = [pEk.tile([P, RS], BF16, tag=f"ekvsb{g}", name=f"ekvsb{g}")
                 for g in range(DG)]
        pMix = tc.alloc_tile_pool(name="pMix", bufs=1)
        mixk8 = [pMix.tile([P, 2, RSP], F8, tag=f"mixk8_{p}",
                           name=f"mixk8_{p}") for p in range(CG // 2)]
        mixv8 = [pMix.tile([P, 2, RSP], F8, tag=f"mixv8_{p}",
                           name=f"mixv8_{p}") for p in range(CG // 2)]
        mixr8 = [pMix.tile([P, 2, RSP], F8, tag=f"mixr8_{p}",
                           name=f"mixr8_{p}") for p in range(CG // 2)]
        pHs = tc.alloc_tile_pool(name="pHs", bufs=1)
        hs = pHs.tile([P, CG, R], BF16, tag="hs")
        ln_stream(xTbv, R, I_LN1W, hs, "ln1", src_bf16=True)
        nc.vector.tensor_scalar_mul(hs[:, :, 0:2], hs[:, :, 0:2], m0[:])

        # ========== Phase B: mixes (fp8 x16) + k/v/r DR matmuls ========
        stg = tc.alloc_tile_pool(name="stg", bufs=4)
        if RSP > RS:
            for mixl in (mixk8, mixv8, mixr8):
                for mt in mixl:
                    nc.vector.memset(mt[:, :, RS:RSP], 0.0)
        MSTRIPS = [(0, 512), (512, RS - 512)]
        for t0, tsz in MSTRIPS:
            for g in range(CG):
                dmix = stg.tile([P, 512 + 1], BF16, tag="dmix", name="dmix")
                nc.vector.tensor_tensor(
                    dmix[:, :tsz], hs[:, g, 1 + t0:1 + t0 + tsz],
                    hs[:, g, t0:t0 + tsz], ALU.subtract)
                for mixl, icoef, on_act in ((mixk8, I_TMK, True),
                                            (mixv8, I_TMV, False),
                                            (mixr8, I_TMR, True)):
                    mb16 = stg.tile([P, 512 + 1], BF16, tag="mb16",
                                    name="mb16")
                    nc.vector.scalar_tensor_tensor(
                        mb16[:, :tsz], dmix[:, :tsz], ccol(g, icoef),
                        hs[:, g, t0:t0 + tsz], ALU.mult, ALU.add)
                    dst = mixl[g // 2][:, g % 2, t0:t0 + tsz]
                    if on_act:
                        nc.scalar.activation(dst, mb16[:, :tsz], ACT.Copy)
                    else:
                        nc.gpsimd.tensor_copy(out=dst, in_=mb16[:, :tsz])
        wpB = tc.alloc_tile_pool(name="wpB", bufs=2)
        stgE = tc.alloc_tile_pool(name="stgE", bufs=3)
        psB = tc.alloc_tile_pool(name="psB", bufs=4, space="PSUM")
        DBLK = 512
        tstripsB = [(0, 512), (512, 512), (1024, RSP - 1024)]

        def mm_dr(whd, wld, rhs8, n_out, evict, wtag, strips=None):
            for d0, dsz in _splits(n_out, DBLK):
                wbh = wpB.tile([P, CG, DBLK], F8, tag="wh", name="wbh")
                nc.sync.dma_start(out=wbh[:, :, :dsz],
                                  in_=whd[:, :, d0:d0 + dsz])
                if wld is not None:
                    wbl = wpB.tile([P, CG, DBLK], F8, tag="wl",
                                   name="wbl")
                    nc.sync.dma_start(out=wbl[:, :, :dsz],
                                      in_=wld[:, :, d0:d0 + dsz])
                wbufs = [wbh] if wld is None else [wbh, wbl]
                for gl in range(dsz // P):
                    g_out = (d0 + gl * P) // P
                    for t0, tsz in (strips or tstripsB):
                        wsz = min(tsz, RS - t0)
                        if wsz <= 0:
                            continue
                        ps = psB.tile([P, TS], F32, tag="mm_ps", name="mm_ps")
                        nmm = len(wbufs) * (CG // 2)
                        i = 0
                        for wb in wbufs:
                            for gp in range(CG // 2):
                                nc.tensor.matmul(
                                    ps[:, :tsz],
                                    wb[:, 2 * gp:2 * gp + 2,
                                       gl * P:(gl + 1) * P],
                                    rhs8[gp][:, :, t0:t0 + tsz],
                                    start=(i == 0), stop=(i == nmm - 1),
                                    perf_mode=DR)
                                i += 1
                        evict(g_out, t0, wsz, ps)

        def evict_k(g, t0, wsz, ps):
            nc.scalar.activation(eksb[g][:, t0:t0 + wsz], ps[:, :wsz],
                                 ACT.Exp, scale=PS_INV)
            if t0 == 0:
                nc.vector.tensor_scalar_mul(eksb[g][:, 0:1], eksb[g][:, 0:1],
                                            m0[:])

        def evict_v(g, t0, wsz, ps):
            nc.vector.scalar_tensor_tensor(
                ekvsb[g][:, t0:t0 + wsz], ps[:, :wsz], PS_INV,
                eksb[g][:, t0:t0 + wsz], ALU.mult, ALU.mult)

        def evict_r(g, t0, wsz, ps):
            srt = stgE.tile([P, TS], BF16, tag="srt", name="srt")
            nc.scalar.activation(srt[:, :wsz], ps[:, :wsz], ACT.Sigmoid,
                                 scale=PS_INV)
            nc.sync.dma_start(out=srdv[:, g, t0:t0 + wsz], in_=srt[:, :wsz])

        for st_ in tstripsB:
            mm_dr(wkh, None, mixk8, Dd, evict_k, "wk", strips=[st_])
        mm_dr(wvh, wvl, mixv8, Dd, evict_v, "wv")
        mm_dr(wrh, None, mixr8, Dd, evict_r, "wr")

        psB.release()
        stgE.release()
        wpB.release()
        stg.release()
        pHs.release()
        pMix.release()

        # ====== Phase C: boundary states (bf16 scans) + AllGather =======
        pRw = tc.alloc_tile_pool(name="pRw", bufs=1, side="right")
        rwkv8 = [pRw.tile([P, 2, RSP], F8, tag=f"rw{p}", name=f"rw{p}")
                 for p in range(DG // 2)]
        if RSP > RS:
            for rwt in rwkv8:
                nc.vector.memset(rwt[:, :, RS:RSP], 0.0)
        wpE = tc.alloc_tile_pool(name="wpE", bufs=2, side="right")
        spE = tc.alloc_tile_pool(name="spE", bufs=2, side="right")
        pC = tc.alloc_tile_pool(name="pC", bufs=2, side="right")
        state = pC.tile([P, 2 * DG], F32, tag="state", name="state")
        for g in range(DG):
            ewbc = ccol(g, I_EW).to_broadcast([P, RS - 1])
            apre = pC.tile([P, RS - 1], BF16, tag="apre", name="apre")
            nc.vector.tensor_tensor_scan(
                apre[:], ewbc, ekvsb[g][:, :RS - 1], 0.0, ALU.mult, ALU.add)
            nc.gpsimd.tensor_copy(out=state[:, g:g + 1],
                                  in_=apre[:, RS - 2:RS - 1])
            bpre = pC.tile([P, RS - 1], BF16, tag="bpre", name="bpre")
            nc.vector.tensor_tensor_scan(
                bpre[:], ewbc, eksb[g][:, :RS - 1], 0.0, ALU.mult, ALU.add)
            nc.gpsimd.tensor_copy(out=state[:, DG + g:DG + g + 1],
                                  in_=bpre[:, RS - 2:RS - 1])
        nc.sync.dma_start(out=cc_in[:], in_=state[:])
        if not no_collective:
            nc.gpsimd.collective_compute(
                "AllGather", ALU.bypass,
                replica_groups=[list(range(n_cores))],
                ins=[cc_in[:].opt()], outs=[cc_out[:].opt()])
        else:
            for jj in range(n_cores):
                nc.sync.dma_start(out=cc_out[jj * P:(jj + 1) * P, :],
                                  in_=cc_in[:])
        gsb = pC.tile([P, n_cores, 2 * DG], F32, tag="gsb", name="gsb")
        nc.sync.dma_start(
            out=gsb[:], in_=cc_out[:].rearrange("(j p) s -> p j s", p=P))
        a0b0 = pC.tile([P, 2 * DG], F32, tag="a0b0", name="a0b0")
        nc.vector.memset(a0b0[:, 0:DG], 0.0)
        nc.vector.memset(a0b0[:, DG:2 * DG], DEN_EPS)
        for j in range(n_cores):
            nc.vector.scalar_tensor_tensor(
                a0b0[:], gsb[:, j, :], selt[:, j:j + 1], a0b0[:],
                ALU.mult, ALU.add)

        # ============ Phase D: WKV scans + rwkv (fp8 x16) ============
        pD = tc.alloc_tile_pool(name="pD", bufs=3)

        def d_front(g):
            ekg = eksb[g][:]
            xkg = ekvsb[g][:]
            srg = pD.tile([P, RS], BF16, tag="srg", name="srg")
            nc.sync.dma_start(out=srg[:], in_=srdv[:, g, :])
            ewb = pD.tile([P, RS], BF16, tag="ewb", name="ewb")
            nc.scalar.activation(ewb[:], ccol(g, I_EW).to_broadcast([P, RS]),
                                 ACT.Copy)
            eub = pD.tile([P, RS], BF16, tag="eub", name="eub")
            nc.scalar.activation(eub[:], ccol(g, I_EU).to_broadcast([P, RS]),
                                 ACT.Copy)
            abuf = pD.tile([P, RS + 1], BF16, tag="abuf", name="abuf")
            nc.gpsimd.tensor_copy(out=abuf[:, 0:1], in_=a0b0[:, g:g + 1])
            nc.vector.tensor_tensor_scan(
                abuf[:, 1:RS + 1], ewb[:], xkg, a0b0[:, g:g + 1],
                ALU.mult, ALU.add)
            bbuf = pD.tile([P, RS + 1], BF16, tag="bbuf", name="bbuf")
            nc.gpsimd.tensor_copy(out=bbuf[:, 0:1],
                                  in_=a0b0[:, DG + g:DG + g + 1])
            nc.vector.tensor_tensor_scan(
                bbuf[:, 1:RS + 1], ewb[:], ekg,
                a0b0[:, DG + g:DG + g + 1], ALU.mult, ALU.add)
            num = pD.tile([P, RS], BF16, tag="num", name="num")
            nc.vector.scalar_tensor_tensor(
                num[:], xkg, ccol(g, I_EU), abuf[:, 0:RS],
                ALU.mult, ALU.add)
            snum = pD.tile([P, RS], BF16, tag="snum", name="snum")
            nc.vector.tensor_tensor(snum[:], num[:], srg[:], ALU.mult)
            t1 = pD.tile([P, RS], BF16, tag="t1", name="t1")
            nc.gpsimd.tensor_tensor(t1[:], ekg, eub[:], ALU.mult)
            den = pD.tile([P, RS], F32, tag="den", name="den")
            nc.gpsimd.tensor_tensor(den[:], t1[:], bbuf[:, 0:RS], ALU.add)
            return snum, den

        def d_back(g, snum, den):
            rden = pD.tile([P, RS], F32, tag="rden", name="rden")
            nc.vector.reciprocal_approx_fast(out=rden[:], in_=den[:])
            nc.vector.scalar_tensor_tensor(
                rwkv8[g // 2][:, g % 2, :RS], snum[:], SA, rden[:],
                ALU.mult, ALU.mult)

        pend = []
        for g in range(DG):
            pend.append((g, d_front(g)))
            if len(pend) > 2:
                gq, fq = pend.pop(0)
                d_back(gq, *fq)
        for gq, fq in pend:
            d_back(gq, *fq)
        pD.release()
        pEk.release()
        pC.release()
        pMx2 = tc.alloc_tile_pool(name="pMx2", bufs=1)
        xk2h = pMx2.tile([P, CG, RO], F8, tag="xk2h")
        xk2l = pMx2.tile([P, CG, RO], F8, tag="xk2l")
        pXr = tc.alloc_tile_pool(name="pXr", bufs=1)
        xr28 = pXr.tile([P, CG, RO], F8, tag="xr28")
        wpG = tc.alloc_tile_pool(name="wpG", bufs=2)
        spG = tc.alloc_tile_pool(name="spG", bufs=2)
        pX2 = tc.alloc_tile_pool(name="pX2", bufs=1)
        x2bf = pX2.tile([P, CG, RS], BF16, tag="x2bf")

        # ========= Phase E: Wo (2t DR) -> x2 = x + attn (DRAM) =========
        psE = tc.alloc_tile_pool(name="psE", bufs=2, space="PSUM")
        CBLK = 512
        for c0, csz in _splits(Cc, CBLK):
            wbh = wpE.tile([P, DG, CBLK], F8, tag="woh", name="woh")
            nc.sync.dma_start(out=wbh[:, :, :csz], in_=woh[:, :, c0:c0 + csz])
            wbl = wpE.tile([P, DG, CBLK], F8, tag="wol", name="wol")
            nc.sync.dma_start(out=wbl[:, :, :csz], in_=wol[:, :, c0:c0 + csz])
            for gl in range(csz // P):
                g_c = (c0 + gl * P) // P
                for t0, tsz in tstripsB:
                    wsz = min(tsz, RS - t0)
                    if wsz <= 0:
                        continue
                    ps = psE.tile([P, TS], F32, tag="wo_ps", name="wo_ps")
                    i = 0
                    for wb in (wbh, wbl):
                        for gp in range(DG // 2):
                            nc.tensor.matmul(
                                ps[:, :tsz],
                                wb[:, 2 * gp:2 * gp + 2, gl * P:(gl + 1) * P],
                                rwkv8[gp][:, :, t0:t0 + tsz],
                                start=(i == 0), stop=(i == DG - 1),
                                perf_mode=DR)
                            i += 1
                    xst = spE.tile([P, TS], F32, tag="xst", name="xst")
                    nc.sync.dma_start(
                        out=xst[:, :wsz],
                        in_=xTv[:, g_c, 1 + t0:1 + t0 + wsz])
                    x2st = spE.tile([P, TS], F32, tag="x2st", name="x2st")
                    nc.vector.scalar_tensor_tensor(
                        x2st[:, :wsz], ps[:, :wsz], PS_INV,
                        xst[:, :wsz], ALU.mult, ALU.add)
                    nc.sync.dma_start(out=x2dv[:, g_c, t0:t0 + wsz],
                                      in_=x2st[:, :wsz])
                    nc.gpsimd.tensor_copy(out=x2bf[:, g_c, t0:t0 + wsz],
                                          in_=x2st[:, :wsz])
        psE.release()
        spE.release()
        wpE.release()
        pRw.release()

        # ====== Phase F: LN2 + mixes2 (xk2 hi/lo fp8, xr2 fp8) ======
        pG2 = tc.alloc_tile_pool(name="pG2", bufs=1)
        g2 = pG2.tile([P, CG, RS], BF16, tag="g2")
        ln_stream(x2bf, RS, I_LN2W, g2, "ln2", sbuf_src=True, lts=512)
        nc.vector.tensor_scalar_mul(g2[:, :, 0:1], g2[:, :, 0:1], m0[:])

        spF = tc.alloc_tile_pool(name="spF", bufs=3)
        for g in range(CG):
            dmix = spF.tile([P, RO], BF16, tag="dmix2", name="dmix2")
            nc.vector.tensor_tensor(dmix[:], g2[:, g, 1:RS],
                                    g2[:, g, 0:RO], ALU.subtract)
            xr2b = spF.tile([P, RO], BF16, tag="xr2b", name="xr2b")
            nc.vector.scalar_tensor_tensor(
                xr2b[:], dmix[:], ccol(g, I_CMR), g2[:, g, 0:RO],
                ALU.mult, ALU.add)
            nc.gpsimd.tensor_copy(out=xr28[:, g, :], in_=xr2b[:])
        for g in range(CG):
            dmix = spF.tile([P, RO], BF16, tag="dmix2", name="dmix2")
            nc.vector.tensor_tensor(dmix[:], g2[:, g, 1:RS],
                                    g2[:, g, 0:RO], ALU.subtract)
            xk2b = spF.tile([P, RO], BF16, tag="xk2b", name="xk2b")
            nc.vector.scalar_tensor_tensor(
                xk2b[:], dmix[:], ccol(g, I_CMK), g2[:, g, 0:RO],
                ALU.mult, ALU.add)
            nc.scalar.activation(xk2h[:, g, :], xk2b[:], ACT.Copy)
            dif = spF.tile([P, RO], BF16, tag="dif", name="dif")
            nc.vector.tensor_tensor(dif[:], xk2b[:], xk2h[:, g, :],
                                    ALU.subtract)
            nc.scalar.activation(xk2l[:, g, :], dif[:], ACT.Copy)
        spF.release()
        pG2.release()
        pX2.release()

        # ====== Phase G: r2 = sigmoid(xr2 @ WcrT) (pure DR) -> DRAM =====
        psG = tc.alloc_tile_pool(name="psG", bufs=3, space="PSUM")
        for c0, csz in _splits(Cc, CBLK):
            wbh = wpG.tile([P, CG, CBLK], F8, tag="wcr", name="wcr")
            nc.sync.dma_start(out=wbh[:, :, :csz], in_=wcrh[:, :, c0:c0 + csz])
            for gl in range(csz // P):
                g_c = (c0 + gl * P) // P
                for t0, tsz in _splits(RO, TS):
                    ps = psG.tile([P, TS], F32, tag="wcr_ps", name="wcr_ps")
                    for gp in range(CG // 2):
                        nc.tensor.matmul(
                            ps[:, :tsz],
                            wbh[:, 2 * gp:2 * gp + 2, gl * P:(gl + 1) * P],
                            xr28[:, 2 * gp:2 * gp + 2, t0:t0 + tsz],
                            start=(gp == 0), stop=(gp == CG // 2 - 1),
                            perf_mode=DR)
                    sgt = spG.tile([P, TS], BF16, tag="sgt", name="sgt")
                    nc.scalar.activation(sgt[:, :tsz], ps[:, :tsz],
                                         ACT.Sigmoid, scale=PS_INV)
                    nc.sync.dma_start(out=sgdv[:, g_c, t0:t0 + tsz],
                                      in_=sgt[:, :tsz])
        psG.release()
        spG.release()
        wpG.release()
        pXr.release()

        # ============ Phase H: FFN (3t DR both matmuls) ============
        FBLK = 512
        FQ = 16
        for t0, tsz in _splits(RO, TS):
            pH = tc.alloc_tile_pool(name=f"pH{t0}", bufs=1)
            sH = tc.alloc_tile_pool(name=f"sH{t0}", bufs=2)
            wpH = tc.alloc_tile_pool(name=f"wpH{t0}", bufs=2)
            psH = tc.alloc_tile_pool(name=f"psH{t0}", bufs=3, space="PSUM")
            psKV = tc.alloc_tile_pool(name=f"psKV{t0}", bufs=1, space="PSUM")
            kf8 = pH.tile([P, FG, TS], F8, tag="kf8", name="kf8")
            kflo = pH.tile([P, FG, TS], F8, tag="kflo", name="kflo")
            # FFN1 3t: z = Wckh@(xh+xl) + Wckl@xh; trl = sqrt(8)*relu(z)
            for f0, fsz in _splits(Ff, FBLK):
                wbh = wpH.tile([P, CG, FBLK], F8, tag="wfh", name="wfh")
                nc.sync.dma_start(out=wbh[:, :, :fsz],
                                  in_=wckh[:, :, f0:f0 + fsz])
                wbl = wpH.tile([P, CG, FBLK], F8, tag="wfl", name="wfl")
                nc.sync.dma_start(out=wbl[:, :, :fsz],
                                  in_=wckl[:, :, f0:f0 + fsz])
                ngl = fsz // P
                trl = sH.tile([P, ngl, TS], BF16, tag="trl", name="trl")
                for fl in range(ngl):
                    ps = psH.tile([P, TS], F32, tag="ffn1_ps", name="ffn1_ps")
                    i = 0
                    nmm = 3 * (CG // 2)
                    for wb, act in ((wbh, xk2h), (wbh, xk2l), (wbl, xk2h)):
                        for gp in range(CG // 2):
                            nc.tensor.matmul(
                                ps[:, :tsz],
                                wb[:, 2 * gp:2 * gp + 2, fl * P:(fl + 1) * P],
                                act[:, 2 * gp:2 * gp + 2, t0:t0 + tsz],
                                start=(i == 0), stop=(i == nmm - 1),
                                perf_mode=DR)
                            i += 1
                    nc.scalar.activation(trl[:, fl, :tsz], ps[:, :tsz],
                                         ACT.Relu, scale=PS_INV * SQ8)
                # kfb = trl^2 = 8*kf; kf8 = e4m3(kfb); kflo = kfb - kf8
                g_f0 = f0 // P
                kfb = sH.tile([P, ngl, TS], BF16, tag="kfb", name="kfb")
                nc.vector.tensor_tensor(kfb[:, :, :tsz], trl[:, :, :tsz],
                                        trl[:, :, :tsz], ALU.mult)
                nc.scalar.activation(kf8[:, g_f0:g_f0 + ngl, :tsz],
                                     kfb[:, :, :tsz], ACT.Copy)
                nc.vector.scalar_tensor_tensor(
                    kflo[:, g_f0:g_f0 + ngl, :tsz], kfb[:, :, :tsz], 1.0,
                    kf8[:, g_f0:g_f0 + ngl, :tsz], ALU.mult, ALU.subtract)
            # FFN2 3t + final: out = x2 + sg*((Wcvh@(kf8+kflo)+Wcvl@kf8)/512)
            for c0, csz in _splits(Cc, CBLK):
                kvps = [psKV.tile([P, TS], F32, tag=f"kv_ps{i}",
                                  name=f"kv_ps{i}")
                        for i in range(csz // P)]
                nq = FG // FQ
                nmm_tot = nq * 3 * (FQ // 2)
                mm_idx = [0] * (csz // P)
                for q in range(nq):
                    f_lo = q * FQ
                    wbh = wpH.tile([P, FQ, CBLK], F8, tag="wf2h", name="wf2h")
                    nc.sync.dma_start(
                        out=wbh[:, :, :csz],
                        in_=wcvh[:, f_lo:f_lo + FQ, c0:c0 + csz])
                    wbl = wpH.tile([P, FQ, CBLK], F8, tag="wf2l", name="wf2l")
                    nc.sync.dma_start(
                        out=wbl[:, :, :csz],
                        in_=wcvl[:, f_lo:f_lo + FQ, c0:c0 + csz])
                    for gl in range(csz // P):
                        for wb, act in ((wbh, kf8), (wbh, kflo), (wbl, kf8)):
                            for fp in range(FQ // 2):
                                fg = f_lo + 2 * fp
                                nc.tensor.matmul(
                                    kvps[gl][:, :tsz],
                                    wb[:, 2 * fp:2 * fp + 2,
                                       gl * P:(gl + 1) * P],
                                    act[:, fg:fg + 2, :tsz],
                                    start=(mm_idx[gl] == 0),
                                    stop=(mm_idx[gl] == nmm_tot - 1),
                                    perf_mode=DR)
                                mm_idx[gl] += 1
                for gl in range(csz // P):
                    g_c = (c0 + gl * P) // P
                    sgs = wpH.tile([P, TS], BF16, tag="sgs", name="sgs")
                    nc.sync.dma_start(out=sgs[:, :tsz],
                                      in_=sgdv[:, g_c, t0:t0 + tsz])
                    ot = wpH.tile([P, TS], BF16, tag="ot", name="ot")
                    nc.vector.scalar_tensor_tensor(
                        ot[:, :tsz], kvps[gl][:, :tsz], 1.0 / (SKF * SW),
                        sgs[:, :tsz], ALU.mult, ALU.mult)
                    x2s = wpH.tile([P, TS], F32, tag="x2s", name="x2s")
                    nc.sync.dma_start(
                        out=x2s[:, :tsz],
                        in_=x2dv[:, g_c, 1 + t0:1 + t0 + tsz])
                    o2 = wpH.tile([P, TS], F32, tag="o2", name="o2")
                    nc.vector.tensor_tensor(o2[:, :tsz], ot[:, :tsz],
                                            x2s[:, :tsz], ALU.add)
                    nc.sync.dma_start(out=outTv[:, g_c, t0:t0 + tsz],
                                      in_=o2[:, :tsz])
            for p in (psKV, psH, wpH, sH, pH):
                p.release()
        pMx2.release()
        dram.release()
        const.release()

    nc.compile()
    return nc


_PROGRAM_CACHE = {}


def _get_program(key, **kw):
    if key not in _PROGRAM_CACHE:
        _PROGRAM_CACHE[key] = build_program(**kw)
    return _PROGRAM_CACHE[key]


def _q8pair(wT_scaled):
    """fp32 [128, KG, N] (already x SW) -> (hi, lo) e4m3 at the same scale."""
    hi = wT_scaled.astype(E4M3)
    lo = (wT_scaled - hi.astype(np.float32)).astype(E4M3)
    return hi, lo


def _host_prep(inputs, Cc=C, Dd=D_ATT, Ff=D_FFN, Bb=B, Tt=T, n_cores=N_CORES):
    P = 128
    CG, DG, FG = Cc // P, Dd // P, Ff // P
    half = Tt // 2
    RO, RS, R = half, half + 1, half + 2

    f = {k: np.asarray(v, np.float32) for k, v in inputs.items()}
    x = f["x"]

    def swz(wT, kg):  # [K, N] fp32 -> [128, kg, N] * SW
        Kdim, Ndim = wT.shape
        return np.ascontiguousarray(
            wT.reshape(kg, P, Ndim).transpose(1, 0, 2)) * SW

    wkh_, _ = _q8pair(swz(f["Wk"].T, CG))
    wvh_, wvl_ = _q8pair(swz(f["Wv"].T, CG))
    wrh_, _ = _q8pair(swz(f["Wr"].T, CG))
    woh_, wol_ = _q8pair(swz(f["Wo"].T, DG))
    wckh_, wckl_ = _q8pair(swz(f["Wck"].T, CG))
    wcvh_, wcvl_ = _q8pair(swz(f["Wcv"].T, FG))
    wcrh_, _ = _q8pair(swz(f["Wcr"].T, CG))

    def col(v):
        return np.ascontiguousarray(
            np.asarray(v, np.float32).reshape(-1).reshape(CG, P).T)

    ew = np.exp(-np.exp(f["time_decay"].astype(np.float64)))
    cvec_h = np.stack([
        col(f["ln1_w"] * SA), col(f["ln1_b"]),
        col(f["tm_k"]), col(f["tm_v"]), col(f["tm_r"]),
        col(ew.astype(np.float32)), col(np.exp(f["time_first"])),
        col(f["ln2_w"] * SA), col(f["ln2_b"]),
        col(f["cm_k"]), col(f["cm_r"]),
    ], axis=-1).astype(np.float32)

    in_maps = []
    for core in range(n_cores):
        b, hh = core // 2, core % 2
        t0 = hh * half
        xr = np.zeros((R, Cc), np.float32)
        lo = t0 - 2
        src_lo = max(lo, 0)
        xr[src_lo - lo:, :] = x[b, src_lo:t0 + RO, :]
        m0 = np.full((P, 1), float(hh), np.float32)
        sel = np.zeros((P, n_cores), np.float32)
        if hh == 1:
            sel[:, core - 1] = 1.0
        xrt = np.ascontiguousarray(xr.T)
        in_maps.append({
            "xT": xrt, "xTb": xrt.astype(ml_dtypes.bfloat16),
            "wkh": wkh_, "wvh": wvh_, "wvl": wvl_, "wrh": wrh_,
            "woh": woh_, "wol": wol_, "wckh": wckh_, "wckl": wckl_,
            "wcvh": wcvh_, "wcvl": wcvl_, "wcrh": wcrh_,
            "cvec": cvec_h, "m0": m0, "sel": sel,
        })
    return in_maps


def kernel(**inputs):
    in_maps = _host_prep(inputs)
    nc = _get_program("full")
    res = run_bass_kernel_spmd(nc, in_maps, core_ids=list(range(N_CORES)))
    half = T // 2
    out = np.empty((B, T, C), np.float32)
    for core in range(N_CORES):
        b, hh = core // 2, core % 2
        out[b, hh * half:(hh + 1) * half, :] = res.results[core]["outT"].T
    return out


# revision 6
# speedup vs baseline: 1.3858x; 1.0114x over previous
"""RWKV-4 block on 8 trn2 cores — fp8e4 DoubleRow version.

Sharding: 8 cores = 4 batch x 2 T-halves (as baseline). All big matmuls run
as fp8e4 DoubleRow (K=256/instr, 0.5 cyc/row). Precision scheme (measured
offline: final rel err ~1.2e-2 vs the 2e-2 gate):
  Wk, Wr, Wcr: pure fp8 (weights e4m3 x64, acts e4m3 x16)
  Wv, Wo:      2-term (weight hi+lo at the same scale; lo rides subnormals)
  Wck, Wcv:    3-term (weight hi+lo AND activation hi+lo, same scale)
Same-scale lo parts make every term share one PSUM scale, so all terms
accumulate natively in PSUM with no combine ops.
"""

import os
import sys

import numpy as np

for _p in ("/opt/trn_rl_repo", "/root/.axon_site/_ro/trn_rl_repo"):
    if os.path.isdir(_p) and _p not in sys.path:
        sys.path.insert(0, _p)

import ml_dtypes  # noqa: E402

import concourse.bass as bass  # noqa: E402,F401
import concourse.mybir as mybir  # noqa: E402
import concourse.tile as tile  # noqa: E402
from concourse import bacc  # noqa: E402
from concourse.bass_utils import run_bass_kernel_spmd  # noqa: E402

F32 = mybir.dt.float32
F32R = mybir.dt.float32r
BF16 = mybir.dt.bfloat16
F8 = mybir.dt.float8e4
ALU = mybir.AluOpType
ACT = mybir.ActivationFunctionType
DR = mybir.MatmulPerfMode.DoubleRow
E4M3 = ml_dtypes.float8_e4m3

B, T, C, D_ATT, D_FFN = 4, 2048, 2048, 2048, 8192
EPS = 1e-5
N_CORES = 8
DEN_EPS = 1e-30

SA = 16.0          # activation fp8 scale
SW = 64.0          # weight fp8 scale
SKF = 8.0          # kf fp8 scale
PS_INV = 1.0 / (SA * SW)     # psum -> true scale (2^-10)
SQ8 = float(np.sqrt(SKF))


def _splits(total, sz):
    return [(s, min(sz, total - s)) for s in range(0, total, sz)]


def _even_splits(total, mx):
    n = -(-total // mx)
    base, rem = divmod(total, n)
    out, s = [], 0
    for i in range(n):
        sz = base + (1 if i < rem else 0)
        out.append((s, sz))
        s += sz
    return out


def build_program(Cc=C, Dd=D_ATT, Ff=D_FFN, rows_out=T // 2, n_cores=N_CORES,
                  no_collective=False):
    P = 128
    CG, DG, FG = Cc // P, Dd // P, Ff // P
    RO = rows_out
    RS = RO + 1
    R = RS + 1
    RSP = -(-RS // 16) * 16   # fp8 moving tiles padded: pair stride %16 == 0
    NV = 11

    nc = bacc.Bacc("TRN2", target_bir_lowering=False, debug=False,
                   num_devices=n_cores)

    xT = nc.dram_tensor("xT", [Cc, R], F32, kind="ExternalInput").ap()
    xTb = nc.dram_tensor("xTb", [Cc, R], BF16, kind="ExternalInput").ap()
    wkh = nc.dram_tensor("wkh", [P, CG, Dd], F8, kind="ExternalInput").ap()
    wvh = nc.dram_tensor("wvh", [P, CG, Dd], F8, kind="ExternalInput").ap()
    wvl = nc.dram_tensor("wvl", [P, CG, Dd], F8, kind="ExternalInput").ap()
    wrh = nc.dram_tensor("wrh", [P, CG, Dd], F8, kind="ExternalInput").ap()
    woh = nc.dram_tensor("woh", [P, DG, Cc], F8, kind="ExternalInput").ap()
    wol = nc.dram_tensor("wol", [P, DG, Cc], F8, kind="ExternalInput").ap()
    wckh = nc.dram_tensor("wckh", [P, CG, Ff], F8, kind="ExternalInput").ap()
    wckl = nc.dram_tensor("wckl", [P, CG, Ff], F8, kind="ExternalInput").ap()
    wcvh = nc.dram_tensor("wcvh", [P, FG, Cc], F8, kind="ExternalInput").ap()
    wcvl = nc.dram_tensor("wcvl", [P, FG, Cc], F8, kind="ExternalInput").ap()
    wcrh = nc.dram_tensor("wcrh", [P, CG, Cc], F8, kind="ExternalInput").ap()
    cvec = nc.dram_tensor("cvec", [P, CG, NV], F32, kind="ExternalInput").ap()
    m0d = nc.dram_tensor("m0", [P, 1], F32, kind="ExternalInput").ap()
    seld = nc.dram_tensor("sel", [P, n_cores], F32, kind="ExternalInput").ap()
    outT = nc.dram_tensor("outT", [Cc, RO], F32, kind="ExternalOutput").ap()

    xTv = xT.rearrange("(g p) r -> p g r", p=P)
    xTbv = xTb.rearrange("(g p) r -> p g r", p=P)
    outTv = outT.rearrange("(g p) r -> p g r", p=P)

    I_LN1W, I_LN1B, I_TMK, I_TMV, I_TMR, I_EW, I_EU, I_LN2W, I_LN2B, \
        I_CMK, I_CMR = range(NV)

    TS = 512
    LTS = 256

    with tile.TileContext(nc) as tc:
        const = tc.alloc_tile_pool(name="const", bufs=1)
        con = const.tile([P, CG, NV], F32, tag="con")
        nc.sync.dma_start(out=con[:], in_=cvec)
        m0 = const.tile([P, 1], F32, tag="m0")
        nc.sync.dma_start(out=m0[:], in_=m0d)
        selt = const.tile([P, n_cores], F32, tag="sel")
        nc.sync.dma_start(out=selt[:], in_=seld)
        onesc = const.tile([P, 1], F32, tag="ones")
        nc.vector.memset(onesc[:], 1.0)
        onesb = const.tile([P, 1], BF16, tag="onesb")
        nc.vector.memset(onesb[:], 1.0)
        epsc = const.tile([1, 1], F32, tag="epsc")
        nc.vector.memset(epsc[:], EPS)
        onesPb = const.tile([1, P], BF16, tag="onesPb")
        nc.vector.memset(onesPb[:], 1.0)

        def ccol(g, i):
            return con[:, g, i:i + 1]

        dram = tc.alloc_tile_pool(name="dram", bufs=1, space="DRAM")
        x2dram = dram.tile([Cc, RS], F32)
        x2dv = x2dram.rearrange("(g p) r -> p g r", p=P)
        srdram = dram.tile([Dd, RS], BF16)
        srdv = srdram.rearrange("(g p) r -> p g r", p=P)
        sgdram = dram.tile([Cc, RO], BF16)
        sgdv = sgdram.rearrange("(g p) r -> p g r", p=P)
        cc_in = dram.tile([P, 2 * DG], F32)
        cc_out = dram.tile([P * n_cores, 2 * DG], F32)

        # ---- LayerNorm (streaming; PE sums via f32r bitcast) ----
        def ln_stream(src_v, nrows, iw, out_sb, name, sbuf_src=False,
                      src_bf16=False, lts=None):
            LTS = lts or 256
            src_dt = BF16 if src_bf16 else F32
            st = tc.alloc_tile_pool(name=f"{name}_st", bufs=1)
            sp = tc.alloc_tile_pool(name=f"{name}_sp", bufs=2)
            spx = tc.alloc_tile_pool(name=f"{name}_spx", bufs=8)
            psum = tc.alloc_tile_pool(name=f"{name}_ps", bufs=2, space="PSUM")
            ssum = st.tile([1, nrows], F32, tag="sum", name="ssum")
            ssq = st.tile([1, nrows], F32, tag="sq", name="ssq")
            for t0, tsz in _splits(nrows, LTS):
                if sbuf_src:
                    xls = src_v[:, :, t0:t0 + tsz]
                else:
                    xlt = sp.tile([P, CG, LTS], src_dt, tag="xls",
                                  name="xls")
                    nc.sync.dma_start(out=xlt[:, :, :tsz],
                                      in_=src_v[:, :, t0:t0 + tsz])
                    xls = xlt[:, :, :tsz]
                xsq = sp.tile([P, CG, LTS], BF16, tag="lnsq", name="xsq")
                nc.scalar.activation(xsq[:, :, :tsz], xls,
                                     ACT.Square)
                ps = psum.tile([1, LTS], F32, tag="ln_ps", name="ps")
                ps2 = psum.tile([1, LTS], F32, tag="ln_ps2", name="ps2")
                for g in range(CG):
                    nc.tensor.matmul(
                        ps[:, :tsz], onesb[:], xls[:, g, :],
                        start=(g == 0), stop=(g == CG - 1))
                    nc.tensor.matmul(
                        ps2[:, :tsz], onesb[:], xsq[:, g, :tsz],
                        start=(g == 0), stop=(g == CG - 1))
                nc.vector.tensor_copy(out=ssum[:, t0:t0 + tsz],
                                      in_=ps[:, :tsz])
                nc.vector.tensor_copy(out=ssq[:, t0:t0 + tsz],
                                      in_=ps2[:, :tsz])
            mu = st.tile([1, nrows], BF16, tag="mu", name="mu")
            rstd = st.tile([1, nrows], BF16, tag="rstd", name="rstd")
            var = st.tile([1, nrows], F32, tag="var", name="var")
            musq = st.tile([1, nrows], F32, tag="musq", name="musq")
            nc.vector.tensor_scalar_mul(mu[:], ssum[:], 1.0 / Cc)
            nc.vector.tensor_scalar_mul(var[:], ssq[:], 1.0 / Cc)
            nc.vector.tensor_tensor(musq[:], mu[:], mu[:], ALU.mult)
            nc.vector.tensor_tensor(var[:], var[:], musq[:], ALU.subtract)
            nc.scalar.activation(var[:], var[:], ACT.Ln, bias=epsc[:])
            nc.scalar.activation(rstd[:], var[:], ACT.Exp, scale=-0.5)
            for t0, tsz in _splits(nrows, LTS):
                if sbuf_src:
                    xls = src_v[:, :, t0:t0 + tsz]
                else:
                    xlt = sp.tile([P, CG, LTS], src_dt, tag="xls",
                                  name="xls")
                    nc.sync.dma_start(out=xlt[:, :, :tsz],
                                      in_=src_v[:, :, t0:t0 + tsz])
                    xls = xlt[:, :, :tsz]
                mups = psum.tile([P, LTS], F32, tag="mups", name="mups")
                nc.tensor.matmul(mups[:, :tsz], onesPb[:],
                                 mu[:, t0:t0 + tsz],
                                 start=True, stop=True)
                rsps = psum.tile([P, LTS], F32, tag="rsps", name="rsps")
                nc.tensor.matmul(rsps[:, :tsz], onesPb[:],
                                 rstd[:, t0:t0 + tsz],
                                 start=True, stop=True)
                for g in range(CG):
                    xm = spx.tile([P, LTS], BF16, tag="ln_xm", name="xm")
                    nc.vector.tensor_tensor(xm[:, :tsz], xls[:, g, :],
                                            mups[:, :tsz], ALU.subtract)
                    nc.vector.scalar_tensor_tensor(
                        out_sb[:, g, t0:t0 + tsz], xm[:, :tsz], ccol(g, iw),
                        rsps[:, :tsz], ALU.mult, ALU.mult)
            for p in (psum, spx, sp, st):
                p.release()

        # ================= Phase A: LN1 (h = 16*ln(x), bf16) ============
        pEk = tc.alloc_tile_pool(name="pEk", bufs=1)
        eksb = [pEk.tile([P, RS], BF16, tag=f"eksb{g}", name=f"eksb{g}")
                for g in range(DG)]
        ekvsb = [pEk.tile([P, RS], BF16, tag=f"ekvsb{g}", name=f"ekvsb{g}")
                 for g in range(DG)]
        pMix = tc.alloc_tile_pool(name="pMix", bufs=1)
        mixk8 = [pMix.tile([P, 2, RSP], F8, tag=f"mixk8_{p}",
                           name=f"mixk8_{p}") for p in range(CG // 2)]
        mixv8 = [pMix.tile([P, 2, RSP], F8, tag=f"mixv8_{p}",
                           name=f"mixv8_{p}") for p in range(CG // 2)]
        mixr8 = [pMix.tile([P, 2, RSP], F8, tag=f"mixr8_{p}",
                           name=f"mixr8_{p}") for p in range(CG // 2)]
        pHs = tc.alloc_tile_pool(name="pHs", bufs=1)
        hs = pHs.tile([P, CG, R], BF16, tag="hs")
        ln_stream(xTbv, R, I_LN1W, hs, "ln1", src_bf16=True)
        nc.vector.tensor_scalar_mul(hs[:, :, 0:2], hs[:, :, 0:2], m0[:])

        # ========== Phase B: mixes (fp8 x16) + k/v/r DR matmuls ========
        stg = tc.alloc_tile_pool(name="stg", bufs=4)
        if RSP > RS:
            for mixl in (mixk8, mixv8, mixr8):
                for mt in mixl:
                    nc.vector.memset(mt[:, :, RS:RSP], 0.0)
        MSTRIPS = [(0, 512), (512, RS - 512)]
        for t0, tsz in MSTRIPS:
            for g in range(CG):
                dmix = stg.tile([P, 512 + 1], BF16, tag="dmix", name="dmix")
                nc.vector.tensor_tensor(
                    dmix[:, :tsz], hs[:, g, 1 + t0:1 + t0 + tsz],
                    hs[:, g, t0:t0 + tsz], ALU.subtract)
                for mixl, icoef, on_act in ((mixk8, I_TMK, True),
                                            (mixv8, I_TMV, False),
                                            (mixr8, I_TMR, True)):
                    mb16 = stg.tile([P, 512 + 1], BF16, tag="mb16",
                                    name="mb16")
                    nc.vector.scalar_tensor_tensor(
                        mb16[:, :tsz], dmix[:, :tsz], ccol(g, icoef),
                        hs[:, g, t0:t0 + tsz], ALU.mult, ALU.add)
                    dst = mixl[g // 2][:, g % 2, t0:t0 + tsz]
                    if on_act:
                        nc.scalar.activation(dst, mb16[:, :tsz], ACT.Copy)
                    else:
                        nc.gpsimd.tensor_copy(out=dst, in_=mb16[:, :tsz])
        wpB = tc.alloc_tile_pool(name="wpB", bufs=2)
        stgE = tc.alloc_tile_pool(name="stgE", bufs=3)
        psB = tc.alloc_tile_pool(name="psB", bufs=4, space="PSUM")
        DBLK = 512
        tstripsB = [(0, 512), (512, 512), (1024, RSP - 1024)]

        def mm_dr(whd, wld, rhs8, n_out, evict, wtag, strips=None):
            for d0, dsz in _splits(n_out, DBLK):
                wbh = wpB.tile([P, CG, DBLK], F8, tag="wh", name="wbh")
                nc.sync.dma_start(out=wbh[:, :, :dsz],
                                  in_=whd[:, :, d0:d0 + dsz])
                if wld is not None:
                    wbl = wpB.tile([P, CG, DBLK], F8, tag="wl",
                                   name="wbl")
                    nc.sync.dma_start(out=wbl[:, :, :dsz],
                                      in_=wld[:, :, d0:d0 + dsz])
                wbufs = [wbh] if wld is None else [wbh, wbl]
                for gl in range(dsz // P):
                    g_out = (d0 + gl * P) // P
                    for t0, tsz in (strips or tstripsB):
                        wsz = min(tsz, RS - t0)
                        if wsz <= 0:
                            continue
                        ps = psB.tile([P, TS], F32, tag="mm_ps", name="mm_ps")
                        nmm = len(wbufs) * (CG // 2)
                        i = 0
                        for wb in wbufs:
                            for gp in range(CG // 2):
                                nc.tensor.matmul(
                                    ps[:, :tsz],
                                    wb[:, 2 * gp:2 * gp + 2,
                                       gl * P:(gl + 1) * P],
                                    rhs8[gp][:, :, t0:t0 + tsz],
                                    start=(i == 0), stop=(i == nmm - 1),
                                    perf_mode=DR)
                                i += 1
                        evict(g_out, t0, wsz, ps)

        def evict_k(g, t0, wsz, ps):
            nc.scalar.activation(eksb[g][:, t0:t0 + wsz], ps[:, :wsz],
                                 ACT.Exp, scale=PS_INV)
            if t0 == 0:
                nc.vector.tensor_scalar_mul(eksb[g][:, 0:1], eksb[g][:, 0:1],
                                            m0[:])

        def evict_v(g, t0, wsz, ps):
            nc.vector.scalar_tensor_tensor(
                ekvsb[g][:, t0:t0 + wsz], ps[:, :wsz], PS_INV,
                eksb[g][:, t0:t0 + wsz], ALU.mult, ALU.mult)

        def evict_r(g, t0, wsz, ps):
            srt = stgE.tile([P, TS], BF16, tag="srt", name="srt")
            nc.scalar.activation(srt[:, :wsz], ps[:, :wsz], ACT.Sigmoid,
                                 scale=PS_INV)
            nc.sync.dma_start(out=srdv[:, g, t0:t0 + wsz], in_=srt[:, :wsz])

        mm_dr(wkh, None, mixk8, Dd, evict_k, "wk", strips=tstripsB[:1])
        mm_dr(wkh, None, mixk8, Dd, evict_k, "wk", strips=tstripsB[1:])
        mm_dr(wvh, wvl, mixv8, Dd, evict_v, "wv")
        mm_dr(wrh, None, mixr8, Dd, evict_r, "wr")

        psB.release()
        stgE.release()
        wpB.release()
        stg.release()
        pHs.release()
        pMix.release()

        # ====== Phase C: boundary states (bf16 scans) + AllGather =======
        pRw = tc.alloc_tile_pool(name="pRw", bufs=1, side="right")
        rwkv8 = [pRw.tile([P, 2, RSP], F8, tag=f"rw{p}", name=f"rw{p}")
                 for p in range(DG // 2)]
        if RSP > RS:
            for rwt in rwkv8:
                nc.vector.memset(rwt[:, :, RS:RSP], 0.0)
        wpE = tc.alloc_tile_pool(name="wpE", bufs=2, side="right")
        spE = tc.alloc_tile_pool(name="spE", bufs=2, side="right")
        pC = tc.alloc_tile_pool(name="pC", bufs=2, side="right")
        state = pC.tile([P, 2 * DG], F32, tag="state", name="state")
        for g in range(DG):
            ewbc = ccol(g, I_EW).to_broadcast([P, RS - 1])
            apre = pC.tile([P, RS - 1], BF16, tag="apre", name="apre")
            nc.vector.tensor_tensor_scan(
                apre[:], ewbc, ekvsb[g][:, :RS - 1], 0.0, ALU.mult, ALU.add)
            nc.gpsimd.tensor_copy(out=state[:, g:g + 1],
                                  in_=apre[:, RS - 2:RS - 1])
            bpre = pC.tile([P, RS - 1], BF16, tag="bpre", name="bpre")
            nc.vector.tensor_tensor_scan(
                bpre[:], ewbc, eksb[g][:, :RS - 1], 0.0, ALU.mult, ALU.add)
            nc.gpsimd.tensor_copy(out=state[:, DG + g:DG + g + 1],
                                  in_=bpre[:, RS - 2:RS - 1])
        nc.sync.dma_start(out=cc_in[:], in_=state[:])
        if not no_collective:
            nc.gpsimd.collective_compute(
                "AllGather", ALU.bypass,
                replica_groups=[list(range(n_cores))],
                ins=[cc_in[:].opt()], outs=[cc_out[:].opt()])
        else:
            for jj in range(n_cores):
                nc.sync.dma_start(out=cc_out[jj * P:(jj + 1) * P, :],
                                  in_=cc_in[:])
        gsb = pC.tile([P, n_cores, 2 * DG], F32, tag="gsb", name="gsb")
        nc.sync.dma_start(
            out=gsb[:], in_=cc_out[:].rearrange("(j p) s -> p j s", p=P))
        a0b0 = pC.tile([P, 2 * DG], F32, tag="a0b0", name="a0b0")
        nc.vector.memset(a0b0[:, 0:DG], 0.0)
        nc.vector.memset(a0b0[:, DG:2 * DG], DEN_EPS)
        for j in range(n_cores):
            nc.vector.scalar_tensor_tensor(
                a0b0[:], gsb[:, j, :], selt[:, j:j + 1], a0b0[:],
                ALU.mult, ALU.add)

        # ============ Phase D: WKV scans + rwkv (fp8 x16) ============
        pD = tc.alloc_tile_pool(name="pD", bufs=3)

        def d_front(g):
            ekg = eksb[g][:]
            xkg = ekvsb[g][:]
            srg = pD.tile([P, RS], BF16, tag="srg", name="srg")
            nc.sync.dma_start(out=srg[:], in_=srdv[:, g, :])
            ewb = pD.tile([P, RS], BF16, tag="ewb", name="ewb")
            nc.scalar.activation(ewb[:], ccol(g, I_EW).to_broadcast([P, RS]),
                                 ACT.Copy)
            eub = pD.tile([P, RS], BF16, tag="eub", name="eub")
            nc.scalar.activation(eub[:], ccol(g, I_EU).to_broadcast([P, RS]),
                                 ACT.Copy)
            abuf = pD.tile([P, RS + 1], BF16, tag="abuf", name="abuf")
            nc.gpsimd.tensor_copy(out=abuf[:, 0:1], in_=a0b0[:, g:g + 1])
            nc.vector.tensor_tensor_scan(
                abuf[:, 1:RS + 1], ewb[:], xkg, a0b0[:, g:g + 1],
                ALU.mult, ALU.add)
            bbuf = pD.tile([P, RS + 1], BF16, tag="bbuf", name="bbuf")
            nc.gpsimd.tensor_copy(out=bbuf[:, 0:1],
                                  in_=a0b0[:, DG + g:DG + g + 1])
            nc.vector.tensor_tensor_scan(
                bbuf[:, 1:RS + 1], ewb[:], ekg,
                a0b0[:, DG + g:DG + g + 1], ALU.mult, ALU.add)
            num = pD.tile([P, RS], BF16, tag="num", name="num")
            nc.vector.scalar_tensor_tensor(
                num[:], xkg, ccol(g, I_EU), abuf[:, 0:RS],
                ALU.mult, ALU.add)
            snum = pD.tile([P, RS], BF16, tag="snum", name="snum")
            nc.vector.tensor_tensor(snum[:], num[:], srg[:], ALU.mult)
            t1 = pD.tile([P, RS], BF16, tag="t1", name="t1")
            nc.gpsimd.tensor_tensor(t1[:], ekg, eub[:], ALU.mult)
            den = pD.tile([P, RS], F32, tag="den", name="den")
            nc.gpsimd.tensor_tensor(den[:], t1[:], bbuf[:, 0:RS], ALU.add)
            return snum, den

        def d_back(g, snum, den):
            rden = pD.tile([P, RS], F32, tag="rden", name="rden")
            nc.vector.reciprocal_approx_fast(out=rden[:], in_=den[:])
            nc.vector.scalar_tensor_tensor(
                rwkv8[g // 2][:, g % 2, :RS], snum[:], SA, rden[:],
                ALU.mult, ALU.mult)

        pend = []
        for g in range(DG):
            pend.append((g, d_front(g)))
            if len(pend) > 3:
                gq, fq = pend.pop(0)
                d_back(gq, *fq)
        for gq, fq in pend:
            d_back(gq, *fq)
        pD.release()
        pEk.release()
        pC.release()
        pMx2 = tc.alloc_tile_pool(name="pMx2", bufs=1)
        xk2h = pMx2.tile([P, CG, RO], F8, tag="xk2h")
        xk2l = pMx2.tile([P, CG, RO], F8, tag="xk2l")
        pXr = tc.alloc_tile_pool(name="pXr", bufs=1)
        xr28 = pXr.tile([P, CG, RO], F8, tag="xr28")
        wpG = tc.alloc_tile_pool(name="wpG", bufs=2)
        spG = tc.alloc_tile_pool(name="spG", bufs=2)
        pX2 = tc.alloc_tile_pool(name="pX2", bufs=1)
        x2bf = pX2.tile([P, CG, RS], BF16, tag="x2bf")

        # ========= Phase E: Wo (2t DR) -> x2 = x + attn (DRAM) =========
        psE = tc.alloc_tile_pool(name="psE", bufs=3, space="PSUM")
        CBLK = 512
        for c0, csz in _splits(Cc, CBLK):
            wbh = wpE.tile([P, DG, CBLK], F8, tag="woh", name="woh")
            nc.sync.dma_start(out=wbh[:, :, :csz], in_=woh[:, :, c0:c0 + csz])
            wbl = wpE.tile([P, DG, CBLK], F8, tag="wol", name="wol")
            nc.sync.dma_start(out=wbl[:, :, :csz], in_=wol[:, :, c0:c0 + csz])
            for gl in range(csz // P):
                g_c = (c0 + gl * P) // P
                for t0, tsz in tstripsB:
                    wsz = min(tsz, RS - t0)
                    if wsz <= 0:
                        continue
                    ps = psE.tile([P, TS], F32, tag="wo_ps", name="wo_ps")
                    i = 0
                    for wb in (wbh, wbl):
                        for gp in range(DG // 2):
                            nc.tensor.matmul(
                                ps[:, :tsz],
                                wb[:, 2 * gp:2 * gp + 2, gl * P:(gl + 1) * P],
                                rwkv8[gp][:, :, t0:t0 + tsz],
                                start=(i == 0), stop=(i == DG - 1),
                                perf_mode=DR)
                            i += 1
                    xst = spE.tile([P, TS], F32, tag="xst", name="xst")
                    nc.sync.dma_start(
                        out=xst[:, :wsz],
                        in_=xTv[:, g_c, 1 + t0:1 + t0 + wsz])
                    x2st = spE.tile([P, TS], F32, tag="x2st", name="x2st")
                    nc.vector.scalar_tensor_tensor(
                        x2st[:, :wsz], ps[:, :wsz], PS_INV,
                        xst[:, :wsz], ALU.mult, ALU.add)
                    nc.sync.dma_start(out=x2dv[:, g_c, t0:t0 + wsz],
                                      in_=x2st[:, :wsz])
                    nc.gpsimd.tensor_copy(out=x2bf[:, g_c, t0:t0 + wsz],
                                          in_=x2st[:, :wsz])
        psE.release()
        spE.release()
        wpE.release()
        pRw.release()

        # ====== Phase F: LN2 + mixes2 (xk2 hi/lo fp8, xr2 fp8) ======
        pG2 = tc.alloc_tile_pool(name="pG2", bufs=1)
        g2 = pG2.tile([P, CG, RS], BF16, tag="g2")
        ln_stream(x2bf, RS, I_LN2W, g2, "ln2", sbuf_src=True, lts=512)
        nc.vector.tensor_scalar_mul(g2[:, :, 0:1], g2[:, :, 0:1], m0[:])

        spF = tc.alloc_tile_pool(name="spF", bufs=3)
        for g in range(CG):
            dmix = spF.tile([P, RO], BF16, tag="dmix2", name="dmix2")
            nc.vector.tensor_tensor(dmix[:], g2[:, g, 1:RS],
                                    g2[:, g, 0:RO], ALU.subtract)
            xr2b = spF.tile([P, RO], BF16, tag="xr2b", name="xr2b")
            nc.vector.scalar_tensor_tensor(
                xr2b[:], dmix[:], ccol(g, I_CMR), g2[:, g, 0:RO],
                ALU.mult, ALU.add)
            nc.gpsimd.tensor_copy(out=xr28[:, g, :], in_=xr2b[:])
        for g in range(CG):
            dmix = spF.tile([P, RO], BF16, tag="dmix2", name="dmix2")
            nc.vector.tensor_tensor(dmix[:], g2[:, g, 1:RS],
                                    g2[:, g, 0:RO], ALU.subtract)
            xk2b = spF.tile([P, RO], BF16, tag="xk2b", name="xk2b")
            nc.vector.scalar_tensor_tensor(
                xk2b[:], dmix[:], ccol(g, I_CMK), g2[:, g, 0:RO],
                ALU.mult, ALU.add)
            nc.scalar.activation(xk2h[:, g, :], xk2b[:], ACT.Copy)
            dif = spF.tile([P, RO], BF16, tag="dif", name="dif")
            nc.vector.tensor_tensor(dif[:], xk2b[:], xk2h[:, g, :],
                                    ALU.subtract)
            nc.scalar.activation(xk2l[:, g, :], dif[:], ACT.Copy)
        spF.release()
        pG2.release()
        pX2.release()

        # ====== Phase G: r2 = sigmoid(xr2 @ WcrT) (pure DR) -> DRAM =====
        psG = tc.alloc_tile_pool(name="psG", bufs=3, space="PSUM")
        for c0, csz in _splits(Cc, CBLK):
            wbh = wpG.tile([P, CG, CBLK], F8, tag="wcr", name="wcr")
            nc.sync.dma_start(out=wbh[:, :, :csz], in_=wcrh[:, :, c0:c0 + csz])
            for gl in range(csz // P):
                g_c = (c0 + gl * P) // P
                for t0, tsz in _splits(RO, TS):
                    ps = psG.tile([P, TS], F32, tag="wcr_ps", name="wcr_ps")
                    for gp in range(CG // 2):
                        nc.tensor.matmul(
                            ps[:, :tsz],
                            wbh[:, 2 * gp:2 * gp + 2, gl * P:(gl + 1) * P],
                            xr28[:, 2 * gp:2 * gp + 2, t0:t0 + tsz],
                            start=(gp == 0), stop=(gp == CG // 2 - 1),
                            perf_mode=DR)
                    sgt = spG.tile([P, TS], BF16, tag="sgt", name="sgt")
                    nc.scalar.activation(sgt[:, :tsz], ps[:, :tsz],
                                         ACT.Sigmoid, scale=PS_INV)
                    nc.sync.dma_start(out=sgdv[:, g_c, t0:t0 + tsz],
                                      in_=sgt[:, :tsz])
        psG.release()
        spG.release()
        wpG.release()
        pXr.release()

        # ============ Phase H: FFN (3t DR both matmuls) ============
        FBLK = 512
        FQ = 16
        for t0, tsz in _splits(RO, TS):
            pH = tc.alloc_tile_pool(name=f"pH{t0}", bufs=1)
            sH = tc.alloc_tile_pool(name=f"sH{t0}", bufs=2)
            wpH = tc.alloc_tile_pool(name=f"wpH{t0}", bufs=2)
            psH = tc.alloc_tile_pool(name=f"psH{t0}", bufs=3, space="PSUM")
            psKV = tc.alloc_tile_pool(name=f"psKV{t0}", bufs=1, space="PSUM")
            kf8 = pH.tile([P, FG, TS], F8, tag="kf8", name="kf8")
            kflo = pH.tile([P, FG, TS], F8, tag="kflo", name="kflo")
            # FFN1 3t: z = Wckh@(xh+xl) + Wckl@xh; trl = sqrt(8)*relu(z)
            for f0, fsz in _splits(Ff, FBLK):
                wbh = wpH.tile([P, CG, FBLK], F8, tag="wfh", name="wfh")
                nc.sync.dma_start(out=wbh[:, :, :fsz],
                                  in_=wckh[:, :, f0:f0 + fsz])
                wbl = wpH.tile([P, CG, FBLK], F8, tag="wfl", name="wfl")
                nc.sync.dma_start(out=wbl[:, :, :fsz],
                                  in_=wckl[:, :, f0:f0 + fsz])
                ngl = fsz // P
                trl = sH.tile([P, ngl, TS], BF16, tag="trl", name="trl")
                for fl in range(ngl):
                    ps = psH.tile([P, TS], F32, tag="ffn1_ps", name="ffn1_ps")
                    i = 0
                    nmm = 3 * (CG // 2)
                    for wb, act in ((wbh, xk2h), (wbh, xk2l), (wbl, xk2h)):
                        for gp in range(CG // 2):
                            nc.tensor.matmul(
                                ps[:, :tsz],
                                wb[:, 2 * gp:2 * gp + 2, fl * P:(fl + 1) * P],
                                act[:, 2 * gp:2 * gp + 2, t0:t0 + tsz],
                                start=(i == 0), stop=(i == nmm - 1),
                                perf_mode=DR)
                            i += 1
                    nc.scalar.activation(trl[:, fl, :tsz], ps[:, :tsz],
                                         ACT.Relu, scale=PS_INV * SQ8)
                # kfb = trl^2 = 8*kf; kf8 = e4m3(kfb); kflo = kfb - kf8
                g_f0 = f0 // P
                kfb = sH.tile([P, ngl, TS], BF16, tag="kfb", name="kfb")
                nc.vector.tensor_tensor(kfb[:, :, :tsz], trl[:, :, :tsz],
                                        trl[:, :, :tsz], ALU.mult)
                nc.scalar.activation(kf8[:, g_f0:g_f0 + ngl, :tsz],
                                     kfb[:, :, :tsz], ACT.Copy)
                nc.vector.scalar_tensor_tensor(
                    kflo[:, g_f0:g_f0 + ngl, :tsz], kfb[:, :, :tsz], 1.0,
                    kf8[:, g_f0:g_f0 + ngl, :tsz], ALU.mult, ALU.subtract)
            # FFN2 3t + final: out = x2 + sg*((Wcvh@(kf8+kflo)+Wcvl@kf8)/512)
            for c0, csz in _splits(Cc, CBLK):
                kvps = [psKV.tile([P, TS], F32, tag=f"kv_ps{i}",
                                  name=f"kv_ps{i}")
                        for i in range(csz // P)]
                nq = FG // FQ
                nmm_tot = nq * 3 * (FQ // 2)
                mm_idx = [0] * (csz // P)
                for q in range(nq):
                    f_lo = q * FQ
                    wbh = wpH.tile([P, FQ, CBLK], F8, tag="wf2h", name="wf2h")
                    nc.sync.dma_start(
                        out=wbh[:, :, :csz],
                        in_=wcvh[:, f_lo:f_lo + FQ, c0:c0 + csz])
                    wbl = wpH.tile([P, FQ, CBLK], F8, tag="wf2l", name="wf2l")
                    nc.sync.dma_start(
                        out=wbl[:, :, :csz],
                        in_=wcvl[:, f_lo:f_lo + FQ, c0:c0 + csz])
                    for gl in range(csz // P):
                        for wb, act in ((wbh, kf8), (wbh, kflo), (wbl, kf8)):
                            for fp in range(FQ // 2):
                                fg = f_lo + 2 * fp
                                nc.tensor.matmul(
                                    kvps[gl][:, :tsz],
                                    wb[:, 2 * fp:2 * fp + 2,
                                       gl * P:(gl + 1) * P],
                                    act[:, fg:fg + 2, :tsz],
                                    start=(mm_idx[gl] == 0),
                                    stop=(mm_idx[gl] == nmm_tot - 1),
                                    perf_mode=DR)
                                mm_idx[gl] += 1
                for gl in range(csz // P):
                    g_c = (c0 + gl * P) // P
                    sgs = wpH.tile([P, TS], BF16, tag="sgs", name="sgs")
                    nc.sync.dma_start(out=sgs[:, :tsz],
                                      in_=sgdv[:, g_c, t0:t0 + tsz])
                    ot = wpH.tile([P, TS], BF16, tag="ot", name="ot")
                    nc.vector.scalar_tensor_tensor(
                        ot[:, :tsz], kvps[gl][:, :tsz], 1.0 / (SKF * SW),
                        sgs[:, :tsz], ALU.mult, ALU.mult)
                    x2s = wpH.tile([P, TS], F32, tag="x2s", name="x2s")
                    nc.sync.dma_start(
                        out=x2s[:, :tsz],
                        in_=x2dv[:, g_c, 1 + t0:1 + t0 + tsz])
                    o2 = wpH.tile([P, TS], F32, tag="o2", name="o2")
                    nc.vector.tensor_tensor(o2[:, :tsz], ot[:, :tsz],
                                            x2s[:, :tsz], ALU.add)
                    nc.sync.dma_start(out=outTv[:, g_c, t0:t0 + tsz],
                                      in_=o2[:, :tsz])
            for p in (psKV, psH, wpH, sH, pH):
                p.release()
        pMx2.release()
        dram.release()
        const.release()

    nc.compile()
    return nc


_PROGRAM_CACHE = {}


def _get_program(key, **kw):
    if key not in _PROGRAM_CACHE:
        _PROGRAM_CACHE[key] = build_program(**kw)
    return _PROGRAM_CACHE[key]


def _q8pair(wT_scaled):
    """fp32 [128, KG, N] (already x SW) -> (hi, lo) e4m3 at the same scale."""
    hi = wT_scaled.astype(E4M3)
    lo = (wT_scaled - hi.astype(np.float32)).astype(E4M3)
    return hi, lo


def _host_prep(inputs, Cc=C, Dd=D_ATT, Ff=D_FFN, Bb=B, Tt=T, n_cores=N_CORES):
    P = 128
    CG, DG, FG = Cc // P, Dd // P, Ff // P
    half = Tt // 2
    RO, RS, R = half, half + 1, half + 2

    f = {k: np.asarray(v, np.float32) for k, v in inputs.items()}
    x = f["x"]

    def swz(wT, kg):  # [K, N] fp32 -> [128, kg, N] * SW
        Kdim, Ndim = wT.shape
        return np.ascontiguousarray(
            wT.reshape(kg, P, Ndim).transpose(1, 0, 2)) * SW

    wkh_, _ = _q8pair(swz(f["Wk"].T, CG))
    wvh_, wvl_ = _q8pair(swz(f["Wv"].T, CG))
    wrh_, _ = _q8pair(swz(f["Wr"].T, CG))
    woh_, wol_ = _q8pair(swz(f["Wo"].T, DG))
    wckh_, wckl_ = _q8pair(swz(f["Wck"].T, CG))
    wcvh_, wcvl_ = _q8pair(swz(f["Wcv"].T, FG))
    wcrh_, _ = _q8pair(swz(f["Wcr"].T, CG))

    def col(v):
        return np.ascontiguousarray(
            np.asarray(v, np.float32).reshape(-1).reshape(CG, P).T)

    ew = np.exp(-np.exp(f["time_decay"].astype(np.float64)))
    cvec_h = np.stack([
        col(f["ln1_w"] * SA), col(f["ln1_b"]),
        col(f["tm_k"]), col(f["tm_v"]), col(f["tm_r"]),
        col(ew.astype(np.float32)), col(np.exp(f["time_first"])),
        col(f["ln2_w"] * SA), col(f["ln2_b"]),
        col(f["cm_k"]), col(f["cm_r"]),
    ], axis=-1).astype(np.float32)

    in_maps = []
    for core in range(n_cores):
        b, hh = core // 2, core % 2
        t0 = hh * half
        xr = np.zeros((R, Cc), np.float32)
        lo = t0 - 2
        src_lo = max(lo, 0)
        xr[src_lo - lo:, :] = x[b, src_lo:t0 + RO, :]
        m0 = np.full((P, 1), float(hh), np.float32)
        sel = np.zeros((P, n_cores), np.float32)
        if hh == 1:
            sel[:, core - 1] = 1.0
        xrt = np.ascontiguousarray(xr.T)
        in_maps.append({
            "xT": xrt, "xTb": xrt.astype(ml_dtypes.bfloat16),
            "wkh": wkh_, "wvh": wvh_, "wvl": wvl_, "wrh": wrh_,
            "woh": woh_, "wol": wol_, "wckh": wckh_, "wckl": wckl_,
            "wcvh": wcvh_, "wcvl": wcvl_, "wcrh": wcrh_,
            "cvec": cvec_h, "m0": m0, "sel": sel,
        })
    return in_maps


def kernel(**inputs):
    in_maps = _host_prep(inputs)
    nc = _get_program("full")
    res = run_bass_kernel_spmd(nc, in_maps, core_ids=list(range(N_CORES)))
    half = T // 2
    out = np.empty((B, T, C), np.float32)
    for core in range(N_CORES):
        b, hh = core // 2, core % 2
        out[b, hh * half:(hh + 1) * half, :] = res.results[core]["outT"].T
    return out


# revision 7
# speedup vs baseline: 1.4019x; 1.0116x over previous
"""RWKV-4 block on 8 trn2 cores — fp8e4 DoubleRow version.

Sharding: 8 cores = 4 batch x 2 T-halves (as baseline). All big matmuls run
as fp8e4 DoubleRow (K=256/instr, 0.5 cyc/row). Precision scheme (measured
offline: final rel err ~1.2e-2 vs the 2e-2 gate):
  Wk, Wr, Wcr: pure fp8 (weights e4m3 x64, acts e4m3 x16)
  Wv, Wo:      2-term (weight hi+lo at the same scale; lo rides subnormals)
  Wck, Wcv:    3-term (weight hi+lo AND activation hi+lo, same scale)
Same-scale lo parts make every term share one PSUM scale, so all terms
accumulate natively in PSUM with no combine ops.
"""

import os
import sys

import numpy as np

for _p in ("/opt/trn_rl_repo", "/root/.axon_site/_ro/trn_rl_repo"):
    if os.path.isdir(_p) and _p not in sys.path:
        sys.path.insert(0, _p)

import ml_dtypes  # noqa: E402

import concourse.bass as bass  # noqa: E402,F401
import concourse.mybir as mybir  # noqa: E402
import concourse.tile as tile  # noqa: E402
from concourse import bacc  # noqa: E402
from concourse.bass_utils import run_bass_kernel_spmd  # noqa: E402

F32 = mybir.dt.float32
F32R = mybir.dt.float32r
BF16 = mybir.dt.bfloat16
F8 = mybir.dt.float8e4
ALU = mybir.AluOpType
ACT = mybir.ActivationFunctionType
DR = mybir.MatmulPerfMode.DoubleRow
E4M3 = ml_dtypes.float8_e4m3

B, T, C, D_ATT, D_FFN = 4, 2048, 2048, 2048, 8192
EPS = 1e-5
N_CORES = 8
DEN_EPS = 1e-30

SA = 16.0          # activation fp8 scale
SW = 64.0          # weight fp8 scale
SKF = 8.0          # kf fp8 scale
PS_INV = 1.0 / (SA * SW)     # psum -> true scale (2^-10)
SQ8 = float(np.sqrt(SKF))


def _splits(total, sz):
    return [(s, min(sz, total - s)) for s in range(0, total, sz)]


def _even_splits(total, mx):
    n = -(-total // mx)
    base, rem = divmod(total, n)
    out, s = [], 0
    for i in range(n):
        sz = base + (1 if i < rem else 0)
        out.append((s, sz))
        s += sz
    return out


def build_program(Cc=C, Dd=D_ATT, Ff=D_FFN, rows_out=T // 2, n_cores=N_CORES,
                  no_collective=False):
    P = 128
    CG, DG, FG = Cc // P, Dd // P, Ff // P
    RO = rows_out
    RS = RO + 1
    R = RS + 1
    RSP = -(-RS // 16) * 16   # fp8 moving tiles padded: pair stride %16 == 0
    NV = 11

    nc = bacc.Bacc("TRN2", target_bir_lowering=False, debug=False,
                   num_devices=n_cores)

    xT = nc.dram_tensor("xT", [Cc, R], F32, kind="ExternalInput").ap()
    xTb = nc.dram_tensor("xTb", [Cc, R], BF16, kind="ExternalInput").ap()
    wkh = nc.dram_tensor("wkh", [P, CG, Dd], F8, kind="ExternalInput").ap()
    wvh = nc.dram_tensor("wvh", [P, CG, Dd], F8, kind="ExternalInput").ap()
    wvl = nc.dram_tensor("wvl", [P, CG, Dd], F8, kind="ExternalInput").ap()
    wrh = nc.dram_tensor("wrh", [P, CG, Dd], F8, kind="ExternalInput").ap()
    woh = nc.dram_tensor("woh", [P, DG, Cc], F8, kind="ExternalInput").ap()
    wol = nc.dram_tensor("wol", [P, DG, Cc], F8, kind="ExternalInput").ap()
    wckh = nc.dram_tensor("wckh", [P, CG, Ff], F8, kind="ExternalInput").ap()
    wckl = nc.dram_tensor("wckl", [P, CG, Ff], F8, kind="ExternalInput").ap()
    wcvh = nc.dram_tensor("wcvh", [P, FG, Cc], F8, kind="ExternalInput").ap()
    wcvl = nc.dram_tensor("wcvl", [P, FG, Cc], F8, kind="ExternalInput").ap()
    wcrh = nc.dram_tensor("wcrh", [P, CG, Cc], F8, kind="ExternalInput").ap()
    cvec = nc.dram_tensor("cvec", [P, CG, NV], F32, kind="ExternalInput").ap()
    m0d = nc.dram_tensor("m0", [P, 1], F32, kind="ExternalInput").ap()
    seld = nc.dram_tensor("sel", [P, n_cores], F32, kind="ExternalInput").ap()
    outT = nc.dram_tensor("outT", [Cc, RO], F32, kind="ExternalOutput").ap()

    xTv = xT.rearrange("(g p) r -> p g r", p=P)
    xTbv = xTb.rearrange("(g p) r -> p g r", p=P)
    outTv = outT.rearrange("(g p) r -> p g r", p=P)

    I_LN1W, I_LN1B, I_TMK, I_TMV, I_TMR, I_EW, I_EU, I_LN2W, I_LN2B, \
        I_CMK, I_CMR = range(NV)

    TS = 512
    LTS = 256

    with tile.TileContext(nc) as tc:
        const = tc.alloc_tile_pool(name="const", bufs=1)
        con = const.tile([P, CG, NV], F32, tag="con")
        nc.sync.dma_start(out=con[:], in_=cvec)
        m0 = const.tile([P, 1], F32, tag="m0")
        nc.sync.dma_start(out=m0[:], in_=m0d)
        selt = const.tile([P, n_cores], F32, tag="sel")
        nc.sync.dma_start(out=selt[:], in_=seld)
        onesc = const.tile([P, 1], F32, tag="ones")
        nc.vector.memset(onesc[:], 1.0)
        onesb = const.tile([P, 1], BF16, tag="onesb")
        nc.vector.memset(onesb[:], 1.0)
        epsc = const.tile([1, 1], F32, tag="epsc")
        nc.vector.memset(epsc[:], EPS)
        onesPb = const.tile([1, P], BF16, tag="onesPb")
        nc.vector.memset(onesPb[:], 1.0)

        def ccol(g, i):
            return con[:, g, i:i + 1]

        dram = tc.alloc_tile_pool(name="dram", bufs=1, space="DRAM")
        x2dram = dram.tile([Cc, RS], F32)
        x2dv = x2dram.rearrange("(g p) r -> p g r", p=P)
        srdram = dram.tile([Dd, RS], BF16)
        srdv = srdram.rearrange("(g p) r -> p g r", p=P)
        sgdram = dram.tile([Cc, RO], BF16)
        sgdv = sgdram.rearrange("(g p) r -> p g r", p=P)
        cc_in = dram.tile([P, 2 * DG], F32)
        cc_out = dram.tile([P * n_cores, 2 * DG], F32)

        # ---- LayerNorm (streaming; PE sums via f32r bitcast) ----
        def ln_stream(src_v, nrows, iw, out_sb, name, sbuf_src=False,
                      src_bf16=False, lts=None):
            LTS = lts or 256
            src_dt = BF16 if src_bf16 else F32
            st = tc.alloc_tile_pool(name=f"{name}_st", bufs=1)
            sp = tc.alloc_tile_pool(name=f"{name}_sp", bufs=2)
            spx = tc.alloc_tile_pool(name=f"{name}_spx", bufs=8)
            psum = tc.alloc_tile_pool(name=f"{name}_ps", bufs=2, space="PSUM")
            ssum = st.tile([1, nrows], F32, tag="sum", name="ssum")
            ssq = st.tile([1, nrows], F32, tag="sq", name="ssq")
            for t0, tsz in _splits(nrows, LTS):
                if sbuf_src:
                    xls = src_v[:, :, t0:t0 + tsz]
                else:
                    xlt = sp.tile([P, CG, LTS], src_dt, tag="xls",
                                  name="xls")
                    nc.sync.dma_start(out=xlt[:, :, :tsz],
                                      in_=src_v[:, :, t0:t0 + tsz])
                    xls = xlt[:, :, :tsz]
                xsq = sp.tile([P, CG, LTS], BF16, tag="lnsq", name="xsq")
                nc.scalar.activation(xsq[:, :, :tsz], xls,
                                     ACT.Square)
                ps = psum.tile([1, LTS], F32, tag="ln_ps", name="ps")
                ps2 = psum.tile([1, LTS], F32, tag="ln_ps2", name="ps2")
                for g in range(CG):
                    nc.tensor.matmul(
                        ps[:, :tsz], onesb[:], xls[:, g, :],
                        start=(g == 0), stop=(g == CG - 1))
                    nc.tensor.matmul(
                        ps2[:, :tsz], onesb[:], xsq[:, g, :tsz],
                        start=(g == 0), stop=(g == CG - 1))
                nc.vector.tensor_copy(out=ssum[:, t0:t0 + tsz],
                                      in_=ps[:, :tsz])
                nc.vector.tensor_copy(out=ssq[:, t0:t0 + tsz],
                                      in_=ps2[:, :tsz])
            mu = st.tile([1, nrows], BF16, tag="mu", name="mu")
            rstd = st.tile([1, nrows], BF16, tag="rstd", name="rstd")
            var = st.tile([1, nrows], F32, tag="var", name="var")
            musq = st.tile([1, nrows], F32, tag="musq", name="musq")
            nc.vector.tensor_scalar_mul(mu[:], ssum[:], 1.0 / Cc)
            nc.vector.tensor_scalar_mul(var[:], ssq[:], 1.0 / Cc)
            nc.vector.tensor_tensor(musq[:], mu[:], mu[:], ALU.mult)
            nc.vector.tensor_tensor(var[:], var[:], musq[:], ALU.subtract)
            nc.scalar.activation(var[:], var[:], ACT.Ln, bias=epsc[:])
            nc.scalar.activation(rstd[:], var[:], ACT.Exp, scale=-0.5)
            for t0, tsz in _splits(nrows, LTS):
                if sbuf_src:
                    xls = src_v[:, :, t0:t0 + tsz]
                else:
                    xlt = sp.tile([P, CG, LTS], src_dt, tag="xls",
                                  name="xls")
                    nc.sync.dma_start(out=xlt[:, :, :tsz],
                                      in_=src_v[:, :, t0:t0 + tsz])
                    xls = xlt[:, :, :tsz]
                mups = psum.tile([P, LTS], F32, tag="mups", name="mups")
                nc.tensor.matmul(mups[:, :tsz], onesPb[:],
                                 mu[:, t0:t0 + tsz],
                                 start=True, stop=True)
                rsps = psum.tile([P, LTS], F32, tag="rsps", name="rsps")
                nc.tensor.matmul(rsps[:, :tsz], onesPb[:],
                                 rstd[:, t0:t0 + tsz],
                                 start=True, stop=True)
                for g in range(CG):
                    xm = spx.tile([P, LTS], BF16, tag="ln_xm", name="xm")
                    nc.vector.tensor_tensor(xm[:, :tsz], xls[:, g, :],
                                            mups[:, :tsz], ALU.subtract)
                    nc.vector.scalar_tensor_tensor(
                        out_sb[:, g, t0:t0 + tsz], xm[:, :tsz], ccol(g, iw),
                        rsps[:, :tsz], ALU.mult, ALU.mult)
            for p in (psum, spx, sp, st):
                p.release()

        # ================= Phase A: LN1 (h = 16*ln(x), bf16) ============
        pEk = tc.alloc_tile_pool(name="pEk", bufs=1)
        eksb = [pEk.tile([P, RS], BF16, tag=f"eksb{g}", name=f"eksb{g}")
                for g in range(DG)]
        ekvsb = [pEk.tile([P, RS], BF16, tag=f"ekvsb{g}", name=f"ekvsb{g}")
                 for g in range(DG)]
        pMix = tc.alloc_tile_pool(name="pMix", bufs=1)
        mixk8 = [pMix.tile([P, 2, RSP], F8, tag=f"mixk8_{p}",
                           name=f"mixk8_{p}") for p in range(CG // 2)]
        mixv8 = [pMix.tile([P, 2, RSP], F8, tag=f"mixv8_{p}",
                           name=f"mixv8_{p}") for p in range(CG // 2)]
        mixr8 = [pMix.tile([P, 2, RSP], F8, tag=f"mixr8_{p}",
                           name=f"mixr8_{p}") for p in range(CG // 2)]
        pHs = tc.alloc_tile_pool(name="pHs", bufs=1)
        hs = pHs.tile([P, CG, R], BF16, tag="hs")
        ln_stream(xTbv, R, I_LN1W, hs, "ln1", src_bf16=True)
        nc.vector.tensor_scalar_mul(hs[:, :, 0:2], hs[:, :, 0:2], m0[:])

        # ========== Phase B: mixes (fp8 x16) + k/v/r DR matmuls ========
        stg = tc.alloc_tile_pool(name="stg", bufs=4)
        if RSP > RS:
            for mixl in (mixk8, mixv8, mixr8):
                for mt in mixl:
                    nc.vector.memset(mt[:, :, RS:RSP], 0.0)
        MSTRIPS = [(0, 512), (512, RS - 512)]
        for t0, tsz in MSTRIPS:
            for g in range(CG):
                dmix = stg.tile([P, 512 + 1], BF16, tag="dmix", name="dmix")
                nc.vector.tensor_tensor(
                    dmix[:, :tsz], hs[:, g, 1 + t0:1 + t0 + tsz],
                    hs[:, g, t0:t0 + tsz], ALU.subtract)
                for mixl, icoef, on_act in ((mixk8, I_TMK, True),
                                            (mixv8, I_TMV, False),
                                            (mixr8, I_TMR, True)):
                    mb16 = stg.tile([P, 512 + 1], BF16, tag="mb16",
                                    name="mb16")
                    nc.vector.scalar_tensor_tensor(
                        mb16[:, :tsz], dmix[:, :tsz], ccol(g, icoef),
                        hs[:, g, t0:t0 + tsz], ALU.mult, ALU.add)
                    dst = mixl[g // 2][:, g % 2, t0:t0 + tsz]
                    if on_act:
                        nc.scalar.activation(dst, mb16[:, :tsz], ACT.Copy)
                    else:
                        nc.gpsimd.tensor_copy(out=dst, in_=mb16[:, :tsz])
        wpB = tc.alloc_tile_pool(name="wpB", bufs=2)
        stgE = tc.alloc_tile_pool(name="stgE", bufs=4)
        psB = tc.alloc_tile_pool(name="psB", bufs=6, space="PSUM")
        DBLK = 512
        tstripsB = [(0, 512), (512, 512), (1024, RSP - 1024)]

        def mm_dr(whd, wld, rhs8, n_out, evict, wtag, strips=None):
            for d0, dsz in _splits(n_out, DBLK):
                wbh = wpB.tile([P, CG, DBLK], F8, tag="wh", name="wbh")
                nc.sync.dma_start(out=wbh[:, :, :dsz],
                                  in_=whd[:, :, d0:d0 + dsz])
                if wld is not None:
                    wbl = wpB.tile([P, CG, DBLK], F8, tag="wl",
                                   name="wbl")
                    nc.sync.dma_start(out=wbl[:, :, :dsz],
                                      in_=wld[:, :, d0:d0 + dsz])
                wbufs = [wbh] if wld is None else [wbh, wbl]
                for gl in range(dsz // P):
                    g_out = (d0 + gl * P) // P
                    for t0, tsz in (strips or tstripsB):
                        wsz = min(tsz, RS - t0)
                        if wsz <= 0:
                            continue
                        ps = psB.tile([P, TS], F32, tag="mm_ps", name="mm_ps")
                        nmm = len(wbufs) * (CG // 2)
                        i = 0
                        for wb in wbufs:
                            for gp in range(CG // 2):
                                nc.tensor.matmul(
                                    ps[:, :tsz],
                                    wb[:, 2 * gp:2 * gp + 2,
                                       gl * P:(gl + 1) * P],
                                    rhs8[gp][:, :, t0:t0 + tsz],
                                    start=(i == 0), stop=(i == nmm - 1),
                                    perf_mode=DR)
                                i += 1
                        evict(g_out, t0, wsz, ps)

        def evict_k(g, t0, wsz, ps):
            nc.scalar.activation(eksb[g][:, t0:t0 + wsz], ps[:, :wsz],
                                 ACT.Exp, scale=PS_INV)
            if t0 == 0:
                nc.vector.tensor_scalar_mul(eksb[g][:, 0:1], eksb[g][:, 0:1],
                                            m0[:])

        def evict_v(g, t0, wsz, ps):
            nc.vector.scalar_tensor_tensor(
                ekvsb[g][:, t0:t0 + wsz], ps[:, :wsz], PS_INV,
                eksb[g][:, t0:t0 + wsz], ALU.mult, ALU.mult)

        def evict_r(g, t0, wsz, ps):
            srt = stgE.tile([P, TS], BF16, tag="srt", name="srt")
            nc.scalar.activation(srt[:, :wsz], ps[:, :wsz], ACT.Sigmoid,
                                 scale=PS_INV)
            nc.sync.dma_start(out=srdv[:, g, t0:t0 + wsz], in_=srt[:, :wsz])

        mm_dr(wkh, None, mixk8, Dd, evict_k, "wk", strips=tstripsB[:1])
        mm_dr(wkh, None, mixk8, Dd, evict_k, "wk", strips=tstripsB[1:])
        mm_dr(wvh, wvl, mixv8, Dd, evict_v, "wv")
        mm_dr(wrh, None, mixr8, Dd, evict_r, "wr")

        psB.release()
        stgE.release()
        wpB.release()
        stg.release()
        pHs.release()
        pMix.release()

        # ====== Phase C: boundary states (bf16 scans) + AllGather =======
        pRw = tc.alloc_tile_pool(name="pRw", bufs=1, side="right")
        rwkv8 = [pRw.tile([P, 2, RSP], F8, tag=f"rw{p}", name=f"rw{p}")
                 for p in range(DG // 2)]
        if RSP > RS:
            for rwt in rwkv8:
                nc.vector.memset(rwt[:, :, RS:RSP], 0.0)
        wpE = tc.alloc_tile_pool(name="wpE", bufs=2, side="right")
        spE = tc.alloc_tile_pool(name="spE", bufs=2, side="right")
        pC = tc.alloc_tile_pool(name="pC", bufs=2, side="right")
        state = pC.tile([P, 2 * DG], F32, tag="state", name="state")
        for g in range(DG):
            ewbc = ccol(g, I_EW).to_broadcast([P, RS - 1])
            apre = pC.tile([P, RS - 1], BF16, tag="apre", name="apre")
            nc.vector.tensor_tensor_scan(
                apre[:], ewbc, ekvsb[g][:, :RS - 1], 0.0, ALU.mult, ALU.add)
            nc.gpsimd.tensor_copy(out=state[:, g:g + 1],
                                  in_=apre[:, RS - 2:RS - 1])
            bpre = pC.tile([P, RS - 1], BF16, tag="bpre", name="bpre")
            nc.vector.tensor_tensor_scan(
                bpre[:], ewbc, eksb[g][:, :RS - 1], 0.0, ALU.mult, ALU.add)
            nc.gpsimd.tensor_copy(out=state[:, DG + g:DG + g + 1],
                                  in_=bpre[:, RS - 2:RS - 1])
        nc.sync.dma_start(out=cc_in[:], in_=state[:])
        if not no_collective:
            nc.gpsimd.collective_compute(
                "AllGather", ALU.bypass,
                replica_groups=[list(range(n_cores))],
                ins=[cc_in[:].opt()], outs=[cc_out[:].opt()])
        else:
            for jj in range(n_cores):
                nc.sync.dma_start(out=cc_out[jj * P:(jj + 1) * P, :],
                                  in_=cc_in[:])
        gsb = pC.tile([P, n_cores, 2 * DG], F32, tag="gsb", name="gsb")
        nc.sync.dma_start(
            out=gsb[:], in_=cc_out[:].rearrange("(j p) s -> p j s", p=P))
        a0b0 = pC.tile([P, 2 * DG], F32, tag="a0b0", name="a0b0")
        nc.vector.memset(a0b0[:, 0:DG], 0.0)
        nc.vector.memset(a0b0[:, DG:2 * DG], DEN_EPS)
        for j in range(n_cores):
            nc.vector.scalar_tensor_tensor(
                a0b0[:], gsb[:, j, :], selt[:, j:j + 1], a0b0[:],
                ALU.mult, ALU.add)

        # ============ Phase D: WKV scans + rwkv (fp8 x16) ============
        pD = tc.alloc_tile_pool(name="pD", bufs=3)

        def d_front(g):
            ekg = eksb[g][:]
            xkg = ekvsb[g][:]
            srg = pD.tile([P, RS], BF16, tag="srg", name="srg")
            nc.sync.dma_start(out=srg[:], in_=srdv[:, g, :])
            ewb = pD.tile([P, RS], BF16, tag="ewb", name="ewb")
            nc.scalar.activation(ewb[:], ccol(g, I_EW).to_broadcast([P, RS]),
                                 ACT.Copy)
            eub = pD.tile([P, RS], BF16, tag="eub", name="eub")
            nc.scalar.activation(eub[:], ccol(g, I_EU).to_broadcast([P, RS]),
                                 ACT.Copy)
            abuf = pD.tile([P, RS + 1], BF16, tag="abuf", name="abuf")
            nc.gpsimd.tensor_copy(out=abuf[:, 0:1], in_=a0b0[:, g:g + 1])
            nc.vector.tensor_tensor_scan(
                abuf[:, 1:RS + 1], ewb[:], xkg, a0b0[:, g:g + 1],
                ALU.mult, ALU.add)
            bbuf = pD.tile([P, RS + 1], BF16, tag="bbuf", name="bbuf")
            nc.gpsimd.tensor_copy(out=bbuf[:, 0:1],
                                  in_=a0b0[:, DG + g:DG + g + 1])
            nc.vector.tensor_tensor_scan(
                bbuf[:, 1:RS + 1], ewb[:], ekg,
                a0b0[:, DG + g:DG + g + 1], ALU.mult, ALU.add)
            num = pD.tile([P, RS], BF16, tag="num", name="num")
            nc.vector.scalar_tensor_tensor(
                num[:], xkg, ccol(g, I_EU), abuf[:, 0:RS],
                ALU.mult, ALU.add)
            snum = pD.tile([P, RS], BF16, tag="snum", name="snum")
            nc.vector.tensor_tensor(snum[:], num[:], srg[:], ALU.mult)
            t1 = pD.tile([P, RS], BF16, tag="t1", name="t1")
            nc.gpsimd.tensor_tensor(t1[:], ekg, eub[:], ALU.mult)
            den = pD.tile([P, RS], F32, tag="den", name="den")
            nc.gpsimd.tensor_tensor(den[:], t1[:], bbuf[:, 0:RS], ALU.add)
            return snum, den

        def d_back(g, snum, den):
            rden = pD.tile([P, RS], F32, tag="rden", name="rden")
            nc.vector.reciprocal_approx_fast(out=rden[:], in_=den[:])
            nc.vector.scalar_tensor_tensor(
                rwkv8[g // 2][:, g % 2, :RS], snum[:], SA, rden[:],
                ALU.mult, ALU.mult)

        pend = []
        for g in range(DG):
            pend.append((g, d_front(g)))
            if len(pend) > 3:
                gq, fq = pend.pop(0)
                d_back(gq, *fq)
        for gq, fq in pend:
            d_back(gq, *fq)
        pD.release()
        pEk.release()
        pC.release()
        pMx2 = tc.alloc_tile_pool(name="pMx2", bufs=1)
        xk2h = pMx2.tile([P, CG, RO], F8, tag="xk2h")
        xk2l = pMx2.tile([P, CG, RO], F8, tag="xk2l")
        pXr = tc.alloc_tile_pool(name="pXr", bufs=1)
        xr28 = pXr.tile([P, CG, RO], F8, tag="xr28")
        wpG = tc.alloc_tile_pool(name="wpG", bufs=2)
        spG = tc.alloc_tile_pool(name="spG", bufs=2)
        pX2 = tc.alloc_tile_pool(name="pX2", bufs=1)
        x2bf = pX2.tile([P, CG, RS], BF16, tag="x2bf")

        # ========= Phase E: Wo (2t DR) -> x2 = x + attn (DRAM) =========
        psE = tc.alloc_tile_pool(name="psE", bufs=3, space="PSUM")
        CBLK = 512
        for c0, csz in _splits(Cc, CBLK):
            wbh = wpE.tile([P, DG, CBLK], F8, tag="woh", name="woh")
            nc.sync.dma_start(out=wbh[:, :, :csz], in_=woh[:, :, c0:c0 + csz])
            wbl = wpE.tile([P, DG, CBLK], F8, tag="wol", name="wol")
            nc.sync.dma_start(out=wbl[:, :, :csz], in_=wol[:, :, c0:c0 + csz])
            for gl in range(csz // P):
                g_c = (c0 + gl * P) // P
                for t0, tsz in tstripsB:
                    wsz = min(tsz, RS - t0)
                    if wsz <= 0:
                        continue
                    ps = psE.tile([P, TS], F32, tag="wo_ps", name="wo_ps")
                    i = 0
                    for wb in (wbh, wbl):
                        for gp in range(DG // 2):
                            nc.tensor.matmul(
                                ps[:, :tsz],
                                wb[:, 2 * gp:2 * gp + 2, gl * P:(gl + 1) * P],
                                rwkv8[gp][:, :, t0:t0 + tsz],
                                start=(i == 0), stop=(i == DG - 1),
                                perf_mode=DR)
                            i += 1
                    xst = spE.tile([P, TS], F32, tag="xst", name="xst")
                    nc.sync.dma_start(
                        out=xst[:, :wsz],
                        in_=xTv[:, g_c, 1 + t0:1 + t0 + wsz])
                    x2st = spE.tile([P, TS], F32, tag="x2st", name="x2st")
                    nc.vector.scalar_tensor_tensor(
                        x2st[:, :wsz], ps[:, :wsz], PS_INV,
                        xst[:, :wsz], ALU.mult, ALU.add)
                    nc.sync.dma_start(out=x2dv[:, g_c, t0:t0 + wsz],
                                      in_=x2st[:, :wsz])
                    nc.gpsimd.tensor_copy(out=x2bf[:, g_c, t0:t0 + wsz],
                                          in_=x2st[:, :wsz])
        psE.release()
        spE.release()
        wpE.release()
        pRw.release()

        # ====== Phase F: LN2 + mixes2 (xk2 hi/lo fp8, xr2 fp8) ======
        pG2 = tc.alloc_tile_pool(name="pG2", bufs=1)
        g2 = pG2.tile([P, CG, RS], BF16, tag="g2")
        ln_stream(x2bf, RS, I_LN2W, g2, "ln2", sbuf_src=True, lts=512)
        nc.vector.tensor_scalar_mul(g2[:, :, 0:1], g2[:, :, 0:1], m0[:])

        spF = tc.alloc_tile_pool(name="spF", bufs=3)
        for g in range(CG):
            dmix = spF.tile([P, RO], BF16, tag="dmix2", name="dmix2")
            nc.vector.tensor_tensor(dmix[:], g2[:, g, 1:RS],
                                    g2[:, g, 0:RO], ALU.subtract)
            xr2b = spF.tile([P, RO], BF16, tag="xr2b", name="xr2b")
            nc.vector.scalar_tensor_tensor(
                xr2b[:], dmix[:], ccol(g, I_CMR), g2[:, g, 0:RO],
                ALU.mult, ALU.add)
            nc.gpsimd.tensor_copy(out=xr28[:, g, :], in_=xr2b[:])
        for g in range(CG):
            dmix = spF.tile([P, RO], BF16, tag="dmix2", name="dmix2")
            nc.vector.tensor_tensor(dmix[:], g2[:, g, 1:RS],
                                    g2[:, g, 0:RO], ALU.subtract)
            xk2b = spF.tile([P, RO], BF16, tag="xk2b", name="xk2b")
            nc.vector.scalar_tensor_tensor(
                xk2b[:], dmix[:], ccol(g, I_CMK), g2[:, g, 0:RO],
                ALU.mult, ALU.add)
            nc.scalar.activation(xk2h[:, g, :], xk2b[:], ACT.Copy)
            dif = spF.tile([P, RO], BF16, tag="dif", name="dif")
            nc.vector.tensor_tensor(dif[:], xk2b[:], xk2h[:, g, :],
                                    ALU.subtract)
            nc.scalar.activation(xk2l[:, g, :], dif[:], ACT.Copy)
        spF.release()
        pG2.release()
        pX2.release()

        # ====== Phase G: r2 = sigmoid(xr2 @ WcrT) (pure DR) -> DRAM =====
        psG = tc.alloc_tile_pool(name="psG", bufs=4, space="PSUM")
        for c0, csz in _splits(Cc, CBLK):
            wbh = wpG.tile([P, CG, CBLK], F8, tag="wcr", name="wcr")
            nc.sync.dma_start(out=wbh[:, :, :csz], in_=wcrh[:, :, c0:c0 + csz])
            for gl in range(csz // P):
                g_c = (c0 + gl * P) // P
                for t0, tsz in _splits(RO, TS):
                    ps = psG.tile([P, TS], F32, tag="wcr_ps", name="wcr_ps")
                    for gp in range(CG // 2):
                        nc.tensor.matmul(
                            ps[:, :tsz],
                            wbh[:, 2 * gp:2 * gp + 2, gl * P:(gl + 1) * P],
                            xr28[:, 2 * gp:2 * gp + 2, t0:t0 + tsz],
                            start=(gp == 0), stop=(gp == CG // 2 - 1),
                            perf_mode=DR)
                    sgt = spG.tile([P, TS], BF16, tag="sgt", name="sgt")
                    nc.scalar.activation(sgt[:, :tsz], ps[:, :tsz],
                                         ACT.Sigmoid, scale=PS_INV)
                    nc.sync.dma_start(out=sgdv[:, g_c, t0:t0 + tsz],
                                      in_=sgt[:, :tsz])
        psG.release()
        spG.release()
        wpG.release()
        pXr.release()

        # ============ Phase H: FFN (3t DR both matmuls) ============
        FBLK = 512
        FQ = 16
        for t0, tsz in _splits(RO, TS):
            pH = tc.alloc_tile_pool(name=f"pH{t0}", bufs=1)
            sH = tc.alloc_tile_pool(name=f"sH{t0}", bufs=2)
            wpH = tc.alloc_tile_pool(name=f"wpH{t0}", bufs=2)
            psH = tc.alloc_tile_pool(name=f"psH{t0}", bufs=4, space="PSUM")
            psKV = tc.alloc_tile_pool(name=f"psKV{t0}", bufs=1, space="PSUM")
            kf8 = pH.tile([P, FG, TS], F8, tag="kf8", name="kf8")
            kflo = pH.tile([P, FG, TS], F8, tag="kflo", name="kflo")
            # FFN1 3t: z = Wckh@(xh+xl) + Wckl@xh; trl = sqrt(8)*relu(z)
            for f0, fsz in _splits(Ff, FBLK):
                wbh = wpH.tile([P, CG, FBLK], F8, tag="wfh", name="wfh")
                nc.sync.dma_start(out=wbh[:, :, :fsz],
                                  in_=wckh[:, :, f0:f0 + fsz])
                wbl = wpH.tile([P, CG, FBLK], F8, tag="wfl", name="wfl")
                nc.sync.dma_start(out=wbl[:, :, :fsz],
                                  in_=wckl[:, :, f0:f0 + fsz])
                ngl = fsz // P
                trl = sH.tile([P, ngl, TS], BF16, tag="trl", name="trl")
                for fl in range(ngl):
                    ps = psH.tile([P, TS], F32, tag="ffn1_ps", name="ffn1_ps")
                    i = 0
                    nmm = 3 * (CG // 2)
                    for wb, act in ((wbh, xk2h), (wbh, xk2l), (wbl, xk2h)):
                        for gp in range(CG // 2):
                            nc.tensor.matmul(
                                ps[:, :tsz],
                                wb[:, 2 * gp:2 * gp + 2, fl * P:(fl + 1) * P],
                                act[:, 2 * gp:2 * gp + 2, t0:t0 + tsz],
                                start=(i == 0), stop=(i == nmm - 1),
                                perf_mode=DR)
                            i += 1
                    nc.scalar.activation(trl[:, fl, :tsz], ps[:, :tsz],
                                         ACT.Relu, scale=PS_INV * SQ8)
                # kfb = trl^2 = 8*kf; kf8 = e4m3(kfb); kflo = kfb - kf8
                g_f0 = f0 // P
                kfb = sH.tile([P, ngl, TS], BF16, tag="kfb", name="kfb")
                nc.vector.tensor_tensor(kfb[:, :, :tsz], trl[:, :, :tsz],
                                        trl[:, :, :tsz], ALU.mult)
                nc.scalar.activation(kf8[:, g_f0:g_f0 + ngl, :tsz],
                                     kfb[:, :, :tsz], ACT.Copy)
                nc.vector.scalar_tensor_tensor(
                    kflo[:, g_f0:g_f0 + ngl, :tsz], kfb[:, :, :tsz], 1.0,
                    kf8[:, g_f0:g_f0 + ngl, :tsz], ALU.mult, ALU.subtract)
            # FFN2 3t + final: out = x2 + sg*((Wcvh@(kf8+kflo)+Wcvl@kf8)/512)
            for c0, csz in _splits(Cc, CBLK):
                kvps = [psKV.tile([P, TS], F32, tag=f"kv_ps{i}",
                                  name=f"kv_ps{i}")
                        for i in range(csz // P)]
                nq = FG // FQ
                nmm_tot = nq * 3 * (FQ // 2)
                mm_idx = [0] * (csz // P)
                for q in range(nq):
                    f_lo = q * FQ
                    wbh = wpH.tile([P, FQ, CBLK], F8, tag="wf2h", name="wf2h")
                    nc.sync.dma_start(
                        out=wbh[:, :, :csz],
                        in_=wcvh[:, f_lo:f_lo + FQ, c0:c0 + csz])
                    wbl = wpH.tile([P, FQ, CBLK], F8, tag="wf2l", name="wf2l")
                    nc.sync.dma_start(
                        out=wbl[:, :, :csz],
                        in_=wcvl[:, f_lo:f_lo + FQ, c0:c0 + csz])
                    for gl in range(csz // P):
                        for wb, act in ((wbh, kf8), (wbh, kflo), (wbl, kf8)):
                            for fp in range(FQ // 2):
                                fg = f_lo + 2 * fp
                                nc.tensor.matmul(
                                    kvps[gl][:, :tsz],
                                    wb[:, 2 * fp:2 * fp + 2,
                                       gl * P:(gl + 1) * P],
                                    act[:, fg:fg + 2, :tsz],
                                    start=(mm_idx[gl] == 0),
                                    stop=(mm_idx[gl] == nmm_tot - 1),
                                    perf_mode=DR)
                                mm_idx[gl] += 1
                for gl in range(csz // P):
                    g_c = (c0 + gl * P) // P
                    sgs = wpH.tile([P, TS], BF16, tag="sgs", name="sgs")
                    nc.sync.dma_start(out=sgs[:, :tsz],
                                      in_=sgdv[:, g_c, t0:t0 + tsz])
                    ot = wpH.tile([P, TS], BF16, tag="ot", name="ot")
                    nc.vector.scalar_tensor_tensor(
                        ot[:, :tsz], kvps[gl][:, :tsz], 1.0 / (SKF * SW),
                        sgs[:, :tsz], ALU.mult, ALU.mult)
                    x2s = wpH.tile([P, TS], F32, tag="x2s", name="x2s")
                    nc.sync.dma_start(
                        out=x2s[:, :tsz],
                        in_=x2dv[:, g_c, 1 + t0:1 + t0 + tsz])
                    o2 = wpH.tile([P, TS], F32, tag="o2", name="o2")
                    nc.vector.tensor_tensor(o2[:, :tsz], ot[:, :tsz],
                                            x2s[:, :tsz], ALU.add)
                    nc.sync.dma_start(out=outTv[:, g_c, t0:t0 + tsz],
                                      in_=o2[:, :tsz])
            for p in (psKV, psH, wpH, sH, pH):
                p.release()
        pMx2.release()
        dram.release()
        const.release()

    nc.compile()
    return nc


_PROGRAM_CACHE = {}


def _get_program(key, **kw):
    if key not in _PROGRAM_CACHE:
        _PROGRAM_CACHE[key] = build_program(**kw)
    return _PROGRAM_CACHE[key]


def _q8pair(wT_scaled):
    """fp32 [128, KG, N] (already x SW) -> (hi, lo) e4m3 at the same scale."""
    hi = wT_scaled.astype(E4M3)
    lo = (wT_scaled - hi.astype(np.float32)).astype(E4M3)
    return hi, lo


def _host_prep(inputs, Cc=C, Dd=D_ATT, Ff=D_FFN, Bb=B, Tt=T, n_cores=N_CORES):
    P = 128
    CG, DG, FG = Cc // P, Dd // P, Ff // P
    half = Tt // 2
    RO, RS, R = half, half + 1, half + 2

    f = {k: np.asarray(v, np.float32) for k, v in inputs.items()}
    x = f["x"]

    def swz(wT, kg):  # [K, N] fp32 -> [128, kg, N] * SW
        Kdim, Ndim = wT.shape
        return np.ascontiguousarray(
            wT.reshape(kg, P, Ndim).transpose(1, 0, 2)) * SW

    wkh_, _ = _q8pair(swz(f["Wk"].T, CG))
    wvh_, wvl_ = _q8pair(swz(f["Wv"].T, CG))
    wrh_, _ = _q8pair(swz(f["Wr"].T, CG))
    woh_, wol_ = _q8pair(swz(f["Wo"].T, DG))
    wckh_, wckl_ = _q8pair(swz(f["Wck"].T, CG))
    wcvh_, wcvl_ = _q8pair(swz(f["Wcv"].T, FG))
    wcrh_, _ = _q8pair(swz(f["Wcr"].T, CG))

    def col(v):
        return np.ascontiguousarray(
            np.asarray(v, np.float32).reshape(-1).reshape(CG, P).T)

    ew = np.exp(-np.exp(f["time_decay"].astype(np.float64)))
    cvec_h = np.stack([
        col(f["ln1_w"] * SA), col(f["ln1_b"]),
        col(f["tm_k"]), col(f["tm_v"]), col(f["tm_r"]),
        col(ew.astype(np.float32)), col(np.exp(f["time_first"])),
        col(f["ln2_w"] * SA), col(f["ln2_b"]),
        col(f["cm_k"]), col(f["cm_r"]),
    ], axis=-1).astype(np.float32)

    in_maps = []
    for core in range(n_cores):
        b, hh = core // 2, core % 2
        t0 = hh * half
        xr = np.zeros((R, Cc), np.float32)
        lo = t0 - 2
        src_lo = max(lo, 0)
        xr[src_lo - lo:, :] = x[b, src_lo:t0 + RO, :]
        m0 = np.full((P, 1), float(hh), np.float32)
        sel = np.zeros((P, n_cores), np.float32)
        if hh == 1:
            sel[:, core - 1] = 1.0
        xrt = np.ascontiguousarray(xr.T)
        in_maps.append({
            "xT": xrt, "xTb": xrt.astype(ml_dtypes.bfloat16),
            "wkh": wkh_, "wvh": wvh_, "wvl": wvl_, "wrh": wrh_,
            "woh": woh_, "wol": wol_, "wckh": wckh_, "wckl": wckl_,
            "wcvh": wcvh_, "wcvl": wcvl_, "wcrh": wcrh_,
            "cvec": cvec_h, "m0": m0, "sel": sel,
        })
    return in_maps


def kernel(**inputs):
    in_maps = _host_prep(inputs)
    nc = _get_program("full")
    res = run_bass_kernel_spmd(nc, in_maps, core_ids=list(range(N_CORES)))
    half = T // 2
    out = np.empty((B, T, C), np.float32)
    for core in range(N_CORES):
        b, hh = core // 2, core % 2
        out[b, hh * half:(hh + 1) * half, :] = res.results[core]["outT"].T
    return out


# revision 8
# speedup vs baseline: 1.4926x; 1.0647x over previous
"""RWKV-4 block on 8 trn2 cores — fp8e4 DoubleRow version.

Sharding: 8 cores = 4 batch x 2 T-halves (as baseline). All big matmuls run
as fp8e4 DoubleRow (K=256/instr, 0.5 cyc/row). Precision scheme (measured
offline: final rel err ~1.2e-2 vs the 2e-2 gate):
  Wk, Wr, Wcr: pure fp8 (weights e4m3 x64, acts e4m3 x16)
  Wv, Wo:      2-term (weight hi+lo at the same scale; lo rides subnormals)
  Wck, Wcv:    3-term (weight hi+lo AND activation hi+lo, same scale)
Same-scale lo parts make every term share one PSUM scale, so all terms
accumulate natively in PSUM with no combine ops.
"""

import os
import sys

import numpy as np

for _p in ("/opt/trn_rl_repo", "/root/.axon_site/_ro/trn_rl_repo"):
    if os.path.isdir(_p) and _p not in sys.path:
        sys.path.insert(0, _p)

import ml_dtypes  # noqa: E402

import concourse.bass as bass  # noqa: E402,F401
import concourse.mybir as mybir  # noqa: E402
import concourse.tile as tile  # noqa: E402
from concourse import bacc  # noqa: E402
from concourse.bass_utils import run_bass_kernel_spmd  # noqa: E402

F32 = mybir.dt.float32
F32R = mybir.dt.float32r
BF16 = mybir.dt.bfloat16
F8 = mybir.dt.float8e4
ALU = mybir.AluOpType
ACT = mybir.ActivationFunctionType
DR = mybir.MatmulPerfMode.DoubleRow
E4M3 = ml_dtypes.float8_e4m3

B, T, C, D_ATT, D_FFN = 4, 2048, 2048, 2048, 8192
EPS = 1e-5
N_CORES = 8
DEN_EPS = 1e-30

SA = 16.0          # activation fp8 scale
SW = 64.0          # weight fp8 scale
SKF = 8.0          # kf fp8 scale
PS_INV = 1.0 / (SA * SW)     # psum -> true scale (2^-10)
SQ8 = float(np.sqrt(SKF))


def _splits(total, sz):
    return [(s, min(sz, total - s)) for s in range(0, total, sz)]


def _even_splits(total, mx):
    n = -(-total // mx)
    base, rem = divmod(total, n)
    out, s = [], 0
    for i in range(n):
        sz = base + (1 if i < rem else 0)
        out.append((s, sz))
        s += sz
    return out


def build_program(Cc=C, Dd=D_ATT, Ff=D_FFN, rows_out=T // 2, n_cores=N_CORES,
                  no_collective=False):
    P = 128
    CG, DG, FG = Cc // P, Dd // P, Ff // P
    RO = rows_out
    RS = RO + 1
    R = RS + 1
    RSP = -(-RS // 16) * 16   # fp8 moving tiles padded: pair stride %16 == 0
    NV = 11

    nc = bacc.Bacc("TRN2", target_bir_lowering=False, debug=False,
                   num_devices=n_cores)

    xT = nc.dram_tensor("xT", [Cc, R], F32, kind="ExternalInput").ap()
    xTb = nc.dram_tensor("xTb", [Cc, R], BF16, kind="ExternalInput").ap()
    wkh = nc.dram_tensor("wkh", [P, CG, Dd], F8, kind="ExternalInput").ap()
    wvh = nc.dram_tensor("wvh", [P, CG, Dd], F8, kind="ExternalInput").ap()
    wvl = nc.dram_tensor("wvl", [P, CG, Dd], F8, kind="ExternalInput").ap()
    wrh = nc.dram_tensor("wrh", [P, CG, Dd], F8, kind="ExternalInput").ap()
    woh = nc.dram_tensor("woh", [P, DG, Cc], F8, kind="ExternalInput").ap()
    wol = nc.dram_tensor("wol", [P, DG, Cc], F8, kind="ExternalInput").ap()
    wckh = nc.dram_tensor("wckh", [P, CG, Ff], F8, kind="ExternalInput").ap()
    wckl = nc.dram_tensor("wckl", [P, CG, Ff], F8, kind="ExternalInput").ap()
    wcvh = nc.dram_tensor("wcvh", [P, FG, Cc], F8, kind="ExternalInput").ap()
    wcvl = nc.dram_tensor("wcvl", [P, FG, Cc], F8, kind="ExternalInput").ap()
    wcrh = nc.dram_tensor("wcrh", [P, CG, Cc], F8, kind="ExternalInput").ap()
    cvec = nc.dram_tensor("cvec", [P, CG, NV], F32, kind="ExternalInput").ap()
    m0d = nc.dram_tensor("m0", [P, 1], F32, kind="ExternalInput").ap()
    seld = nc.dram_tensor("sel", [P, n_cores], F32, kind="ExternalInput").ap()
    outT = nc.dram_tensor("outT", [Cc, RO], F32, kind="ExternalOutput").ap()

    xTv = xT.rearrange("(g p) r -> p g r", p=P)
    xTbv = xTb.rearrange("(g p) r -> p g r", p=P)
    outTv = outT.rearrange("(g p) r -> p g r", p=P)

    I_LN1W, I_LN1B, I_TMK, I_TMV, I_TMR, I_EW, I_EU, I_LN2W, I_LN2B, \
        I_CMK, I_CMR = range(NV)

    TS = 512
    LTS = 256

    with tile.TileContext(nc) as tc:
        const = tc.alloc_tile_pool(name="const", bufs=1)
        con = const.tile([P, CG, NV], F32, tag="con")
        nc.sync.dma_start(out=con[:], in_=cvec)
        m0 = const.tile([P, 1], F32, tag="m0")
        nc.sync.dma_start(out=m0[:], in_=m0d)
        selt = const.tile([P, n_cores], F32, tag="sel")
        nc.sync.dma_start(out=selt[:], in_=seld)
        onesc = const.tile([P, 1], F32, tag="ones")
        nc.vector.memset(onesc[:], 1.0)
        onesb = const.tile([P, 1], BF16, tag="onesb")
        nc.vector.memset(onesb[:], 1.0)
        epsc = const.tile([1, 1], F32, tag="epsc")
        nc.vector.memset(epsc[:], EPS)
        onesPb = const.tile([1, P], BF16, tag="onesPb")
        nc.vector.memset(onesPb[:], 1.0)

        def ccol(g, i):
            return con[:, g, i:i + 1]

        dram = tc.alloc_tile_pool(name="dram", bufs=1, space="DRAM")
        x2dram = dram.tile([Cc, RS], F32)
        x2dv = x2dram.rearrange("(g p) r -> p g r", p=P)
        srdram = dram.tile([Dd, RS], BF16)
        srdv = srdram.rearrange("(g p) r -> p g r", p=P)
        sgdram = dram.tile([Cc, RO], BF16)
        sgdv = sgdram.rearrange("(g p) r -> p g r", p=P)
        cc_in = dram.tile([P, 2 * DG], F32)
        cc_out = dram.tile([P * n_cores, 2 * DG], F32)

        # ---- LayerNorm (streaming; PE sums via f32r bitcast) ----
        def ln_stream(src_v, nrows, iw, out_sb, name, sbuf_src=False,
                      src_bf16=False, lts=None):
            LTS = lts or 256
            src_dt = BF16 if src_bf16 else F32
            st = tc.alloc_tile_pool(name=f"{name}_st", bufs=1)
            sp = tc.alloc_tile_pool(name=f"{name}_sp", bufs=2)
            spx = tc.alloc_tile_pool(name=f"{name}_spx", bufs=8)
            psum = tc.alloc_tile_pool(name=f"{name}_ps", bufs=2, space="PSUM")
            ssum = st.tile([1, nrows], F32, tag="sum", name="ssum")
            ssq = st.tile([1, nrows], F32, tag="sq", name="ssq")
            for t0, tsz in _splits(nrows, LTS):
                if sbuf_src:
                    xls = src_v[:, :, t0:t0 + tsz]
                else:
                    xlt = sp.tile([P, CG, LTS], src_dt, tag="xls",
                                  name="xls")
                    nc.sync.dma_start(out=xlt[:, :, :tsz],
                                      in_=src_v[:, :, t0:t0 + tsz])
                    xls = xlt[:, :, :tsz]
                xsq = sp.tile([P, CG, LTS], BF16, tag="lnsq", name="xsq")
                nc.scalar.activation(xsq[:, :, :tsz], xls,
                                     ACT.Square)
                ps = psum.tile([1, LTS], F32, tag="ln_ps", name="ps")
                ps2 = psum.tile([1, LTS], F32, tag="ln_ps2", name="ps2")
                for g in range(CG):
                    nc.tensor.matmul(
                        ps[:, :tsz], onesb[:], xls[:, g, :],
                        start=(g == 0), stop=(g == CG - 1))
                    nc.tensor.matmul(
                        ps2[:, :tsz], onesb[:], xsq[:, g, :tsz],
                        start=(g == 0), stop=(g == CG - 1))
                nc.vector.tensor_copy(out=ssum[:, t0:t0 + tsz],
                                      in_=ps[:, :tsz])
                nc.vector.tensor_copy(out=ssq[:, t0:t0 + tsz],
                                      in_=ps2[:, :tsz])
            mu = st.tile([1, nrows], BF16, tag="mu", name="mu")
            rstd = st.tile([1, nrows], BF16, tag="rstd", name="rstd")
            var = st.tile([1, nrows], F32, tag="var", name="var")
            musq = st.tile([1, nrows], F32, tag="musq", name="musq")
            nc.vector.tensor_scalar_mul(mu[:], ssum[:], 1.0 / Cc)
            nc.vector.tensor_scalar_mul(var[:], ssq[:], 1.0 / Cc)
            nc.vector.tensor_tensor(musq[:], mu[:], mu[:], ALU.mult)
            nc.vector.tensor_tensor(var[:], var[:], musq[:], ALU.subtract)
            nc.scalar.activation(var[:], var[:], ACT.Ln, bias=epsc[:])
            nc.scalar.activation(rstd[:], var[:], ACT.Exp, scale=-0.5)
            for t0, tsz in _splits(nrows, LTS):
                if sbuf_src:
                    xls = src_v[:, :, t0:t0 + tsz]
                else:
                    xlt = sp.tile([P, CG, LTS], src_dt, tag="xls",
                                  name="xls")
                    nc.sync.dma_start(out=xlt[:, :, :tsz],
                                      in_=src_v[:, :, t0:t0 + tsz])
                    xls = xlt[:, :, :tsz]
                mups = psum.tile([P, LTS], F32, tag="mups", name="mups")
                nc.tensor.matmul(mups[:, :tsz], onesPb[:],
                                 mu[:, t0:t0 + tsz],
                                 start=True, stop=True)
                rsps = psum.tile([P, LTS], F32, tag="rsps", name="rsps")
                nc.tensor.matmul(rsps[:, :tsz], onesPb[:],
                                 rstd[:, t0:t0 + tsz],
                                 start=True, stop=True)
                for g in range(CG):
                    xm = spx.tile([P, LTS], BF16, tag="ln_xm", name="xm")
                    nc.vector.tensor_tensor(xm[:, :tsz], xls[:, g, :],
                                            mups[:, :tsz], ALU.subtract)
                    nc.vector.scalar_tensor_tensor(
                        out_sb[:, g, t0:t0 + tsz], xm[:, :tsz], ccol(g, iw),
                        rsps[:, :tsz], ALU.mult, ALU.mult)
            for p in (psum, spx, sp, st):
                p.release()

        # ================= Phase A: LN1 (h = 16*ln(x), bf16) ============
        pEk = tc.alloc_tile_pool(name="pEk", bufs=1)
        eksb = [pEk.tile([P, RS], BF16, tag=f"eksb{g}", name=f"eksb{g}")
                for g in range(DG)]
        ekvsb = [pEk.tile([P, RS], BF16, tag=f"ekvsb{g}", name=f"ekvsb{g}")
                 for g in range(DG)]
        pMix = tc.alloc_tile_pool(name="pMix", bufs=1)
        mixk8 = [pMix.tile([P, 2, RSP], F8, tag=f"mixk8_{p}",
                           name=f"mixk8_{p}") for p in range(CG // 2)]
        mixv8 = [pMix.tile([P, 2, RSP], F8, tag=f"mixv8_{p}",
                           name=f"mixv8_{p}") for p in range(CG // 2)]
        mixr8 = [pMix.tile([P, 2, RSP], F8, tag=f"mixr8_{p}",
                           name=f"mixr8_{p}") for p in range(CG // 2)]
        pHs = tc.alloc_tile_pool(name="pHs", bufs=1)
        hs = pHs.tile([P, CG, R], BF16, tag="hs")
        ln_stream(xTbv, R, I_LN1W, hs, "ln1", src_bf16=True)
        nc.vector.tensor_scalar_mul(hs[:, :, 0:2], hs[:, :, 0:2], m0[:])

        # ========== Phase B: mixes (fp8 x16) + k/v/r DR matmuls ========
        stg = tc.alloc_tile_pool(name="stg", bufs=4)
        if RSP > RS:
            for mixl in (mixk8, mixv8, mixr8):
                for mt in mixl:
                    nc.vector.memset(mt[:, :, RS:RSP], 0.0)
        MSTRIPS = [(0, 512), (512, RS - 512)]
        for t0, tsz in MSTRIPS:
            for g in range(CG):
                dmix = stg.tile([P, 512 + 1], BF16, tag="dmix", name="dmix")
                nc.vector.tensor_tensor(
                    dmix[:, :tsz], hs[:, g, 1 + t0:1 + t0 + tsz],
                    hs[:, g, t0:t0 + tsz], ALU.subtract)
                for mixl, icoef, on_act in ((mixk8, I_TMK, True),
                                            (mixv8, I_TMV, False),
                                            (mixr8, I_TMR, True)):
                    mb16 = stg.tile([P, 512 + 1], BF16, tag="mb16",
                                    name="mb16")
                    nc.vector.scalar_tensor_tensor(
                        mb16[:, :tsz], dmix[:, :tsz], ccol(g, icoef),
                        hs[:, g, t0:t0 + tsz], ALU.mult, ALU.add)
                    dst = mixl[g // 2][:, g % 2, t0:t0 + tsz]
                    if on_act:
                        nc.scalar.activation(dst, mb16[:, :tsz], ACT.Copy)
                    else:
                        nc.gpsimd.tensor_copy(out=dst, in_=mb16[:, :tsz])
        wpB = tc.alloc_tile_pool(name="wpB", bufs=2)
        stgE = tc.alloc_tile_pool(name="stgE", bufs=4)
        psB = tc.alloc_tile_pool(name="psB", bufs=6, space="PSUM")
        DBLK = 512
        tstripsB = [(0, 512), (512, 512), (1024, RSP - 1024)]

        def mm_dr(whd, wld, rhs8, n_out, evict, wtag, strips=None):
            for d0, dsz in _splits(n_out, DBLK):
                wbh = wpB.tile([P, CG, DBLK], F8, tag="wh", name="wbh")
                nc.sync.dma_start(out=wbh[:, :, :dsz],
                                  in_=whd[:, :, d0:d0 + dsz])
                if wld is not None:
                    wbl = wpB.tile([P, CG, DBLK], F8, tag="wl",
                                   name="wbl")
                    nc.sync.dma_start(out=wbl[:, :, :dsz],
                                      in_=wld[:, :, d0:d0 + dsz])
                wbufs = [wbh] if wld is None else [wbh, wbl]
                for gl in range(dsz // P):
                    g_out = (d0 + gl * P) // P
                    for t0, tsz in (strips or tstripsB):
                        wsz = min(tsz, RS - t0)
                        if wsz <= 0:
                            continue
                        ps = psB.tile([P, TS], F32, tag="mm_ps", name="mm_ps")
                        nmm = len(wbufs) * (CG // 2)
                        i = 0
                        for wb in wbufs:
                            for gp in range(CG // 2):
                                nc.tensor.matmul(
                                    ps[:, :tsz],
                                    wb[:, 2 * gp:2 * gp + 2,
                                       gl * P:(gl + 1) * P],
                                    rhs8[gp][:, :, t0:t0 + tsz],
                                    start=(i == 0), stop=(i == nmm - 1),
                                    perf_mode=DR)
                                i += 1
                        evict(g_out, t0, wsz, ps)

        def evict_k(g, t0, wsz, ps):
            nc.scalar.activation(eksb[g][:, t0:t0 + wsz], ps[:, :wsz],
                                 ACT.Exp, scale=PS_INV)
            if t0 == 0:
                nc.vector.tensor_scalar_mul(eksb[g][:, 0:1], eksb[g][:, 0:1],
                                            m0[:])

        def evict_v(g, t0, wsz, ps):
            nc.vector.scalar_tensor_tensor(
                ekvsb[g][:, t0:t0 + wsz], ps[:, :wsz], PS_INV,
                eksb[g][:, t0:t0 + wsz], ALU.mult, ALU.mult)

        def evict_r(g, t0, wsz, ps):
            srt = stgE.tile([P, TS], BF16, tag="srt", name="srt")
            nc.scalar.activation(srt[:, :wsz], ps[:, :wsz], ACT.Sigmoid,
                                 scale=PS_INV)
            nc.sync.dma_start(out=srdv[:, g, t0:t0 + wsz], in_=srt[:, :wsz])

        mm_dr(wkh, None, mixk8, Dd, evict_k, "wk", strips=tstripsB[:1])
        mm_dr(wkh, None, mixk8, Dd, evict_k, "wk", strips=tstripsB[1:])
        mm_dr(wvh, wvl, mixv8, Dd, evict_v, "wv")
        mm_dr(wrh, None, mixr8, Dd, evict_r, "wr")

        psB.release()
        stgE.release()
        wpB.release()
        stg.release()
        pHs.release()
        pMix.release()

        # ====== Phase C: boundary states (bf16 scans) + AllGather =======
        pRw = tc.alloc_tile_pool(name="pRw", bufs=1, side="right")
        rwkv8 = [pRw.tile([P, 2, RSP], F8, tag=f"rw{p}", name=f"rw{p}")
                 for p in range(DG // 2)]
        if RSP > RS:
            for rwt in rwkv8:
                nc.vector.memset(rwt[:, :, RS:RSP], 0.0)
        wpE = tc.alloc_tile_pool(name="wpE", bufs=2, side="right")
        spE = tc.alloc_tile_pool(name="spE", bufs=2, side="right")
        pC = tc.alloc_tile_pool(name="pC", bufs=2, side="right")
        state = pC.tile([P, 2 * DG], F32, tag="state", name="state")
        for g in range(DG):
            ewbc = ccol(g, I_EW).to_broadcast([P, RS - 1])
            apre = pC.tile([P, RS - 1], BF16, tag="apre", name="apre")
            nc.vector.tensor_tensor_scan(
                apre[:], ewbc, ekvsb[g][:, :RS - 1], 0.0, ALU.mult, ALU.add)
            nc.gpsimd.tensor_copy(out=state[:, g:g + 1],
                                  in_=apre[:, RS - 2:RS - 1])
            bpre = pC.tile([P, RS - 1], BF16, tag="bpre", name="bpre")
            nc.vector.tensor_tensor_scan(
                bpre[:], ewbc, eksb[g][:, :RS - 1], 0.0, ALU.mult, ALU.add)
            nc.gpsimd.tensor_copy(out=state[:, DG + g:DG + g + 1],
                                  in_=bpre[:, RS - 2:RS - 1])
        nc.sync.dma_start(out=cc_in[:], in_=state[:])
        if not no_collective:
            nc.gpsimd.collective_compute(
                "AllGather", ALU.bypass,
                replica_groups=[list(range(n_cores))],
                ins=[cc_in[:].opt()], outs=[cc_out[:].opt()])
        else:
            for jj in range(n_cores):
                nc.sync.dma_start(out=cc_out[jj * P:(jj + 1) * P, :],
                                  in_=cc_in[:])
        gsb = pC.tile([P, n_cores, 2 * DG], F32, tag="gsb", name="gsb")
        nc.sync.dma_start(
            out=gsb[:], in_=cc_out[:].rearrange("(j p) s -> p j s", p=P))
        a0b0 = pC.tile([P, 2 * DG], F32, tag="a0b0", name="a0b0")
        nc.vector.memset(a0b0[:, 0:DG], 0.0)
        nc.vector.memset(a0b0[:, DG:2 * DG], DEN_EPS)
        for j in range(n_cores):
            nc.vector.scalar_tensor_tensor(
                a0b0[:], gsb[:, j, :], selt[:, j:j + 1], a0b0[:],
                ALU.mult, ALU.add)

        # ============ Phase D: WKV scans + rwkv (fp8 x16) ============
        pD = tc.alloc_tile_pool(name="pD", bufs=3)

        def d_front(g):
            ekg = eksb[g][:]
            xkg = ekvsb[g][:]
            srg = pD.tile([P, RS], BF16, tag="srg", name="srg")
            nc.sync.dma_start(out=srg[:], in_=srdv[:, g, :])
            ewb = pD.tile([P, RS], BF16, tag="ewb", name="ewb")
            nc.scalar.activation(ewb[:], ccol(g, I_EW).to_broadcast([P, RS]),
                                 ACT.Copy)
            eub = pD.tile([P, RS], BF16, tag="eub", name="eub")
            nc.scalar.activation(eub[:], ccol(g, I_EU).to_broadcast([P, RS]),
                                 ACT.Copy)
            abuf = pD.tile([P, RS + 1], BF16, tag="abuf", name="abuf")
            nc.gpsimd.tensor_copy(out=abuf[:, 0:1], in_=a0b0[:, g:g + 1])
            nc.vector.tensor_tensor_scan(
                abuf[:, 1:RS + 1], ewb[:], xkg, a0b0[:, g:g + 1],
                ALU.mult, ALU.add)
            bbuf = pD.tile([P, RS + 1], BF16, tag="bbuf", name="bbuf")
            nc.gpsimd.tensor_copy(out=bbuf[:, 0:1],
                                  in_=a0b0[:, DG + g:DG + g + 1])
            nc.vector.tensor_tensor_scan(
                bbuf[:, 1:RS + 1], ewb[:], ekg,
                a0b0[:, DG + g:DG + g + 1], ALU.mult, ALU.add)
            num = pD.tile([P, RS], BF16, tag="num", name="num")
            nc.vector.scalar_tensor_tensor(
                num[:], xkg, ccol(g, I_EU), abuf[:, 0:RS],
                ALU.mult, ALU.add)
            snum = pD.tile([P, RS], BF16, tag="snum", name="snum")
            nc.vector.tensor_tensor(snum[:], num[:], srg[:], ALU.mult)
            t1 = pD.tile([P, RS], BF16, tag="t1", name="t1")
            nc.gpsimd.tensor_tensor(t1[:], ekg, eub[:], ALU.mult)
            den = pD.tile([P, RS], F32, tag="den", name="den")
            nc.gpsimd.tensor_tensor(den[:], t1[:], bbuf[:, 0:RS], ALU.add)
            return snum, den

        def d_back(g, snum, den):
            rden = pD.tile([P, RS], F32, tag="rden", name="rden")
            nc.vector.reciprocal_approx_fast(out=rden[:], in_=den[:])
            nc.vector.scalar_tensor_tensor(
                rwkv8[g // 2][:, g % 2, :RS], snum[:], SA, rden[:],
                ALU.mult, ALU.mult)

        pend = []
        for g in range(DG):
            pend.append((g, d_front(g)))
            if len(pend) > 3:
                gq, fq = pend.pop(0)
                d_back(gq, *fq)
        for gq, fq in pend:
            d_back(gq, *fq)
        pD.release()
        pEk.release()
        pC.release()
        pMx2 = tc.alloc_tile_pool(name="pMx2", bufs=1)
        xk2h = pMx2.tile([P, CG, RO], F8, tag="xk2h")
        xk2l = pMx2.tile([P, CG, RO], F8, tag="xk2l")
        pXr = tc.alloc_tile_pool(name="pXr", bufs=1)
        xr28 = pXr.tile([P, CG, RO], F8, tag="xr28")
        wpG = tc.alloc_tile_pool(name="wpG", bufs=2)
        spG = tc.alloc_tile_pool(name="spG", bufs=2)
        pX2 = tc.alloc_tile_pool(name="pX2", bufs=1)
        x2bf = pX2.tile([P, CG, RS], BF16, tag="x2bf")

        # ========= Phase E: Wo (2t DR) -> x2 = x + attn (DRAM) =========
        psE = tc.alloc_tile_pool(name="psE", bufs=3, space="PSUM")
        CBLK = 512
        for c0, csz in _splits(Cc, CBLK):
            wbh = wpE.tile([P, DG, CBLK], F8, tag="woh", name="woh")
            nc.sync.dma_start(out=wbh[:, :, :csz], in_=woh[:, :, c0:c0 + csz])
            wbl = wpE.tile([P, DG, CBLK], F8, tag="wol", name="wol")
            nc.sync.dma_start(out=wbl[:, :, :csz], in_=wol[:, :, c0:c0 + csz])
            for gl in range(csz // P):
                g_c = (c0 + gl * P) // P
                for t0, tsz in tstripsB:
                    wsz = min(tsz, RS - t0)
                    if wsz <= 0:
                        continue
                    ps = psE.tile([P, TS], F32, tag="wo_ps", name="wo_ps")
                    i = 0
                    for wb in (wbh, wbl):
                        for gp in range(DG // 2):
                            nc.tensor.matmul(
                                ps[:, :tsz],
                                wb[:, 2 * gp:2 * gp + 2, gl * P:(gl + 1) * P],
                                rwkv8[gp][:, :, t0:t0 + tsz],
                                start=(i == 0), stop=(i == DG - 1),
                                perf_mode=DR)
                            i += 1
                    xst = spE.tile([P, TS], F32, tag="xst", name="xst")
                    nc.sync.dma_start(
                        out=xst[:, :wsz],
                        in_=xTv[:, g_c, 1 + t0:1 + t0 + wsz])
                    x2st = spE.tile([P, TS], F32, tag="x2st", name="x2st")
                    nc.vector.scalar_tensor_tensor(
                        x2st[:, :wsz], ps[:, :wsz], PS_INV,
                        xst[:, :wsz], ALU.mult, ALU.add)
                    nc.sync.dma_start(out=x2dv[:, g_c, t0:t0 + wsz],
                                      in_=x2st[:, :wsz])
                    nc.gpsimd.tensor_copy(out=x2bf[:, g_c, t0:t0 + wsz],
                                          in_=x2st[:, :wsz])
        psE.release()
        spE.release()
        wpE.release()
        pRw.release()

        # ====== Phase F: LN2 + mixes2 (xk2 hi/lo fp8, xr2 fp8) ======
        pG2 = tc.alloc_tile_pool(name="pG2", bufs=1)
        g2 = pG2.tile([P, CG, RS], BF16, tag="g2")
        ln_stream(x2bf, RS, I_LN2W, g2, "ln2", sbuf_src=True, lts=512)
        nc.vector.tensor_scalar_mul(g2[:, :, 0:1], g2[:, :, 0:1], m0[:])

        spF = tc.alloc_tile_pool(name="spF", bufs=3)
        for g in range(CG):
            dmix = spF.tile([P, RO], BF16, tag="dmix2", name="dmix2")
            nc.vector.tensor_tensor(dmix[:], g2[:, g, 1:RS],
                                    g2[:, g, 0:RO], ALU.subtract)
            xr2b = spF.tile([P, RO], BF16, tag="xr2b", name="xr2b")
            nc.vector.scalar_tensor_tensor(
                xr2b[:], dmix[:], ccol(g, I_CMR), g2[:, g, 0:RO],
                ALU.mult, ALU.add)
            nc.gpsimd.tensor_copy(out=xr28[:, g, :], in_=xr2b[:])
        for g in range(CG):
            dmix = spF.tile([P, RO], BF16, tag="dmix2", name="dmix2")
            nc.vector.tensor_tensor(dmix[:], g2[:, g, 1:RS],
                                    g2[:, g, 0:RO], ALU.subtract)
            xk2b = spF.tile([P, RO], BF16, tag="xk2b", name="xk2b")
            nc.vector.scalar_tensor_tensor(
                xk2b[:], dmix[:], ccol(g, I_CMK), g2[:, g, 0:RO],
                ALU.mult, ALU.add)
            nc.scalar.activation(xk2h[:, g, :], xk2b[:], ACT.Copy)
            dif = spF.tile([P, RO], BF16, tag="dif", name="dif")
            nc.vector.tensor_tensor(dif[:], xk2b[:], xk2h[:, g, :],
                                    ALU.subtract)
            nc.scalar.activation(xk2l[:, g, :], dif[:], ACT.Copy)
        spF.release()
        pG2.release()
        pX2.release()

        # ====== Phase G: r2 = sigmoid(xr2 @ WcrT) (pure DR) -> DRAM =====
        psG = tc.alloc_tile_pool(name="psG", bufs=4, space="PSUM")
        for c0, csz in _splits(Cc, CBLK):
            wbh = wpG.tile([P, CG, CBLK], F8, tag="wcr", name="wcr")
            nc.sync.dma_start(out=wbh[:, :, :csz], in_=wcrh[:, :, c0:c0 + csz])
            for gl in range(csz // P):
                g_c = (c0 + gl * P) // P
                for t0, tsz in _splits(RO, TS):
                    ps = psG.tile([P, TS], F32, tag="wcr_ps", name="wcr_ps")
                    for gp in range(CG // 2):
                        nc.tensor.matmul(
                            ps[:, :tsz],
                            wbh[:, 2 * gp:2 * gp + 2, gl * P:(gl + 1) * P],
                            xr28[:, 2 * gp:2 * gp + 2, t0:t0 + tsz],
                            start=(gp == 0), stop=(gp == CG // 2 - 1),
                            perf_mode=DR)
                    sgt = spG.tile([P, TS], BF16, tag="sgt", name="sgt")
                    nc.scalar.activation(sgt[:, :tsz], ps[:, :tsz],
                                         ACT.Sigmoid, scale=PS_INV)
                    nc.sync.dma_start(out=sgdv[:, g_c, t0:t0 + tsz],
                                      in_=sgt[:, :tsz])
        psG.release()
        spG.release()
        wpG.release()
        pXr.release()

        # ============ Phase H: FFN (3t DR both matmuls) ============
        FBLK = 512
        FQ = 16
        for t0, tsz in _splits(RO, TS):
            pH = tc.alloc_tile_pool(name=f"pH{t0}", bufs=1)
            sH = tc.alloc_tile_pool(name=f"sH{t0}", bufs=2)
            wpH = tc.alloc_tile_pool(name=f"wpH{t0}", bufs=2)
            psH = tc.alloc_tile_pool(name=f"psH{t0}", bufs=4, space="PSUM")
            psKV = tc.alloc_tile_pool(name=f"psKV{t0}", bufs=1, space="PSUM")
            kf8 = pH.tile([P, FG, TS], F8, tag="kf8", name="kf8")
            # FFN1 3t: z = Wckh@(xh+xl) + Wckl@xh; trl = sqrt(8)*relu(z)
            for f0, fsz in _splits(Ff, FBLK):
                wbh = wpH.tile([P, CG, FBLK], F8, tag="wfh", name="wfh")
                nc.sync.dma_start(out=wbh[:, :, :fsz],
                                  in_=wckh[:, :, f0:f0 + fsz])
                wbl = wpH.tile([P, CG, FBLK], F8, tag="wfl", name="wfl")
                nc.sync.dma_start(out=wbl[:, :, :fsz],
                                  in_=wckl[:, :, f0:f0 + fsz])
                ngl = fsz // P
                trl = sH.tile([P, ngl, TS], BF16, tag="trl", name="trl")
                for fl in range(ngl):
                    ps = psH.tile([P, TS], F32, tag="ffn1_ps", name="ffn1_ps")
                    i = 0
                    nmm = 3 * (CG // 2)
                    for wb, act in ((wbh, xk2h), (wbh, xk2l), (wbl, xk2h)):
                        for gp in range(CG // 2):
                            nc.tensor.matmul(
                                ps[:, :tsz],
                                wb[:, 2 * gp:2 * gp + 2, fl * P:(fl + 1) * P],
                                act[:, 2 * gp:2 * gp + 2, t0:t0 + tsz],
                                start=(i == 0), stop=(i == nmm - 1),
                                perf_mode=DR)
                            i += 1
                    nc.scalar.activation(trl[:, fl, :tsz], ps[:, :tsz],
                                         ACT.Relu, scale=PS_INV * SQ8)
                # kf8 = e4m3(trl^2) = e4m3(8*kf) in one ACT Square
                g_f0 = f0 // P
                nc.scalar.activation(kf8[:, g_f0:g_f0 + ngl, :tsz],
                                     trl[:, :, :tsz], ACT.Square)
            # FFN2 3t + final: out = x2 + sg*((Wcvh@(kf8+kflo)+Wcvl@kf8)/512)
            for c0, csz in _splits(Cc, CBLK):
                kvps = [psKV.tile([P, TS], F32, tag=f"kv_ps{i}",
                                  name=f"kv_ps{i}")
                        for i in range(csz // P)]
                nq = FG // FQ
                nmm_tot = nq * 2 * (FQ // 2)
                mm_idx = [0] * (csz // P)
                for q in range(nq):
                    f_lo = q * FQ
                    wbh = wpH.tile([P, FQ, CBLK], F8, tag="wf2h", name="wf2h")
                    nc.sync.dma_start(
                        out=wbh[:, :, :csz],
                        in_=wcvh[:, f_lo:f_lo + FQ, c0:c0 + csz])
                    wbl = wpH.tile([P, FQ, CBLK], F8, tag="wf2l", name="wf2l")
                    nc.sync.dma_start(
                        out=wbl[:, :, :csz],
                        in_=wcvl[:, f_lo:f_lo + FQ, c0:c0 + csz])
                    for gl in range(csz // P):
                        for wb, act in ((wbh, kf8), (wbl, kf8)):
                            for fp in range(FQ // 2):
                                fg = f_lo + 2 * fp
                                nc.tensor.matmul(
                                    kvps[gl][:, :tsz],
                                    wb[:, 2 * fp:2 * fp + 2,
                                       gl * P:(gl + 1) * P],
                                    act[:, fg:fg + 2, :tsz],
                                    start=(mm_idx[gl] == 0),
                                    stop=(mm_idx[gl] == nmm_tot - 1),
                                    perf_mode=DR)
                                mm_idx[gl] += 1
                for gl in range(csz // P):
                    g_c = (c0 + gl * P) // P
                    sgs = wpH.tile([P, TS], BF16, tag="sgs", name="sgs")
                    nc.sync.dma_start(out=sgs[:, :tsz],
                                      in_=sgdv[:, g_c, t0:t0 + tsz])
                    ot = wpH.tile([P, TS], BF16, tag="ot", name="ot")
                    nc.vector.scalar_tensor_tensor(
                        ot[:, :tsz], kvps[gl][:, :tsz], 1.0 / (SKF * SW),
                        sgs[:, :tsz], ALU.mult, ALU.mult)
                    x2s = wpH.tile([P, TS], F32, tag="x2s", name="x2s")
                    nc.sync.dma_start(
                        out=x2s[:, :tsz],
                        in_=x2dv[:, g_c, 1 + t0:1 + t0 + tsz])
                    o2 = wpH.tile([P, TS], F32, tag="o2", name="o2")
                    nc.vector.tensor_tensor(o2[:, :tsz], ot[:, :tsz],
                                            x2s[:, :tsz], ALU.add)
                    nc.sync.dma_start(out=outTv[:, g_c, t0:t0 + tsz],
                                      in_=o2[:, :tsz])
            for p in (psKV, psH, wpH, sH, pH):
                p.release()
        pMx2.release()
        dram.release()
        const.release()

    nc.compile()
    return nc


_PROGRAM_CACHE = {}


def _get_program(key, **kw):
    if key not in _PROGRAM_CACHE:
        _PROGRAM_CACHE[key] = build_program(**kw)
    return _PROGRAM_CACHE[key]


def _q8pair(wT_scaled):
    """fp32 [128, KG, N] (already x SW) -> (hi, lo) e4m3 at the same scale."""
    hi = wT_scaled.astype(E4M3)
    lo = (wT_scaled - hi.astype(np.float32)).astype(E4M3)
    return hi, lo


def _host_prep(inputs, Cc=C, Dd=D_ATT, Ff=D_FFN, Bb=B, Tt=T, n_cores=N_CORES):
    P = 128
    CG, DG, FG = Cc // P, Dd // P, Ff // P
    half = Tt // 2
    RO, RS, R = half, half + 1, half + 2

    f = {k: np.asarray(v, np.float32) for k, v in inputs.items()}
    x = f["x"]

    def swz(wT, kg):  # [K, N] fp32 -> [128, kg, N] * SW
        Kdim, Ndim = wT.shape
        return np.ascontiguousarray(
            wT.reshape(kg, P, Ndim).transpose(1, 0, 2)) * SW

    wkh_, _ = _q8pair(swz(f["Wk"].T, CG))
    wvh_, wvl_ = _q8pair(swz(f["Wv"].T, CG))
    wrh_, _ = _q8pair(swz(f["Wr"].T, CG))
    woh_, wol_ = _q8pair(swz(f["Wo"].T, DG))
    wckh_, wckl_ = _q8pair(swz(f["Wck"].T, CG))
    wcvh_, wcvl_ = _q8pair(swz(f["Wcv"].T, FG))
    wcrh_, _ = _q8pair(swz(f["Wcr"].T, CG))

    def col(v):
        return np.ascontiguousarray(
            np.asarray(v, np.float32).reshape(-1).reshape(CG, P).T)

    ew = np.exp(-np.exp(f["time_decay"].astype(np.float64)))
    cvec_h = np.stack([
        col(f["ln1_w"] * SA), col(f["ln1_b"]),
        col(f["tm_k"]), col(f["tm_v"]), col(f["tm_r"]),
        col(ew.astype(np.float32)), col(np.exp(f["time_first"])),
        col(f["ln2_w"] * SA), col(f["ln2_b"]),
        col(f["cm_k"]), col(f["cm_r"]),
    ], axis=-1).astype(np.float32)

    in_maps = []
    for core in range(n_cores):
        b, hh = core // 2, core % 2
        t0 = hh * half
        xr = np.zeros((R, Cc), np.float32)
        lo = t0 - 2
        src_lo = max(lo, 0)
        xr[src_lo - lo:, :] = x[b, src_lo:t0 + RO, :]
        m0 = np.full((P, 1), float(hh), np.float32)
        sel = np.zeros((P, n_cores), np.float32)
        if hh == 1:
            sel[:, core - 1] = 1.0
        xrt = np.ascontiguousarray(xr.T)
        in_maps.append({
            "xT": xrt, "xTb": xrt.astype(ml_dtypes.bfloat16),
            "wkh": wkh_, "wvh": wvh_, "wvl": wvl_, "wrh": wrh_,
            "woh": woh_, "wol": wol_, "wckh": wckh_, "wckl": wckl_,
            "wcvh": wcvh_, "wcvl": wcvl_, "wcrh": wcrh_,
            "cvec": cvec_h, "m0": m0, "sel": sel,
        })
    return in_maps


def kernel(**inputs):
    in_maps = _host_prep(inputs)
    nc = _get_program("full")
    res = run_bass_kernel_spmd(nc, in_maps, core_ids=list(range(N_CORES)))
    half = T // 2
    out = np.empty((B, T, C), np.float32)
    for core in range(N_CORES):
        b, hh = core // 2, core % 2
        out[b, hh * half:(hh + 1) * half, :] = res.results[core]["outT"].T
    return out


# revision 10
# speedup vs baseline: 1.4998x; 1.0048x over previous
"""RWKV-4 block on 8 trn2 cores — fp8e4 DoubleRow version.

Sharding: 8 cores = 4 batch x 2 T-halves (as baseline). All big matmuls run
as fp8e4 DoubleRow (K=256/instr, 0.5 cyc/row). Precision scheme (measured
offline: final rel err ~1.9e-2 vs the 2e-2 gate):
  Wk, Wr, Wcr: pure fp8 (weights e4m3 x64, acts e4m3 x16)
  Wv, Wo:      2-term (weight hi+lo at the same scale; lo rides subnormals)
  Wck: 3-term (weight+act hi/lo); Wcv: 2-term (weight hi+lo, kf pure)
Same-scale lo parts make every term share one PSUM scale, so all terms
accumulate natively in PSUM with no combine ops.
"""

import os
import sys

import numpy as np

for _p in ("/opt/trn_rl_repo", "/root/.axon_site/_ro/trn_rl_repo"):
    if os.path.isdir(_p) and _p not in sys.path:
        sys.path.insert(0, _p)

import ml_dtypes  # noqa: E402

import concourse.bass as bass  # noqa: E402,F401
import concourse.mybir as mybir  # noqa: E402
import concourse.tile as tile  # noqa: E402
from concourse import bacc  # noqa: E402
from concourse.bass_utils import run_bass_kernel_spmd  # noqa: E402

F32 = mybir.dt.float32
F32R = mybir.dt.float32r
BF16 = mybir.dt.bfloat16
F8 = mybir.dt.float8e4
ALU = mybir.AluOpType
ACT = mybir.ActivationFunctionType
DR = mybir.MatmulPerfMode.DoubleRow
E4M3 = ml_dtypes.float8_e4m3

B, T, C, D_ATT, D_FFN = 4, 2048, 2048, 2048, 8192
EPS = 1e-5
N_CORES = 8
DEN_EPS = 1e-30

SA = 16.0          # activation fp8 scale
SW = 64.0          # weight fp8 scale
SKF = 8.0          # kf fp8 scale
PS_INV = 1.0 / (SA * SW)     # psum -> true scale (2^-10)
SQ8 = float(np.sqrt(SKF))


def _splits(total, sz):
    return [(s, min(sz, total - s)) for s in range(0, total, sz)]


def _even_splits(total, mx):
    n = -(-total // mx)
    base, rem = divmod(total, n)
    out, s = [], 0
    for i in range(n):
        sz = base + (1 if i < rem else 0)
        out.append((s, sz))
        s += sz
    return out


def build_program(Cc=C, Dd=D_ATT, Ff=D_FFN, rows_out=T // 2, n_cores=N_CORES,
                  no_collective=False):
    P = 128
    CG, DG, FG = Cc // P, Dd // P, Ff // P
    RO = rows_out
    RS = RO + 1
    R = RS + 1
    RSP = -(-RS // 16) * 16   # fp8 moving tiles padded: pair stride %16 == 0
    NV = 11

    nc = bacc.Bacc("TRN2", target_bir_lowering=False, debug=False,
                   num_devices=n_cores)

    xT = nc.dram_tensor("xT", [Cc, R], F32, kind="ExternalInput").ap()
    xTb = nc.dram_tensor("xTb", [Cc, R], BF16, kind="ExternalInput").ap()
    wkh = nc.dram_tensor("wkh", [P, CG, Dd], F8, kind="ExternalInput").ap()
    wvh = nc.dram_tensor("wvh", [P, CG, Dd], F8, kind="ExternalInput").ap()
    wvl = nc.dram_tensor("wvl", [P, CG, Dd], F8, kind="ExternalInput").ap()
    wrh = nc.dram_tensor("wrh", [P, CG, Dd], F8, kind="ExternalInput").ap()
    woh = nc.dram_tensor("woh", [P, DG, Cc], F8, kind="ExternalInput").ap()
    wol = nc.dram_tensor("wol", [P, DG, Cc], F8, kind="ExternalInput").ap()
    wckh = nc.dram_tensor("wckh", [P, CG, Ff], F8, kind="ExternalInput").ap()
    wckl = nc.dram_tensor("wckl", [P, CG, Ff], F8, kind="ExternalInput").ap()
    wcvh = nc.dram_tensor("wcvh", [P, FG, Cc], F8, kind="ExternalInput").ap()
    wcvl = nc.dram_tensor("wcvl", [P, FG, Cc], F8, kind="ExternalInput").ap()
    wcrh = nc.dram_tensor("wcrh", [P, CG, Cc], F8, kind="ExternalInput").ap()
    cvec = nc.dram_tensor("cvec", [P, CG, NV], F32, kind="ExternalInput").ap()
    m0d = nc.dram_tensor("m0", [P, 1], F32, kind="ExternalInput").ap()
    seld = nc.dram_tensor("sel", [P, n_cores], F32, kind="ExternalInput").ap()
    outT = nc.dram_tensor("outT", [Cc, RO], F32, kind="ExternalOutput").ap()

    xTv = xT.rearrange("(g p) r -> p g r", p=P)
    xTbv = xTb.rearrange("(g p) r -> p g r", p=P)
    outTv = outT.rearrange("(g p) r -> p g r", p=P)

    I_LN1W, I_LN1B, I_TMK, I_TMV, I_TMR, I_EW, I_EU, I_LN2W, I_LN2B, \
        I_CMK, I_CMR = range(NV)

    TS = 512
    LTS = 256

    with tile.TileContext(nc) as tc:
        const = tc.alloc_tile_pool(name="const", bufs=1)
        con = const.tile([P, CG, NV], F32, tag="con")
        nc.sync.dma_start(out=con[:], in_=cvec)
        m0 = const.tile([P, 1], F32, tag="m0")
        nc.sync.dma_start(out=m0[:], in_=m0d)
        selt = const.tile([P, n_cores], F32, tag="sel")
        nc.sync.dma_start(out=selt[:], in_=seld)
        onesc = const.tile([P, 1], F32, tag="ones")
        nc.vector.memset(onesc[:], 1.0)
        onesb = const.tile([P, 1], BF16, tag="onesb")
        nc.vector.memset(onesb[:], 1.0)
        epsc = const.tile([1, 1], F32, tag="epsc")
        nc.vector.memset(epsc[:], EPS)
        onesPb = const.tile([1, P], BF16, tag="onesPb")
        nc.vector.memset(onesPb[:], 1.0)

        def ccol(g, i):
            return con[:, g, i:i + 1]

        dram = tc.alloc_tile_pool(name="dram", bufs=1, space="DRAM")
        x2dram = dram.tile([Cc, RS], F32)
        x2dv = x2dram.rearrange("(g p) r -> p g r", p=P)
        srdram = dram.tile([Dd, RS], BF16)
        srdv = srdram.rearrange("(g p) r -> p g r", p=P)
        sgdram = dram.tile([Cc, RO], BF16)
        sgdv = sgdram.rearrange("(g p) r -> p g r", p=P)
        cc_in = dram.tile([P, 2 * DG], F32)
        cc_out = dram.tile([P * n_cores, 2 * DG], F32)

        # ---- LayerNorm (streaming; PE sums via f32r bitcast) ----
        def ln_stream(src_v, nrows, iw, out_sb, name, sbuf_src=False,
                      src_bf16=False, lts=None):
            LTS = lts or 256
            src_dt = BF16 if src_bf16 else F32
            st = tc.alloc_tile_pool(name=f"{name}_st", bufs=1)
            sp = tc.alloc_tile_pool(name=f"{name}_sp", bufs=2)
            spx = tc.alloc_tile_pool(name=f"{name}_spx", bufs=8)
            psum = tc.alloc_tile_pool(name=f"{name}_ps", bufs=2, space="PSUM")
            ssum = st.tile([1, nrows], F32, tag="sum", name="ssum")
            ssq = st.tile([1, nrows], F32, tag="sq", name="ssq")
            for t0, tsz in _splits(nrows, LTS):
                if sbuf_src:
                    xls = src_v[:, :, t0:t0 + tsz]
                else:
                    xlt = sp.tile([P, CG, LTS], src_dt, tag="xls",
                                  name="xls")
                    nc.sync.dma_start(out=xlt[:, :, :tsz],
                                      in_=src_v[:, :, t0:t0 + tsz])
                    xls = xlt[:, :, :tsz]
                xsq = sp.tile([P, CG, LTS], BF16, tag="lnsq", name="xsq")
                nc.scalar.activation(xsq[:, :, :tsz], xls,
                                     ACT.Square)
                ps = psum.tile([1, LTS], F32, tag="ln_ps", name="ps")
                ps2 = psum.tile([1, LTS], F32, tag="ln_ps2", name="ps2")
                for g in range(CG):
                    nc.tensor.matmul(
                        ps[:, :tsz], onesb[:], xls[:, g, :],
                        start=(g == 0), stop=(g == CG - 1))
                    nc.tensor.matmul(
                        ps2[:, :tsz], onesb[:], xsq[:, g, :tsz],
                        start=(g == 0), stop=(g == CG - 1))
                nc.vector.tensor_copy(out=ssum[:, t0:t0 + tsz],
                                      in_=ps[:, :tsz])
                nc.vector.tensor_copy(out=ssq[:, t0:t0 + tsz],
                                      in_=ps2[:, :tsz])
            mu = st.tile([1, nrows], BF16, tag="mu", name="mu")
            rstd = st.tile([1, nrows], BF16, tag="rstd", name="rstd")
            var = st.tile([1, nrows], F32, tag="var", name="var")
            musq = st.tile([1, nrows], F32, tag="musq", name="musq")
            nc.vector.tensor_scalar_mul(mu[:], ssum[:], 1.0 / Cc)
            nc.vector.tensor_scalar_mul(var[:], ssq[:], 1.0 / Cc)
            nc.vector.tensor_tensor(musq[:], mu[:], mu[:], ALU.mult)
            nc.vector.tensor_tensor(var[:], var[:], musq[:], ALU.subtract)
            nc.scalar.activation(var[:], var[:], ACT.Ln, bias=epsc[:])
            nc.scalar.activation(rstd[:], var[:], ACT.Exp, scale=-0.5)
            for t0, tsz in _splits(nrows, LTS):
                if sbuf_src:
                    xls = src_v[:, :, t0:t0 + tsz]
                else:
                    xlt = sp.tile([P, CG, LTS], src_dt, tag="xls",
                                  name="xls")
                    nc.sync.dma_start(out=xlt[:, :, :tsz],
                                      in_=src_v[:, :, t0:t0 + tsz])
                    xls = xlt[:, :, :tsz]
                mups = psum.tile([P, LTS], F32, tag="mups", name="mups")
                nc.tensor.matmul(mups[:, :tsz], onesPb[:],
                                 mu[:, t0:t0 + tsz],
                                 start=True, stop=True)
                rsps = psum.tile([P, LTS], F32, tag="rsps", name="rsps")
                nc.tensor.matmul(rsps[:, :tsz], onesPb[:],
                                 rstd[:, t0:t0 + tsz],
                                 start=True, stop=True)
                for g in range(CG):
                    xm = spx.tile([P, LTS], BF16, tag="ln_xm", name="xm")
                    nc.vector.tensor_tensor(xm[:, :tsz], xls[:, g, :],
                                            mups[:, :tsz], ALU.subtract)
                    nc.vector.scalar_tensor_tensor(
                        out_sb[:, g, t0:t0 + tsz], xm[:, :tsz], ccol(g, iw),
                        rsps[:, :tsz], ALU.mult, ALU.mult)
            for p in (psum, spx, sp, st):
                p.release()

        # ================= Phase A: LN1 (h = 16*ln(x), bf16) ============
        pEk = tc.alloc_tile_pool(name="pEk", bufs=1)
        eksb = [pEk.tile([P, RS], BF16, tag=f"eksb{g}", name=f"eksb{g}")
                for g in range(DG)]
        ekvsb = [pEk.tile([P, RS], BF16, tag=f"ekvsb{g}", name=f"ekvsb{g}")
                 for g in range(DG)]
        pMix = tc.alloc_tile_pool(name="pMix", bufs=1)
        mixk8 = [pMix.tile([P, 2, RSP], F8, tag=f"mixk8_{p}",
                           name=f"mixk8_{p}") for p in range(CG // 2)]
        mixv8 = [pMix.tile([P, 2, RSP], F8, tag=f"mixv8_{p}",
                           name=f"mixv8_{p}") for p in range(CG // 2)]
        mixr8 = [pMix.tile([P, 2, RSP], F8, tag=f"mixr8_{p}",
                           name=f"mixr8_{p}") for p in range(CG // 2)]
        pHs = tc.alloc_tile_pool(name="pHs", bufs=1)
        hs = pHs.tile([P, CG, R], BF16, tag="hs")
        ln_stream(xTbv, R, I_LN1W, hs, "ln1", src_bf16=True)
        nc.vector.tensor_scalar_mul(hs[:, :, 0:2], hs[:, :, 0:2], m0[:])

        # ========== Phase B: mixes (fp8 x16) + k/v/r DR matmuls ========
        stg = tc.alloc_tile_pool(name="stg", bufs=4)
        if RSP > RS:
            for mixl in (mixk8, mixv8, mixr8):
                for mt in mixl:
                    nc.vector.memset(mt[:, :, RS:RSP], 0.0)
        MSTRIPS = [(0, 512), (512, RS - 512)]
        for t0, tsz in MSTRIPS:
            for g in range(CG):
                dmix = stg.tile([P, 512 + 1], BF16, tag="dmix", name="dmix")
                nc.vector.tensor_tensor(
                    dmix[:, :tsz], hs[:, g, 1 + t0:1 + t0 + tsz],
                    hs[:, g, t0:t0 + tsz], ALU.subtract)
                for mixl, icoef, on_act in ((mixk8, I_TMK, True),
                                            (mixv8, I_TMV, False),
                                            (mixr8, I_TMR, True)):
                    mb16 = stg.tile([P, 512 + 1], BF16, tag="mb16",
                                    name="mb16")
                    nc.vector.scalar_tensor_tensor(
                        mb16[:, :tsz], dmix[:, :tsz], ccol(g, icoef),
                        hs[:, g, t0:t0 + tsz], ALU.mult, ALU.add)
                    dst = mixl[g // 2][:, g % 2, t0:t0 + tsz]
                    if on_act:
                        nc.scalar.activation(dst, mb16[:, :tsz], ACT.Copy)
                    else:
                        nc.gpsimd.tensor_copy(out=dst, in_=mb16[:, :tsz])
        wpB = tc.alloc_tile_pool(name="wpB", bufs=2)
        stgE = tc.alloc_tile_pool(name="stgE", bufs=4)
        psB = tc.alloc_tile_pool(name="psB", bufs=6, space="PSUM")
        DBLK = 512
        tstripsB = [(0, 512), (512, 512), (1024, RSP - 1024)]

        def mm_dr(whd, wld, rhs8, n_out, evict, wtag, strips=None):
            for d0, dsz in _splits(n_out, DBLK):
                wbh = wpB.tile([P, CG, DBLK], F8, tag="wh", name="wbh")
                nc.sync.dma_start(out=wbh[:, :, :dsz],
                                  in_=whd[:, :, d0:d0 + dsz])
                if wld is not None:
                    wbl = wpB.tile([P, CG, DBLK], F8, tag="wl",
                                   name="wbl")
                    nc.sync.dma_start(out=wbl[:, :, :dsz],
                                      in_=wld[:, :, d0:d0 + dsz])
                wbufs = [wbh] if wld is None else [wbh, wbl]
                for gl in range(dsz // P):
                    g_out = (d0 + gl * P) // P
                    for t0, tsz in (strips or tstripsB):
                        wsz = min(tsz, RS - t0)
                        if wsz <= 0:
                            continue
                        ps = psB.tile([P, TS], F32, tag="mm_ps", name="mm_ps")
                        nmm = len(wbufs) * (CG // 2)
                        i = 0
                        for wb in wbufs:
                            for gp in range(CG // 2):
                                nc.tensor.matmul(
                                    ps[:, :tsz],
                                    wb[:, 2 * gp:2 * gp + 2,
                                       gl * P:(gl + 1) * P],
                                    rhs8[gp][:, :, t0:t0 + tsz],
                                    start=(i == 0), stop=(i == nmm - 1),
                                    perf_mode=DR)
                                i += 1
                        evict(g_out, t0, wsz, ps)

        def evict_k(g, t0, wsz, ps):
            nc.scalar.activation(eksb[g][:, t0:t0 + wsz], ps[:, :wsz],
                                 ACT.Exp, scale=PS_INV)
            if t0 == 0:
                nc.vector.tensor_scalar_mul(eksb[g][:, 0:1], eksb[g][:, 0:1],
                                            m0[:])

        def evict_v(g, t0, wsz, ps):
            nc.vector.scalar_tensor_tensor(
                ekvsb[g][:, t0:t0 + wsz], ps[:, :wsz], PS_INV,
                eksb[g][:, t0:t0 + wsz], ALU.mult, ALU.mult)

        def evict_r(g, t0, wsz, ps):
            srt = stgE.tile([P, TS], BF16, tag="srt", name="srt")
            nc.scalar.activation(srt[:, :wsz], ps[:, :wsz], ACT.Sigmoid,
                                 scale=PS_INV)
            nc.sync.dma_start(out=srdv[:, g, t0:t0 + wsz], in_=srt[:, :wsz])

        mm_dr(wkh, None, mixk8, Dd, evict_k, "wk", strips=tstripsB[:1])
        mm_dr(wkh, None, mixk8, Dd, evict_k, "wk", strips=tstripsB[1:])
        mm_dr(wvh, wvl, mixv8, Dd, evict_v, "wv")
        mm_dr(wrh, None, mixr8, Dd, evict_r, "wr")

        psB.release()
        stgE.release()
        wpB.release()
        stg.release()
        pHs.release()
        pMix.release()

        # ====== Phase C: boundary states (bf16 scans) + AllGather =======
        pRw = tc.alloc_tile_pool(name="pRw", bufs=1, side="right")
        rwkv8 = [pRw.tile([P, 2, RSP], F8, tag=f"rw{p}", name=f"rw{p}")
                 for p in range(DG // 2)]
        if RSP > RS:
            for rwt in rwkv8:
                nc.vector.memset(rwt[:, :, RS:RSP], 0.0)
        wpE = tc.alloc_tile_pool(name="wpE", bufs=2, side="right")
        spE = tc.alloc_tile_pool(name="spE", bufs=2, side="right")
        pC = tc.alloc_tile_pool(name="pC", bufs=2, side="right")
        state = pC.tile([P, 2 * DG], F32, tag="state", name="state")
        for g in range(DG):
            ewbc = ccol(g, I_EW).to_broadcast([P, RS - 1])
            apre = pC.tile([P, RS - 1], BF16, tag="apre", name="apre")
            nc.vector.tensor_tensor_scan(
                apre[:], ewbc, ekvsb[g][:, :RS - 1], 0.0, ALU.mult, ALU.add)
            nc.gpsimd.tensor_copy(out=state[:, g:g + 1],
                                  in_=apre[:, RS - 2:RS - 1])
            bpre = pC.tile([P, RS - 1], BF16, tag="bpre", name="bpre")
            nc.vector.tensor_tensor_scan(
                bpre[:], ewbc, eksb[g][:, :RS - 1], 0.0, ALU.mult, ALU.add)
            nc.gpsimd.tensor_copy(out=state[:, DG + g:DG + g + 1],
                                  in_=bpre[:, RS - 2:RS - 1])
        nc.sync.dma_start(out=cc_in[:], in_=state[:])
        if not no_collective:
            nc.gpsimd.collective_compute(
                "AllGather", ALU.bypass,
                replica_groups=[list(range(n_cores))],
                ins=[cc_in[:].opt()], outs=[cc_out[:].opt()])
        else:
            for jj in range(n_cores):
                nc.sync.dma_start(out=cc_out[jj * P:(jj + 1) * P, :],
                                  in_=cc_in[:])
        gsb = pC.tile([P, n_cores, 2 * DG], F32, tag="gsb", name="gsb")
        nc.sync.dma_start(
            out=gsb[:], in_=cc_out[:].rearrange("(j p) s -> p j s", p=P))
        a0b0 = pC.tile([P, 2 * DG], F32, tag="a0b0", name="a0b0")
        nc.vector.memset(a0b0[:, 0:DG], 0.0)
        nc.vector.memset(a0b0[:, DG:2 * DG], DEN_EPS)
        for j in range(n_cores):
            nc.vector.scalar_tensor_tensor(
                a0b0[:], gsb[:, j, :], selt[:, j:j + 1], a0b0[:],
                ALU.mult, ALU.add)

        # ============ Phase D: WKV scans + rwkv (fp8 x16) ============
        pD = tc.alloc_tile_pool(name="pD", bufs=3)

        def d_front(g):
            ekg = eksb[g][:]
            xkg = ekvsb[g][:]
            srg = pD.tile([P, RS], BF16, tag="srg", name="srg")
            nc.sync.dma_start(out=srg[:], in_=srdv[:, g, :])
            ewb = pD.tile([P, RS], BF16, tag="ewb", name="ewb")
            nc.scalar.activation(ewb[:], ccol(g, I_EW).to_broadcast([P, RS]),
                                 ACT.Copy)
            eub = pD.tile([P, RS], BF16, tag="eub", name="eub")
            nc.scalar.activation(eub[:], ccol(g, I_EU).to_broadcast([P, RS]),
                                 ACT.Copy)
            abuf = pD.tile([P, RS + 1], BF16, tag="abuf", name="abuf")
            nc.gpsimd.tensor_copy(out=abuf[:, 0:1], in_=a0b0[:, g:g + 1])
            nc.vector.tensor_tensor_scan(
                abuf[:, 1:RS + 1], ewb[:], xkg, a0b0[:, g:g + 1],
                ALU.mult, ALU.add)
            bbuf = pD.tile([P, RS + 1], BF16, tag="bbuf", name="bbuf")
            nc.gpsimd.tensor_copy(out=bbuf[:, 0:1],
                                  in_=a0b0[:, DG + g:DG + g + 1])
            nc.vector.tensor_tensor_scan(
                bbuf[:, 1:RS + 1], ewb[:], ekg,
                a0b0[:, DG + g:DG + g + 1], ALU.mult, ALU.add)
            num = pD.tile([P, RS], BF16, tag="num", name="num")
            nc.vector.scalar_tensor_tensor(
                num[:], xkg, ccol(g, I_EU), abuf[:, 0:RS],
                ALU.mult, ALU.add)
            snum = pD.tile([P, RS], BF16, tag="snum", name="snum")
            nc.vector.tensor_tensor(snum[:], num[:], srg[:], ALU.mult)
            t1 = pD.tile([P, RS], BF16, tag="t1", name="t1")
            nc.gpsimd.tensor_tensor(t1[:], ekg, eub[:], ALU.mult)
            den = pD.tile([P, RS], F32, tag="den", name="den")
            nc.gpsimd.tensor_tensor(den[:], t1[:], bbuf[:, 0:RS], ALU.add)
            return snum, den

        def d_back(g, snum, den):
            rden = pD.tile([P, RS], F32, tag="rden", name="rden")
            nc.vector.reciprocal_approx_fast(out=rden[:], in_=den[:])
            nc.vector.scalar_tensor_tensor(
                rwkv8[g // 2][:, g % 2, :RS], snum[:], SA, rden[:],
                ALU.mult, ALU.mult)

        pend = []
        for g in range(DG):
            pend.append((g, d_front(g)))
            if len(pend) > 3:
                gq, fq = pend.pop(0)
                d_back(gq, *fq)
        for gq, fq in pend:
            d_back(gq, *fq)
        pD.release()
        pEk.release()
        pC.release()
        pMx2 = tc.alloc_tile_pool(name="pMx2", bufs=1)
        xk2h = pMx2.tile([P, CG, RO], F8, tag="xk2h")
        xk2l = pMx2.tile([P, CG, RO], F8, tag="xk2l")
        pXr = tc.alloc_tile_pool(name="pXr", bufs=1)
        xr28 = pXr.tile([P, CG, RO], F8, tag="xr28")
        wpG = tc.alloc_tile_pool(name="wpG", bufs=2)
        spG = tc.alloc_tile_pool(name="spG", bufs=2)
        pX2 = tc.alloc_tile_pool(name="pX2", bufs=1)
        x2bf = pX2.tile([P, CG, RS], BF16, tag="x2bf")

        # ========= Phase E: Wo (2t DR) -> x2 = x + attn (DRAM) =========
        psE = tc.alloc_tile_pool(name="psE", bufs=3, space="PSUM")
        CBLK = 512
        for c0, csz in _splits(Cc, CBLK):
            wbh = wpE.tile([P, DG, CBLK], F8, tag="woh", name="woh")
            nc.sync.dma_start(out=wbh[:, :, :csz], in_=woh[:, :, c0:c0 + csz])
            wbl = wpE.tile([P, DG, CBLK], F8, tag="wol", name="wol")
            nc.sync.dma_start(out=wbl[:, :, :csz], in_=wol[:, :, c0:c0 + csz])
            for gl in range(csz // P):
                g_c = (c0 + gl * P) // P
                for t0, tsz in tstripsB:
                    wsz = min(tsz, RS - t0)
                    if wsz <= 0:
                        continue
                    ps = psE.tile([P, TS], F32, tag="wo_ps", name="wo_ps")
                    i = 0
                    for wb in (wbh, wbl):
                        for gp in range(DG // 2):
                            nc.tensor.matmul(
                                ps[:, :tsz],
                                wb[:, 2 * gp:2 * gp + 2, gl * P:(gl + 1) * P],
                                rwkv8[gp][:, :, t0:t0 + tsz],
                                start=(i == 0), stop=(i == DG - 1),
                                perf_mode=DR)
                            i += 1
                    xst = spE.tile([P, TS], F32, tag="xst", name="xst")
                    nc.sync.dma_start(
                        out=xst[:, :wsz],
                        in_=xTv[:, g_c, 1 + t0:1 + t0 + wsz])
                    x2st = spE.tile([P, TS], F32, tag="x2st", name="x2st")
                    nc.vector.scalar_tensor_tensor(
                        x2st[:, :wsz], ps[:, :wsz], PS_INV,
                        xst[:, :wsz], ALU.mult, ALU.add)
                    nc.sync.dma_start(out=x2dv[:, g_c, t0:t0 + wsz],
                                      in_=x2st[:, :wsz])
                    nc.gpsimd.tensor_copy(out=x2bf[:, g_c, t0:t0 + wsz],
                                          in_=x2st[:, :wsz])
        psE.release()
        spE.release()
        wpE.release()
        pRw.release()

        # ====== Phase F: LN2 + mixes2 (xk2 hi/lo fp8, xr2 fp8) ======
        pG2 = tc.alloc_tile_pool(name="pG2", bufs=1)
        g2 = pG2.tile([P, CG, RS], BF16, tag="g2")
        ln_stream(x2bf, RS, I_LN2W, g2, "ln2", sbuf_src=True, lts=512)
        nc.vector.tensor_scalar_mul(g2[:, :, 0:1], g2[:, :, 0:1], m0[:])

        spF = tc.alloc_tile_pool(name="spF", bufs=3)
        for g in range(CG):
            dmix = spF.tile([P, RO], BF16, tag="dmix2", name="dmix2")
            nc.vector.tensor_tensor(dmix[:], g2[:, g, 1:RS],
                                    g2[:, g, 0:RO], ALU.subtract)
            xr2b = spF.tile([P, RO], BF16, tag="xr2b", name="xr2b")
            nc.vector.scalar_tensor_tensor(
                xr2b[:], dmix[:], ccol(g, I_CMR), g2[:, g, 0:RO],
                ALU.mult, ALU.add)
            nc.gpsimd.tensor_copy(out=xr28[:, g, :], in_=xr2b[:])
        for g in range(CG):
            dmix = spF.tile([P, RO], BF16, tag="dmix2", name="dmix2")
            nc.vector.tensor_tensor(dmix[:], g2[:, g, 1:RS],
                                    g2[:, g, 0:RO], ALU.subtract)
            xk2b = spF.tile([P, RO], BF16, tag="xk2b", name="xk2b")
            nc.vector.scalar_tensor_tensor(
                xk2b[:], dmix[:], ccol(g, I_CMK), g2[:, g, 0:RO],
                ALU.mult, ALU.add)
            nc.scalar.activation(xk2h[:, g, :], xk2b[:], ACT.Copy)
            dif = spF.tile([P, RO], BF16, tag="dif", name="dif")
            nc.vector.tensor_tensor(dif[:], xk2b[:], xk2h[:, g, :],
                                    ALU.subtract)
            nc.scalar.activation(xk2l[:, g, :], dif[:], ACT.Copy)
        spF.release()
        pG2.release()
        pX2.release()

        # ====== Phase G: r2 = sigmoid(xr2 @ WcrT) (pure DR) -> DRAM =====
        psG = tc.alloc_tile_pool(name="psG", bufs=4, space="PSUM")
        for c0, csz in _splits(Cc, CBLK):
            wbh = wpG.tile([P, CG, CBLK], F8, tag="wcr", name="wcr")
            nc.sync.dma_start(out=wbh[:, :, :csz], in_=wcrh[:, :, c0:c0 + csz])
            for gl in range(csz // P):
                g_c = (c0 + gl * P) // P
                for t0, tsz in _splits(RO, TS):
                    ps = psG.tile([P, TS], F32, tag="wcr_ps", name="wcr_ps")
                    for gp in range(CG // 2):
                        nc.tensor.matmul(
                            ps[:, :tsz],
                            wbh[:, 2 * gp:2 * gp + 2, gl * P:(gl + 1) * P],
                            xr28[:, 2 * gp:2 * gp + 2, t0:t0 + tsz],
                            start=(gp == 0), stop=(gp == CG // 2 - 1),
                            perf_mode=DR)
                    sgt = spG.tile([P, TS], BF16, tag="sgt", name="sgt")
                    nc.scalar.activation(sgt[:, :tsz], ps[:, :tsz],
                                         ACT.Sigmoid, scale=PS_INV)
                    nc.sync.dma_start(out=sgdv[:, g_c, t0:t0 + tsz],
                                      in_=sgt[:, :tsz])
        psG.release()
        spG.release()
        wpG.release()
        pXr.release()

        # ============ Phase H: FFN (3t DR both matmuls) ============
        FBLK = 512
        FQ = 16
        for t0, tsz in _splits(RO, TS):
            pH = tc.alloc_tile_pool(name=f"pH{t0}", bufs=1)
            sH = tc.alloc_tile_pool(name=f"sH{t0}", bufs=2)
            wpH = tc.alloc_tile_pool(name=f"wpH{t0}", bufs=3)
            psH = tc.alloc_tile_pool(name=f"psH{t0}", bufs=4, space="PSUM")
            psKV = tc.alloc_tile_pool(name=f"psKV{t0}", bufs=1, space="PSUM")
            kf8 = pH.tile([P, FG, TS], F8, tag="kf8", name="kf8")
            # FFN1 3t: z = Wckh@(xh+xl) + Wckl@xh; trl = sqrt(8)*relu(z)
            for f0, fsz in _splits(Ff, FBLK):
                wbh = wpH.tile([P, CG, FBLK], F8, tag="wfh", name="wfh")
                nc.sync.dma_start(out=wbh[:, :, :fsz],
                                  in_=wckh[:, :, f0:f0 + fsz])
                wbl = wpH.tile([P, CG, FBLK], F8, tag="wfl", name="wfl")
                nc.sync.dma_start(out=wbl[:, :, :fsz],
                                  in_=wckl[:, :, f0:f0 + fsz])
                ngl = fsz // P
                trl = sH.tile([P, ngl, TS], BF16, tag="trl", name="trl")
                for fl in range(ngl):
                    ps = psH.tile([P, TS], F32, tag="ffn1_ps", name="ffn1_ps")
                    i = 0
                    nmm = 3 * (CG // 2)
                    for wb, act in ((wbh, xk2h), (wbh, xk2l), (wbl, xk2h)):
                        for gp in range(CG // 2):
                            nc.tensor.matmul(
                                ps[:, :tsz],
                                wb[:, 2 * gp:2 * gp + 2, fl * P:(fl + 1) * P],
                                act[:, 2 * gp:2 * gp + 2, t0:t0 + tsz],
                                start=(i == 0), stop=(i == nmm - 1),
                                perf_mode=DR)
                            i += 1
                    nc.scalar.activation(trl[:, fl, :tsz], ps[:, :tsz],
                                         ACT.Relu, scale=PS_INV * SQ8)
                # kf8 = e4m3(trl^2) = e4m3(8*kf) in one ACT Square
                g_f0 = f0 // P
                nc.scalar.activation(kf8[:, g_f0:g_f0 + ngl, :tsz],
                                     trl[:, :, :tsz], ACT.Square)
            # FFN2 3t + final: out = x2 + sg*((Wcvh@(kf8+kflo)+Wcvl@kf8)/512)
            for c0, csz in _splits(Cc, CBLK):
                kvps = [psKV.tile([P, TS], F32, tag=f"kv_ps{i}",
                                  name=f"kv_ps{i}")
                        for i in range(csz // P)]
                nq = FG // FQ
                nmm_tot = nq * 2 * (FQ // 2)
                mm_idx = [0] * (csz // P)
                for q in range(nq):
                    f_lo = q * FQ
                    wbh = wpH.tile([P, FQ, CBLK], F8, tag="wf2h", name="wf2h")
                    nc.sync.dma_start(
                        out=wbh[:, :, :csz],
                        in_=wcvh[:, f_lo:f_lo + FQ, c0:c0 + csz])
                    wbl = wpH.tile([P, FQ, CBLK], F8, tag="wf2l", name="wf2l")
                    nc.sync.dma_start(
                        out=wbl[:, :, :csz],
                        in_=wcvl[:, f_lo:f_lo + FQ, c0:c0 + csz])
                    for gl in range(csz // P):
                        for wb, act in ((wbh, kf8), (wbl, kf8)):
                            for fp in range(FQ // 2):
                                fg = f_lo + 2 * fp
                                nc.tensor.matmul(
                                    kvps[gl][:, :tsz],
                                    wb[:, 2 * fp:2 * fp + 2,
                                       gl * P:(gl + 1) * P],
                                    act[:, fg:fg + 2, :tsz],
                                    start=(mm_idx[gl] == 0),
                                    stop=(mm_idx[gl] == nmm_tot - 1),
                                    perf_mode=DR)
                                mm_idx[gl] += 1
                for gl in range(csz // P):
                    g_c = (c0 + gl * P) // P
                    sgs = wpH.tile([P, TS], BF16, tag="sgs", name="sgs")
                    nc.sync.dma_start(out=sgs[:, :tsz],
                                      in_=sgdv[:, g_c, t0:t0 + tsz])
                    ot = wpH.tile([P, TS], BF16, tag="ot", name="ot")
                    nc.vector.scalar_tensor_tensor(
                        ot[:, :tsz], kvps[gl][:, :tsz], 1.0 / (SKF * SW),
                        sgs[:, :tsz], ALU.mult, ALU.mult)
                    x2s = wpH.tile([P, TS], F32, tag="x2s", name="x2s")
                    nc.sync.dma_start(
                        out=x2s[:, :tsz],
                        in_=x2dv[:, g_c, 1 + t0:1 + t0 + tsz])
                    o2 = wpH.tile([P, TS], F32, tag="o2", name="o2")
                    nc.vector.tensor_tensor(o2[:, :tsz], ot[:, :tsz],
                                            x2s[:, :tsz], ALU.add)
                    nc.sync.dma_start(out=outTv[:, g_c, t0:t0 + tsz],
                                      in_=o2[:, :tsz])
            for p in (psKV, psH, wpH, sH, pH):
                p.release()
        pMx2.release()
        dram.release()
        const.release()

    nc.compile()
    return nc


_PROGRAM_CACHE = {}


def _get_program(key, **kw):
    if key not in _PROGRAM_CACHE:
        _PROGRAM_CACHE[key] = build_program(**kw)
    return _PROGRAM_CACHE[key]


def _q8pair(wT_scaled):
    """fp32 [128, KG, N] (already x SW) -> (hi, lo) e4m3 at the same scale."""
    hi = wT_scaled.astype(E4M3)
    lo = (wT_scaled - hi.astype(np.float32)).astype(E4M3)
    return hi, lo


def _host_prep(inputs, Cc=C, Dd=D_ATT, Ff=D_FFN, Bb=B, Tt=T, n_cores=N_CORES):
    P = 128
    CG, DG, FG = Cc // P, Dd // P, Ff // P
    half = Tt // 2
    RO, RS, R = half, half + 1, half + 2

    f = {k: np.asarray(v, np.float32) for k, v in inputs.items()}
    x = f["x"]

    def swz(wT, kg):  # [K, N] fp32 -> [128, kg, N] * SW
        Kdim, Ndim = wT.shape
        return np.ascontiguousarray(
            wT.reshape(kg, P, Ndim).transpose(1, 0, 2)) * SW

    wkh_, _ = _q8pair(swz(f["Wk"].T, CG))
    wvh_, wvl_ = _q8pair(swz(f["Wv"].T, CG))
    wrh_, _ = _q8pair(swz(f["Wr"].T, CG))
    woh_, wol_ = _q8pair(swz(f["Wo"].T, DG))
    wckh_, wckl_ = _q8pair(swz(f["Wck"].T, CG))
    wcvh_, wcvl_ = _q8pair(swz(f["Wcv"].T, FG))
    wcrh_, _ = _q8pair(swz(f["Wcr"].T, CG))

    def col(v):
        return np.ascontiguousarray(
            np.asarray(v, np.float32).reshape(-1).reshape(CG, P).T)

    ew = np.exp(-np.exp(f["time_decay"].astype(np.float64)))
    cvec_h = np.stack([
        col(f["ln1_w"] * SA), col(f["ln1_b"]),
        col(f["tm_k"]), col(f["tm_v"]), col(f["tm_r"]),
        col(ew.astype(np.float32)), col(np.exp(f["time_first"])),
        col(f["ln2_w"] * SA), col(f["ln2_b"]),
        col(f["cm_k"]), col(f["cm_r"]),
    ], axis=-1).astype(np.float32)

    in_maps = []
    for core in range(n_cores):
        b, hh = core // 2, core % 2
        t0 = hh * half
        xr = np.zeros((R, Cc), np.float32)
        lo = t0 - 2
        src_lo = max(lo, 0)
        xr[src_lo - lo:, :] = x[b, src_lo:t0 + RO, :]
        m0 = np.full((P, 1), float(hh), np.float32)
        sel = np.zeros((P, n_cores), np.float32)
        if hh == 1:
            sel[:, core - 1] = 1.0
        xrt = np.ascontiguousarray(xr.T)
        in_maps.append({
            "xT": xrt, "xTb": xrt.astype(ml_dtypes.bfloat16),
            "wkh": wkh_, "wvh": wvh_, "wvl": wvl_, "wrh": wrh_,
            "woh": woh_, "wol": wol_, "wckh": wckh_, "wckl": wckl_,
            "wcvh": wcvh_, "wcvl": wcvl_, "wcrh": wcrh_,
            "cvec": cvec_h, "m0": m0, "sel": sel,
        })
    return in_maps


def kernel(**inputs):
    in_maps = _host_prep(inputs)
    nc = _get_program("full")
    res = run_bass_kernel_spmd(nc, in_maps, core_ids=list(range(N_CORES)))
    half = T // 2
    out = np.empty((B, T, C), np.float32)
    for core in range(N_CORES):
        b, hh = core // 2, core % 2
        out[b, hh * half:(hh + 1) * half, :] = res.results[core]["outT"].T
    return out
